# revision 53
# baseline (speedup 1.0000x reference)
"""Causal self-attention (B=4, T=2048, D=1024, H=16, hd=64) on 8 trn2 cores.

Sharding: 4-way data parallel over batch x 2-way tensor parallel over heads.
Core c handles batch c//2 and heads [8*(c%2), 8*(c%2)+8). Each core computes
its heads' partial contribution to the out-projection (a full [T, D] tensor);
the host sums the two head-group partials per batch and adds b_out.

Host-side prep (layout only): x is transposed to xT [D, T] and converted to
bf16, w_qkv is regrouped per core and converted to bf16, w_out likewise.
End-to-end rel err vs the fp32 reference is ~5e-3 (gate: 2e-2).

Per-core dataflow (single NeuronCore, Tile framework):
  1. xT streams straight from DRAM into SBUF (no on-device transposes).
  2. qT/kT [hd,T] are computed head-PAIR-stacked ([128,T] per pair) with
     w_qkv chunks as stationary; V [T,hd] per head is computed naturally and
     scattered into v1 (64 cols per (head, k-tile), bf16).
  3. Scores are computed TRANSPOSED: S^T[k,q] = K @ Q^T via two K=64 matmuls
     per (pair, k-tile) into one [128, 2, 512] two-bank PSUM tile; ONE wide
     exp (ACT, scale folded in) produces P^T for both heads in bf16.
     Causality: k-tiles above the diagonal are skipped, diagonal blocks get
     an affine_select zero-mask after exp, partial k-tiles use narrower
     matmuls.
  4. AV runs NON-transposed: per (head, q-subtile 128) the P^T block is the
     STATIONARY operand and V [128,64] streams, so each matmul moves 64
     rows instead of 512. All 8 (head, subtile) accumulators of a pair pack
     into ONE psum bank (zero-on-first-touch after a single start=True).
     Softmax sums ride as 1-column matmuls (P^T stationary, ones moving)
     into a shared, DVE-zeroed sums bank; AV emission trails its exp by 8
     k-tile units (carry) so the PE never waits on ACT latency.
  5. Normalization is a DVE broadcast multiply (reciprocal of sums, then
     attn * recip -> bf16). The normalized attn [q, 512] is PE-transposed
     (bf16) into the out-projection's stationary operand; out = attnT.T @
     w_out accumulates over 4 hd-chunks. All out-projections are deferred
     into the last q-group's phase, which is otherwise exp(ACT)-bound.

Scheduling: emission-level software pipelining interleaves attention for
q-group g with the projection of group g+1. PSUM (8 banks) is budgeted as
4 scores + 2 AV + 1 sums + 1 projection scratch; in the prologue and final
drain the idle scores/AV banks widen the projection scratch rotation to 5.
A few dummy matmuls at t=0 cover the first DMA latency and finish the PE
p-state ramp before real work lands.
"""

import numpy as np

import concourse.bass as bass
import concourse.mybir as mybir
import concourse.tile as tile
from concourse import bacc
from concourse.bass_utils import run_bass_kernel_spmd
from concourse.masks import make_identity

B, T, D = 4, 2048, 1024
H, HD = 16, 64
HPC = 8              # heads per core
PAIRS = HPC // 2
CH = D // 128        # K-chunks for the QKV projection
TG = 512             # T-group / q-group width
SCALE = 1.0 / 8.0    # 1/sqrt(HD)
NDUMMY = 18          # prologue PE-warm dummy matmuls

F32 = mybir.dt.float32
F32R = mybir.dt.float32r
BF16 = mybir.dt.bfloat16
FP8 = mybir.dt.float8e4
DR = mybir.MatmulPerfMode.DoubleRow
AF = mybir.ActivationFunctionType

# w_qkv and w_out are scaled by 2**5 on the host so their fp8(e4m3) hi/lo
# splits stay out of the subnormal range; q', k', v' come out 32x larger, so
# the softmax scale absorbs 1/(32*32) and the device output is 32*32x the
# true projection (divided back out on the host).
WSCALE = 32.0
OUT_SCALE = 1.0 / (WSCALE * WSCALE)


def _r(ap):
    return ap.bitcast(F32R)


def build_tile_program(tc, xh, xl, wqh, wql, wouth, woutl, out, bqkv=None,
                       seq_len=T):
    """Emit the per-core program. seq_len is parametrized for small-scale
    simulation tests; the real kernel uses seq_len=T=2048.

    Emission is software-pipelined: attention for q-group g (latency-bound
    serial chains sT -> exp -> mask -> AV) is interleaved at emission level
    with the transposes + QKV projection of group g+1 (dense PE work), so the
    in-order PE fills attention's dependency stalls with projection matmuls.
    """
    nc = tc.nc
    n_tg = seq_len // TG
    n_tt = seq_len // 128
    with_bias = bqkv is not None

    from contextlib import ExitStack
    with ExitStack() as ctx:
        const = ctx.enter_context(tc.tile_pool(name="const", bufs=1))
        wpool = ctx.enter_context(tc.tile_pool(name="wpool", bufs=1))
        xt_pool = ctx.enter_context(tc.tile_pool(name="xt", bufs=4))
        qt_pool = ctx.enter_context(tc.tile_pool(name="qt", bufs=2 * PAIRS))
        pt_pool = ctx.enter_context(tc.tile_pool(name="pt", bufs=14))
        an_pool = ctx.enter_context(tc.tile_pool(name="an", bufs=16))
        at_pool = ctx.enter_context(tc.tile_pool(name="at", bufs=16))
        ob_pool = ctx.enter_context(tc.tile_pool(name="ob", bufs=6))
        rc_pool = ctx.enter_context(tc.tile_pool(name="rc", bufs=2))
        mm_ps = ctx.enter_context(tc.tile_pool(name="mmps", bufs=1, space="PSUM"))
        st_ps = ctx.enter_context(tc.tile_pool(name="stps", bufs=2, space="PSUM"))
        av_ps = ctx.enter_context(tc.tile_pool(name="avps", bufs=2, space="PSUM"))
        sums_ps = ctx.enter_context(tc.tile_pool(name="sums", bufs=1, space="PSUM"))
        identity = const.tile([128, 128], F32, tag="ident")
        ident_bf = const.tile([128, 128], BF16, tag="identbf")
        ones_bf = const.tile([128, 1], BF16, tag="ones_bf")

        def make_consts():
            # deferred so the weight SWDGE generation isn't queued behind
            # make_identity on the Pool engine at t=0; the identity is only
            # needed by the (late) attnT transposes
            make_identity(nc, identity[:])
            nc.vector.tensor_copy(out=ident_bf[:], in_=identity[:])
            nc.vector.memset(ones_bf[:], 1.0)
        if with_bias:
            ones_f = const.tile([128, 64], F32, tag="ones_f")
            nc.vector.memset(ones_f[:], 1.0)
            b_sb = const.tile([1, 3 * HPC * HD], F32R, tag="bias")
            nc.sync.dma_start(out=b_sb[:], in_=bqkv.bitcast(F32R)[:])
            ones_row = const.tile([1, TG], F32R, tag="ones_row")
            nc.vector.tensor_copy(
                out=ones_row[:], in_=ones_f[0:1, 0:1].broadcast_to([1, TG]))

        # softmax sums: one persistent bank; cols = parity*32 + pair*8 + h*4+s
        sums = sums_ps.tile([128, 64], F32, tag="sums")

        # resident weights (fp8 hi/lo pairs); w_out pair-packed: pair p rows
        # [128p, 128p+128). The DMAs are deferred until after group 0's x
        # loads so the first transposes aren't queued behind the weight
        # traffic.
        wh_sb = wpool.tile([128, CH, 3 * HPC * HD], FP8, tag="wqh")
        wl_sb = wpool.tile([128, CH, 3 * HPC * HD], FP8, tag="wql")
        # wout fp8 hi/lo, chunk-PAIR packed for DoubleRow: tile cp holds rows
        # [256cp, 256cp+256) as [128, 2, D]
        wouth_sb = [wpool.tile([128, 2, D], FP8, tag=f"wouth{p}",
                               name=f"wouth{p}") for p in range(2)]
        woutl_sb = [wpool.tile([128, 2, D], FP8, tag=f"woutl{p}",
                               name=f"woutl{p}") for p in range(2)]

        def load_weights():
            # The cost model serializes ALL transfers through one DMA_ENGINES
            # lane (~bytes/360GB/s each), fed by two descriptor-gen lanes:
            # HWDGE (shared SP+ACT, 625ns/dma) and SWDGE (gpsimd, ~1.1us/dma).
            # The prologue is transfer-order-bound and the PE is in-order, so
            # transfers are issued in exact consumption order of the prologue
            # units (q chains h0/h1, k chains h0/h1, then V), with q/k weight
            # columns split so q unblocks first. All on the scalar HWDGE
            # queue; x rides the sync HWDGE queue; later-group x and wout
            # ride SWDGE.
            for c0 in (0, 4):
                for w_sb, wsrc in ((wh_sb, wqh), (wl_sb, wql)):
                    nc.scalar.dma_start(
                        out=w_sb[:, c0:c0 + 4, 0:1024],
                        in_=wsrc[128 * c0:128 * (c0 + 4), 0:1024].rearrange(
                            "(c p) n -> p c n", p=128))
            for w_sb, wsrc in ((wh_sb, wqh), (wl_sb, wql)):
                for c0 in range(0, CH, 4):
                    nc.scalar.dma_start(
                        out=w_sb[:, c0:c0 + 4, 1024:1536],
                        in_=wsrc[128 * c0:128 * (c0 + 4), 1024:1536].rearrange(
                            "(c p) n -> p c n", p=128))

        def load_wout():
            # deferred into the phase-0 fill (behind the group-1 x loads on
            # SWDGE): first consumed by the deferred group-0 out-projections
            # in phase 1 (~45us in)
            with tc.tile_wait_until(0.016):
                for cp in range(2):
                    for w_sb, wsrc in ((wouth_sb, wouth), (woutl_sb, woutl)):
                        nc.gpsimd.dma_start(
                            out=w_sb[cp][:],
                            in_=wsrc[256 * cp:256 * (cp + 1), :].rearrange(
                                "(c p) n -> p c n", p=128))
        # kT per pair, head 2p on partitions [0:64), head 2p+1 on [64:128)
        kts = [wpool.tile([128, seq_len], BF16, tag=f"kt{p}", name=f"kt{p}")
               for p in range(PAIRS)]
        # V in bf16: per (head, k-tile) a [128, 64] stationary block
        v1 = wpool.tile([128, HPC, n_tt, HD], BF16, tag="v1")

        qts_of = {}  # g -> [qt tiles per pair]

        # psum scratch for the projection/transpose/oproj chains. In steady
        # state only the mm bank is free; in the prologue and final drain the
        # scores/AV banks are idle, so rotate through them too (the st slot is
        # [128,2,512]; its first bank is used as a [128,512] scratch).
        # "banked" mode (prologue only, F32) additionally uses the st slots'
        # SECOND psum banks (zero regions are 2KB, so the two banks of one
        # slot hold independent accumulation groups), giving 7 concurrent
        # scratch accumulators for the q+k chain overlap.
        _ps_state = {"wide": False, "banked": False, "i": 0, "st": {}}

        def set_wide_scratch(wide, banked=False):
            _ps_state["wide"] = wide
            _ps_state["banked"] = banked

        def scratch_ps(dtype):
            if not _ps_state["wide"]:
                return mm_ps.tile([128, 512], dtype, tag="mm", name="mm")
            if _ps_state["banked"] and dtype == F32:
                i = _ps_state["i"] = (_ps_state["i"] + 1) % 7
                if i == 0:
                    return mm_ps.tile([128, 512], dtype, tag="mm", name="mm")
                if i in (1, 2, 3, 4):
                    buf, bank = divmod(i - 1, 2)
                    if bank == 0:
                        _ps_state["st"][buf] = st_ps.tile(
                            [128, 2, 512], dtype, tag="st", name="stx")
                    return _ps_state["st"][buf][:, bank, :]
                return av_ps.tile([128, 512], dtype, tag="av", name="avx")
            i = _ps_state["i"] = (_ps_state["i"] + 1) % 5
            if i == 0:
                return mm_ps.tile([128, 512], dtype, tag="mm", name="mm")
            if i in (1, 2):
                return st_ps.tile([128, 512], dtype, tag="st", name="stx")
            return av_ps.tile([128, 512], dtype, tag="av", name="avx")

        def transpose_units(g):
            """x arrives pre-transposed and fp8-hi/lo-split from the host:
            just DMA the group's xT columns. Group 0 (prologue, latency-
            critical) goes on the sync HWDGE queue split in half-chunks;
            later groups ride the then-idle SWDGE ring as one transfer per
            hi/lo half."""
            xt = (xt_pool.tile([128, CH, TG], FP8, tag="xth", name=f"xth{g}"),
                  xt_pool.tile([128, CH, TG], FP8, tag="xtl", name=f"xtl{g}"))
            gsl = slice(g * TG, (g + 1) * TG)

            def u():
                if g == 0:
                    # lo directly after hi per chunk-half: every chain's
                    # third (lo*hi) term needs x-lo early
                    for c0 in (0, 4):
                        for t, src in ((xt[0], xh), (xt[1], xl)):
                            nc.sync.dma_start(
                                out=t[:, c0:c0 + 4, :],
                                in_=src[128 * c0:128 * (c0 + 4), gsl].rearrange(
                                    "(c p) n -> p c n", p=128))
                else:
                    # wait-ts biases the tile scheduler so these preps don't
                    # jump the serial DMA lane ahead of the prologue-critical
                    # weight transfers
                    with tc.tile_wait_until(0.010 + 0.038 * (g - 1)):
                        for t, src in ((xt[0], xh), (xt[1], xl)):
                            nc.gpsimd.dma_start(
                                out=t[:, :, :],
                                in_=src[:, gsl].rearrange(
                                    "(c p) n -> p c n", p=128))
            return xt, [u]

        def qkv_units(g, xt):
            """12 units: 4 qt chains, 4 kT chains, 4 V chains."""
            qts = qts_of.setdefault(g, [])
            units = []

            half = {}

            xth, xtl = xt

            def qk_chain(p, qk, h):
                # split into two half-chains (finer interleave granularity).
                # Each half contracts 4 D-chunks = 2 fp8 DoubleRow chunk-pairs
                # x 3 hi/lo cross terms (lo*lo dropped). Terms are ordered
                # hi*hi first so the prologue chains can start as soon as the
                # hi transfers land (lo halves trickle in later).
                if h == 0:
                    half[(p, qk)] = scratch_ps(F32)
                ps = half[(p, qk)]
                col = qk * 512 + p * 128
                first = True
                for wt, xt_ in ((wh_sb, xth), (wl_sb, xth), (wh_sb, xtl)):
                    for c0 in range(4 * h, 4 * h + 4, 2):
                        cs = slice(c0, c0 + 2)
                        nc.tensor.matmul(
                            ps[:, :TG], wt[:, cs, col:col + 128],
                            xt_[:, cs, :],
                            start=(h == 0 and first),
                            stop=(h == 1 and c0 == CH - 2 and xt_ is xtl
                                  and not with_bias),
                            perf_mode=DR)
                        first = False
                if h == 0:
                    return
                if with_bias:
                    nc.tensor.matmul(
                        ps[:, :TG], b_sb[0:1, col:col + 128],
                        ones_row[0:1, :], start=False, stop=True)
                if qk == 0:
                    qt = qt_pool.tile([128, TG], BF16, tag="qt")
                    nc.vector.tensor_copy(out=qt[:], in_=ps[:, :TG])
                    qts.append(qt)
                else:
                    nc.vector.tensor_copy(
                        out=kts[p][:, g * TG:(g + 1) * TG], in_=ps[:, :TG])

            def v_chain(t4):
                tt = g * (TG // 128) + t4
                ps = scratch_ps(F32)
                first = True
                tsl = slice(128 * t4, 128 * (t4 + 1))
                for xt_, wt in ((xth, wh_sb), (xth, wl_sb), (xtl, wh_sb)):
                    for c0 in range(0, CH, 2):
                        cs = slice(c0, c0 + 2)
                        nc.tensor.matmul(
                            ps[:, :512], xt_[:, cs, tsl],
                            wt[:, cs, 1024:1536],
                            start=first,
                            stop=(c0 == CH - 2 and xt_ is xtl
                                  and not with_bias),
                            perf_mode=DR)
                        first = False
                if with_bias:
                    nc.tensor.matmul(
                        ps[:, :512], ones_row[0:1, 0:128],
                        b_sb[0:1, 1024:1536], start=False, stop=True)
                nc.vector.tensor_copy(
                    out=v1[:, :, tt, :],
                    in_=ps[:, :512].rearrange("p (h d) -> p h d", h=HPC))

            k_units = []
            v_units = []
            # in the prologue the qk chains are gated on their W chunks
            # landing (~1us apart): run all h=0 half-chains (chunks 0-3)
            # before any h=1 (chunks 4-7). Requires 4 concurrent psum
            # accumulators - only legal in the wide-scratch prologue.
            hmajor = _ps_state["wide"]
            for lst, qk in ((units, 0), (k_units, 1)):
                if hmajor:
                    for h in range(2):
                        for p in range(PAIRS):
                            lst.append(lambda p=p, h=h, qk=qk: qk_chain(p, qk, h))
                else:
                    for p in range(PAIRS):
                        lst.append(lambda p=p, qk=qk: qk_chain(p, qk, 0))
                        lst.append(lambda p=p, qk=qk: qk_chain(p, qk, 1))
            for t4 in range(TG // 128):
                v_units.append(lambda t4=t4: v_chain(t4))
            return units, k_units, v_units

        def attention_units(g):
            """Per pair: one unit per k-tile (sT+exp+mask, AV carried by one),
            then a normalization unit; finally the out-projection units."""
            units = []
            pending_finish = []
            an_tiles = [an_pool.tile([128, 512], BF16, tag="an",
                                     name=f"an_g{g}_s{s}")
                        for s in range(TG // 128)]
            # zero this group's sums columns once (all 4 pairs' 8-col slices)
            nc.vector.memset(sums[:, (g % 2) * 32:(g % 2) * 32 + 32], 0.0)
            qts = qts_of[g]
            for p in range(PAIRS):
                nkt = 4 * (g + 1)
                soff = (g % 2) * 32 + p * 8
                state = {}

                def start_pair(p=p, state=state, soff=soff):
                    state["av"] = av_ps.tile([128, 512], F32, tag="av",
                                             name=f"av_g{g}_p{p}")
                    state["carry"] = []
                    state["first"] = True

                def kt_unit(kt, pos, p=p, state=state, nkt=nkt,
                            sp=start_pair):
                    if pos == 0:
                        sp()
                    # pop the carried AV FIRST: it is ready now, and the
                    # scores below may head-block on a busy st slot
                    if len(state["carry"]) >= 8:
                        state["emit_av"](*state["carry"].pop(0))
                    qt = qts[p]
                    rdiag = kt - 4 * g
                    col0 = 128 * rdiag if rdiag > 0 else 0
                    ksl = slice(128 * kt, 128 * (kt + 1))
                    st = st_ps.tile([128, 2, 512], F32, tag="st")
                    nc.tensor.matmul(
                        st[:, 0, col0:], kts[p][0:64, ksl], qt[0:64, col0:])
                    nc.tensor.matmul(
                        st[:, 1, col0:], kts[p][64:128, ksl], qt[64:128, col0:])
                    pt = pt_pool.tile([128, 2, 512], BF16, tag="pt")
                    nc.scalar.activation(
                        pt[:, :, col0:], st[:, :, col0:], AF.Exp,
                        scale=SCALE / (WSCALE * WSCALE))
                    if rdiag >= 0:
                        for h in (0, 1):
                            # keep P^T[k, q] only where q >= k (within-block)
                            nc.gpsimd.affine_select(
                                out=pt[:, h, col0:col0 + 128],
                                in_=pt[:, h, col0:col0 + 128],
                                compare_op=mybir.AluOpType.is_ge,
                                fill=0.0, base=0, pattern=[[1, 128]],
                                channel_multiplier=-1)
                    state["carry"].append((kt, pos, rdiag, pt))

                def emit_av(kt, pos, rdiag, pt, p=p, state=state, nkt=nkt,
                            soff=soff):
                    av = state["av"]
                    s0 = max(rdiag, 0)
                    last = pos == nkt - 1
                    for s in range(s0, TG // 128):
                        for h in (0, 1):
                            nc.tensor.matmul(
                                av[:, 128 * s + 64 * h:128 * s + 64 * h + 64],
                                pt[:, h, 128 * s:128 * (s + 1)],
                                v1[:, 2 * p + h, kt, :],
                                start=state["first"],
                                stop=(last and s == 3 and h == 1))
                            state["first"] = False
                            nc.tensor.matmul(
                                sums[:, soff + 4 * h + s:soff + 4 * h + s + 1],
                                pt[:, h, 128 * s:128 * (s + 1)],
                                ones_bf[:],
                                start=False, stop=False,
                                skip_group_check=True)

                state["emit_av"] = emit_av

                def emit_block(mm, state, p, soff, stop_last):
                    av = state["av"]
                    for i, (isd, kt, pt, s, h) in enumerate(mm):
                        nc.tensor.matmul(
                            av[:, 128 * s + 64 * h:128 * s + 64 * h + 64],
                            pt[:, h, 128 * s:128 * (s + 1)],
                            v1[:, 2 * p + h, kt, :],
                            start=state["first"],
                            stop=(stop_last and i == len(mm) - 1))
                        state["first"] = False
                        nc.tensor.matmul(
                            sums[:, soff + 4 * h + s:soff + 4 * h + s + 1],
                            pt[:, h, 128 * s:128 * (s + 1)],
                            ones_bf[:],
                            start=False, stop=False,
                            skip_group_check=True)

                def flush_unit(p=p, state=state, soff=soff):
                    # flush the carried non-diagonal AVs at pair end; the
                    # affine-masked diagonal blocks wait for Pool latency, so
                    # they are deferred into finish_unit (emitted two k-tile
                    # units into the NEXT pair) to avoid head-blocking the
                    # PE's 4-deep dependency wait queue
                    mm = []
                    for kt, pos, rdiag, pt in state["carry"]:
                        for s in range(max(rdiag, 0), TG // 128):
                            for h in (0, 1):
                                mm.append((s == rdiag, kt, pt, s, h))
                    state["carry"] = []
                    mm.sort(key=lambda t: t[0])
                    ndiag = sum(1 for t in mm if t[0])
                    split = len(mm) - ndiag
                    emit_block(mm[:split], state, p, soff, stop_last=False)
                    state["diag"] = mm[split:]

                def norm_unit(p=p, state=state, soff=soff):
                    emit_block(state.pop("diag"), state, p, soff,
                               stop_last=True)
                    av = state["av"]
                    rc = rc_pool.tile([128, 2, 4], F32, tag="rc")
                    nc.vector.reciprocal(
                        rc[:], sums[:, soff:soff + 8].rearrange(
                            "p (h s) -> p h s", h=2))
                    for s in range(TG // 128):
                        nc.vector.tensor_mul(
                            an_tiles[s][:, 128 * p:128 * (p + 1)].rearrange(
                                "p (h d) -> p h d", h=2),
                            av[:, 128 * s:128 * (s + 1)].rearrange(
                                "p (h d) -> p h d", h=2),
                            rc[:, :, s:s + 1].broadcast_to([128, 2, 64]))

                pair_units = []
                for pos, kt in enumerate(range(nkt)):
                    pair_units.append(
                        lambda kt=kt, pos=pos, f=kt_unit: f(kt, pos))
                if pending_finish:
                    pair_units.insert(min(6, max(2, len(pair_units) - 2)),
                                      pending_finish.pop())
                units += pair_units
                units.append(flush_unit)
                pending_finish.append(norm_unit)

            if pending_finish:
                units.append(pending_finish.pop())

            ats = {}

            def trans_unit(s):
                ps = scratch_ps(BF16)
                for c in range(PAIRS):
                    nc.tensor.transpose(
                        ps[:, 128 * c:128 * (c + 1)],
                        an_tiles[s][:, 128 * c:128 * (c + 1)],
                        ident_bf[:])
                # fp8 hi/lo split of attnT for the DoubleRow out-projection.
                # The hi copy runs on the (mostly idle) Pool engine so the
                # DVE isn't the serial bottleneck of the projection drain.
                ath = at_pool.tile([128, 512], FP8, tag="ath")
                atl = at_pool.tile([128, 512], FP8, tag="atl")
                if _ps_state["wide"]:
                    # final drain: ACT is idle and DVE is the serial
                    # bottleneck, so split the quant pair across both
                    nc.scalar.copy(ath[:], ps[:])
                else:
                    nc.vector.tensor_copy(out=ath[:], in_=ps[:])
                nc.vector.tensor_sub(atl[:], ps[:], ath[:])
                ats[s] = (ath, atl)

            def oproj_unit(s, nh):
                row0 = g * TG + 128 * s
                ath, atl = ats[s]
                ps = scratch_ps(F32)
                nsl = slice(512 * nh, 512 * (nh + 1))
                first = True
                for at_, wo in ((ath, wouth_sb), (ath, woutl_sb),
                                (atl, wouth_sb)):
                    for cp in range(2):
                        nc.tensor.matmul(
                            ps[:, :512],
                            at_[:, 256 * cp:256 * (cp + 1)].rearrange(
                                "p (c m) -> p c m", c=2),
                            wo[cp][:, :, nsl],
                            start=first,
                            stop=(at_ is atl and cp == 1),
                            perf_mode=DR)
                        first = False
                ob = ob_pool.tile([128, 512], BF16, tag="ob")
                if _ps_state["wide"] and (s + nh) % 2 == 0:
                    nc.scalar.copy(ob[:], ps[:, :512])
                else:
                    nc.vector.tensor_copy(out=ob[:], in_=ps[:, :512])
                # in the final drain the sync HWDGE queue serializes the last
                # 8 output stores (~625ns descriptor-gen each); spread them
                # over the idle queues so the tail isn't gated on it
                eng = (nc.sync if not _ps_state["wide"] else
                       (nc.sync, nc.gpsimd, nc.scalar)[(2 * s + nh) % 3])
                eng.dma_start(
                    out=out[row0:row0 + 128, 512 * nh:512 * (nh + 1)],
                    in_=ob[:])

            ounits = []
            for s in range(TG // 128):
                ounits.append(lambda s=s: trans_unit(s))
            for s in range(TG // 128):
                for nh in range(2):
                    ounits.append(lambda s=s, nh=nh: oproj_unit(s, nh))
            return units, ounits

        def interleave(a_units, b_units, pre_b=0):
            # pre_b: emit that many b-units before any a-unit (phase 0 needs
            # the group-0 V chains emitted before the first AV flush so the
            # tile framework sees the writes first)
            for u in b_units[:pre_b]:
                u()
            b_units = b_units[pre_b:]
            na, nb = len(a_units), len(b_units)
            ia = ib = 0
            while ia < na or ib < nb:
                fa = (na - ia) / na if na else 0.0
                fb = (nb - ib) / nb if nb else 0.0
                if ia < na and (fa > fb or ib >= nb):
                    a_units[ia]()
                    ia += 1
                else:
                    b_units[ib]()
                    ib += 1

        # prologue: group 0 projection (weight DMAs after group 0's x loads).
        # The scores/AV banks are idle here, so scratch rotates through them.
        set_wide_scratch(True, banked=True)
        # fill the initial x/w DMA latency with dummy PE work (also completes
        # the tensor engine's p-state ramp before real work lands); plain
        # matmuls on a memset tile need no identity, so they start ~1us in
        dummy = const.tile([128, 256], BF16, tag="dummy")
        nc.vector.memset(dummy[:], 0.0)
        xt0, tunits0 = transpose_units(0)
        for u in tunits0:
            u()
        load_weights()
        make_consts()
        # SWDGE gate: the gpsimd descriptor ring would otherwise eagerly
        # prepare the later-group x / wout transfers at t~1us and their
        # transfers would jump the (serial) DMA lane ahead of the
        # prologue-critical weight transfers. A tiny Pool-engine copy that
        # waits on the first x tile holds the ring back until the prologue
        # transfers are in flight.
        gate = const.tile([1, 1], FP8, tag="gate")
        nc.gpsimd.tensor_copy(out=gate[:], in_=xt0[1][0:1, CH - 1, 0:1])
        for _ in range(NDUMMY):
            wps = scratch_ps(F32)
            nc.tensor.matmul(wps[:, :256], dummy[:, 0:128], dummy[:],
                             start=True, stop=True)
        # prologue runs only the q and k chains (their weights land first);
        # group 0's V chains are deferred into the phase-0 fill, where they
        # interleave with the (V-independent) score units while the V-column
        # weights land. Unit order follows the DMA landing order: all h=0
        # halves (chunks 0-3) before h=1 (chunks 4-7). This holds up to 7
        # open psum accumulations (4 q + 3 k), which the banked scratch
        # rotation provides; k p3 waits until the q psums close.
        _ps_state["i"] = 0
        q0, k0, v0 = qkv_units(0, xt0)
        for u in [q0[0], q0[1], q0[2], q0[3],      # q h0 p0-3
                  k0[0], k0[1], k0[2],             # k h0 p0-2
                  q0[4], q0[5], q0[6], q0[7],      # q h1 p0-3 (close q psums)
                  k0[3],                           # k h0 p3
                  k0[4], k0[5], k0[6], k0[7]]:     # k h1 p0-3
            u()
        set_wide_scratch(False)
        # steady state: attention(g) interleaved with transposes(g+1) +
        # projection(g+1); out-projections are deferred up to two groups so
        # the last (largest) attention group still has dense PE fill
        pending_oproj = []  # deferred out-projection unit lists, oldest first
        fill_carry = v0     # group g's V chains lead the phase-g fill: their
        # only consumer is the AV flush (safely late in emission), and the
        # ACT-bound later phases need the q/k fill to shrink, not grow
        for g in range(n_tg):
            attn, ounits = attention_units(g)
            fill = fill_carry
            fill_carry = []
            if g + 1 < n_tg:
                xt1, tunits = transpose_units(g + 1)
                qu, ku, vu = qkv_units(g + 1, xt1)
                fill += tunits + ([load_wout] if g == 0 else []) + qu + ku
                fill_carry = vu
            if g == n_tg - 2 and pending_oproj:
                # phase 2 is ACT(exp)-bound with PE slack: give it the
                # oldest deferred out-projection group
                fill += pending_oproj.pop(0)
            if g + 1 == n_tg:
                if g == 0:
                    fill.append(load_wout)
                # last group is exp(ACT)-bound and has no next-group
                # projection: feed it the remaining deferred out-projections
                while pending_oproj:
                    fill += pending_oproj.pop(0)
            interleave(attn, fill, pre_b=len(v0) if g == 0 else 0)
            pending_oproj.append(ounits)
        # final drain: attention is done, scores/AV banks are idle again
        set_wide_scratch(True)
        for ou in pending_oproj:
            for u in ou:
                u()


def build_program(with_bias, seq_len=T):
    nc = bacc.Bacc("TRN2", target_bir_lowering=False, debug=False,
                   enable_asserts=False, num_devices=8)
    xh = nc.dram_tensor("xh", [D, seq_len], FP8, kind="ExternalInput").ap()
    xl = nc.dram_tensor("xl", [D, seq_len], FP8, kind="ExternalInput").ap()
    wqh = nc.dram_tensor("wqh", [D, 3 * HPC * HD], FP8,
                         kind="ExternalInput").ap()
    wql = nc.dram_tensor("wql", [D, 3 * HPC * HD], FP8,
                         kind="ExternalInput").ap()
    wouth = nc.dram_tensor("wouth", [HPC * HD, D], FP8,
                           kind="ExternalInput").ap()
    woutl = nc.dram_tensor("woutl", [HPC * HD, D], FP8,
                           kind="ExternalInput").ap()
    out = nc.dram_tensor("out", [seq_len, D], BF16,
                         kind="ExternalOutput").ap()
    bqkv = None
    if with_bias:
        bqkv = nc.dram_tensor("bqkv", [1, 3 * HPC * HD], F32,
                              kind="ExternalInput").ap()
    with tile.TileContext(nc) as tc:
        build_tile_program(tc, xh, xl, wqh, wql, wouth, woutl, out, bqkv,
                           seq_len=seq_len)
    nc.compile()
    return nc


_PROGRAM_CACHE = {}


def _get_program(with_bias):
    if with_bias not in _PROGRAM_CACHE:
        _PROGRAM_CACHE[with_bias] = build_program(with_bias)
    return _PROGRAM_CACHE[with_bias]


def _split8(a):
    """fp8(e4m3) hi/lo split: a ~= hi + lo to ~7 mantissa bits."""
    import ml_dtypes
    hi = a.astype(ml_dtypes.float8_e4m3)
    lo = (a - hi.astype(np.float32)).astype(ml_dtypes.float8_e4m3)
    return hi, lo


def make_core_inputs(x_core, wqkv_core, wout_core, b_core=None):
    """Quantized inputs for ONE core: x_core [T, D], wqkv_core [D, 1536]
    (q|k|v grouped), wout_core [512, D]."""
    xh, xl = _split8(np.ascontiguousarray(x_core.T))
    wqh, wql = _split8(np.ascontiguousarray(wqkv_core) * WSCALE)
    wouth, woutl = _split8(np.ascontiguousarray(wout_core) * WSCALE)
    m = {
        "xh": xh, "xl": xl, "wqh": wqh, "wql": wql,
        "wouth": wouth, "woutl": woutl,
    }
    if b_core is not None:
        m["bqkv"] = np.ascontiguousarray(b_core * WSCALE).reshape(
            1, -1).astype(np.float32)
    return m


def make_in_maps(x, w_qkv, b_qkv, w_out, with_bias):
    """Per-core input dicts: core c -> batch c//2, head group c%2."""
    in_maps = []
    for core in range(8):
        b, gr = divmod(core, 2)
        qc = slice(512 * gr, 512 * (gr + 1))
        kc = slice(D + 512 * gr, D + 512 * (gr + 1))
        vc = slice(2 * D + 512 * gr, 2 * D + 512 * (gr + 1))
        wq = np.concatenate([w_qkv[:, qc], w_qkv[:, kc], w_qkv[:, vc]], axis=1)
        bq = (np.concatenate([b_qkv[qc], b_qkv[kc], b_qkv[vc]])
              if with_bias else None)
        in_maps.append(make_core_inputs(
            x[b], wq, w_out[512 * gr:512 * (gr + 1), :], bq))
    return in_maps


def kernel(x, w_qkv, b_qkv, w_out, b_out):
    x = np.asarray(x, dtype=np.float32)
    w_qkv = np.asarray(w_qkv, dtype=np.float32)
    b_qkv = np.asarray(b_qkv, dtype=np.float32)
    w_out = np.asarray(w_out, dtype=np.float32)
    b_out = np.asarray(b_out, dtype=np.float32)

    with_bias = bool(np.any(b_qkv))
    nc = _get_program(with_bias)
    in_maps = make_in_maps(x, w_qkv, b_qkv, w_out, with_bias)
    res = run_bass_kernel_spmd(nc, in_maps, core_ids=list(range(8))).results

    out = np.empty((B, T, D), dtype=np.float32)
    for b in range(B):
        out[b] = ((res[2 * b]["out"].astype(np.float32)
                   + res[2 * b + 1]["out"].astype(np.float32)) * OUT_SCALE
                  + b_out[None, :])
    return out



# revision 73
# speedup vs baseline: 1.0144x; 1.0144x over previous
"""Causal self-attention (B=4, T=2048, D=1024, H=16, hd=64) on 8 trn2 cores.

Sharding: 4-way data parallel over batch x 2-way tensor parallel over heads.
Core c handles batch c//2 and heads [8*(c%2), 8*(c%2)+8). Each core computes
its heads' partial contribution to the out-projection (a full [T, D] tensor);
the host sums the two head-group partials per batch and adds b_out.

Host-side prep: x is transposed to xT [D, T] and split into fp8(e4m3)
hi + lo residual planes; w_qkv (x32) and w_out (x32) likewise. The QKV and
out-projections then run as fp8 DoubleRow matmuls (2 K-chunks per pass, 0.5
cycles/row = 4x bf16 MAC rate) with three hi/lo cross terms (hi*hi + hi*lo +
lo*hi, the lo*lo term dropped), which is 0.75x the bf16 cost at ~bf16
accuracy. The weight x32 scaling keeps the lo planes out of fp8's subnormal
range; the softmax scale absorbs 1/1024 and the host divides the output
partials by 1024. End-to-end rel err vs the fp32 reference is ~4.4e-3
(gate: 2e-2).

Per-core dataflow (single NeuronCore, Tile framework):
  1. xT hi/lo stream straight from DRAM into SBUF (no on-device work).
  2. qT/kT [hd,T] are computed head-PAIR-stacked ([128,T] per pair) with
     w_qkv chunks as stationary DoubleRow fp8 matmuls; V [T,hd] per head is
     computed naturally and scattered into v1 (64 cols per (head, k-tile),
     bf16).
  3. Scores are computed TRANSPOSED: S^T[k,q] = K @ Q^T via two K=64 matmuls
     per (pair, k-tile) into one [128, 2, 512] two-bank PSUM tile; ONE wide
     exp (ACT, scale folded in) produces P^T for both heads in bf16.
     Causality: k-tiles above the diagonal are skipped, diagonal blocks get
     an affine_select zero-mask after exp, partial k-tiles use narrower
     matmuls.
  4. AV runs NON-transposed: per (head, q-subtile 128) the P^T block is the
     STATIONARY operand and V [128,64] streams, so each matmul moves 64
     rows instead of 512. All 8 (head, subtile) accumulators of a pair pack
     into ONE psum bank (zero-on-first-touch after a single start=True).
     Softmax sums ride as 1-column matmuls (P^T stationary, ones moving)
     into a shared, DVE-zeroed sums bank; AV emission trails its exp by 8
     k-tile units (carry) so the PE never waits on ACT latency.
  5. Normalization is a DVE broadcast multiply (reciprocal of sums, then
     attn * recip -> bf16). The normalized attn [q, 512] is PE-transposed
     (bf16), then quantized to an fp8 hi/lo pair (DVE copy + subtract; the
     hi copy moves to ACT in the drain where DVE is the serial bottleneck)
     as the out-projection's stationary operand; out = attnT.T @ w_out
     accumulates as 6 DoubleRow matmuls over 2 feature-chunk-pairs x 3
     hi/lo terms. All out-projections are deferred into the last q-group's
     phase + final drain, which are otherwise exp(ACT)-bound.

Scheduling: emission-level software pipelining interleaves attention for
q-group g with the transposes + q/k projections of group g+1; each group's
V chains ride at the FRONT of its own phase (their only consumer, the AV
flush, is emitted safely later). PSUM (8 banks) is budgeted as 4 scores +
2 AV + 1 sums + 1 projection scratch; the prologue additionally uses the
score slots' second banks for 7 concurrent q/k accumulators, and the final
drain widens the scratch rotation to 5. The cost model serializes all DMA
transfers through one lane fed by two descriptor-gen queues (HWDGE ~625ns,
SWDGE ~1.2us per dma_start), so the prologue issues transfers in exact
consumption order and gates the eager SWDGE ring (later-group x, w_out)
behind a Pool-engine copy of the last group-0 x tile; dummy matmuls at t=0
cover the first DMA latency and the PE p-state ramp (full clock needs 3us
of continuous execution - any >100ns gap drops it to half rate).
"""

import numpy as np

import concourse.bass as bass
import concourse.mybir as mybir
import concourse.tile as tile
from concourse import bacc
from concourse.bass_utils import run_bass_kernel_spmd
from concourse.masks import make_identity

B, T, D = 4, 2048, 1024
H, HD = 16, 64
HPC = 8              # heads per core
PAIRS = HPC // 2
CH = D // 128        # K-chunks for the QKV projection
TG = 512             # T-group / q-group width
SCALE = 1.0 / 8.0    # 1/sqrt(HD)
NDUMMY = 18          # prologue PE-warm dummy matmuls

F32 = mybir.dt.float32
F32R = mybir.dt.float32r
BF16 = mybir.dt.bfloat16
FP8 = mybir.dt.float8e4
DR = mybir.MatmulPerfMode.DoubleRow
AF = mybir.ActivationFunctionType

# w_qkv and w_out are scaled by 2**5 on the host so their fp8(e4m3) hi/lo
# splits stay out of the subnormal range; q', k', v' come out 32x larger, so
# the softmax scale absorbs 1/(32*32) and the device output is 32*32x the
# true projection (divided back out on the host).
WSCALE = 32.0
OUT_SCALE = 1.0 / (WSCALE * WSCALE)


def _r(ap):
    return ap.bitcast(F32R)


def build_tile_program(tc, xh, xl, wqh, wql, wouth, woutl, out, bqkv=None,
                       seq_len=T):
    """Emit the per-core program. seq_len is parametrized for small-scale
    simulation tests; the real kernel uses seq_len=T=2048.

    Emission is software-pipelined: attention for q-group g (latency-bound
    serial chains sT -> exp -> mask -> AV) is interleaved at emission level
    with the transposes + QKV projection of group g+1 (dense PE work), so the
    in-order PE fills attention's dependency stalls with projection matmuls.
    """
    nc = tc.nc
    n_tg = seq_len // TG
    n_tt = seq_len // 128
    with_bias = bqkv is not None

    from contextlib import ExitStack
    with ExitStack() as ctx:
        const = ctx.enter_context(tc.tile_pool(name="const", bufs=1))
        wpool = ctx.enter_context(tc.tile_pool(name="wpool", bufs=1))
        xt_pool = ctx.enter_context(tc.tile_pool(name="xt", bufs=4))
        qt_pool = ctx.enter_context(tc.tile_pool(name="qt", bufs=2 * PAIRS))
        pt_pool = ctx.enter_context(tc.tile_pool(name="pt", bufs=14))
        an_pool = ctx.enter_context(tc.tile_pool(name="an", bufs=16))
        at_pool = ctx.enter_context(tc.tile_pool(name="at", bufs=16))
        ob_pool = ctx.enter_context(tc.tile_pool(name="ob", bufs=6))
        rc_pool = ctx.enter_context(tc.tile_pool(name="rc", bufs=2))
        mm_ps = ctx.enter_context(tc.tile_pool(name="mmps", bufs=1, space="PSUM"))
        st_ps = ctx.enter_context(tc.tile_pool(name="stps", bufs=2, space="PSUM"))
        av_ps = ctx.enter_context(tc.tile_pool(name="avps", bufs=2, space="PSUM"))
        sums_ps = ctx.enter_context(tc.tile_pool(name="sums", bufs=1, space="PSUM"))
        identity = const.tile([128, 128], F32, tag="ident")
        ident_bf = const.tile([128, 128], BF16, tag="identbf")
        ones_bf = const.tile([128, 1], BF16, tag="ones_bf")

        def make_consts():
            # deferred so the weight SWDGE generation isn't queued behind
            # make_identity on the Pool engine at t=0; the identity is only
            # needed by the (late) attnT transposes
            make_identity(nc, identity[:])
            nc.vector.tensor_copy(out=ident_bf[:], in_=identity[:])
            nc.vector.memset(ones_bf[:], 1.0)
        if with_bias:
            ones_f = const.tile([128, 64], F32, tag="ones_f")
            nc.vector.memset(ones_f[:], 1.0)
            b_sb = const.tile([1, 3 * HPC * HD], F32R, tag="bias")
            nc.sync.dma_start(out=b_sb[:], in_=bqkv.bitcast(F32R)[:])
            ones_row = const.tile([1, TG], F32R, tag="ones_row")
            nc.vector.tensor_copy(
                out=ones_row[:], in_=ones_f[0:1, 0:1].broadcast_to([1, TG]))

        # softmax sums: one persistent bank; cols = parity*32 + pair*8 + h*4+s
        sums = sums_ps.tile([128, 64], F32, tag="sums")

        # resident weights (fp8 hi/lo pairs); w_out pair-packed: pair p rows
        # [128p, 128p+128). The DMAs are deferred until after group 0's x
        # loads so the first transposes aren't queued behind the weight
        # traffic.
        wh_sb = wpool.tile([128, CH, 3 * HPC * HD], FP8, tag="wqh")
        wl_sb = wpool.tile([128, CH, 3 * HPC * HD], FP8, tag="wql")
        # wout fp8 hi/lo, chunk-PAIR packed for DoubleRow: tile cp holds rows
        # [256cp, 256cp+256) as [128, 2, D]
        wouth_sb = [wpool.tile([128, 2, D], FP8, tag=f"wouth{p}",
                               name=f"wouth{p}") for p in range(2)]
        woutl_sb = [wpool.tile([128, 2, D], FP8, tag=f"woutl{p}",
                               name=f"woutl{p}") for p in range(2)]

        def load_weights():
            # The cost model serializes ALL transfers through one DMA_ENGINES
            # lane (~bytes/360GB/s each), fed by two descriptor-gen lanes:
            # HWDGE (shared SP+ACT, 625ns/dma) and SWDGE (gpsimd, ~1.1us/dma).
            # The prologue is transfer-order-bound and the PE is in-order, so
            # transfers are issued in exact consumption order of the prologue
            # units (q chains h0/h1, k chains h0/h1, then V), with q/k weight
            # columns split so q unblocks first. All on the scalar HWDGE
            # queue; x rides the sync HWDGE queue; later-group x and wout
            # ride SWDGE.
            for c0 in (0, 4):
                for w_sb, wsrc in ((wh_sb, wqh), (wl_sb, wql)):
                    nc.scalar.dma_start(
                        out=w_sb[:, c0:c0 + 4, 0:1024],
                        in_=wsrc[128 * c0:128 * (c0 + 4), 0:1024].rearrange(
                            "(c p) n -> p c n", p=128))
            for w_sb, wsrc in ((wh_sb, wqh), (wl_sb, wql)):
                for c0 in range(0, CH, 4):
                    nc.scalar.dma_start(
                        out=w_sb[:, c0:c0 + 4, 1024:1536],
                        in_=wsrc[128 * c0:128 * (c0 + 4), 1024:1536].rearrange(
                            "(c p) n -> p c n", p=128))

        def load_wout():
            # deferred into the phase-0 fill (behind the group-1 x loads on
            # SWDGE): first consumed by the deferred group-0 out-projections
            # in phase 1 (~45us in)
            with tc.tile_wait_until(0.016):
                for cp in range(2):
                    for w_sb, wsrc in ((wouth_sb, wouth), (woutl_sb, woutl)):
                        nc.gpsimd.dma_start(
                            out=w_sb[cp][:],
                            in_=wsrc[256 * cp:256 * (cp + 1), :].rearrange(
                                "(c p) n -> p c n", p=128))
        # kT per pair, head 2p on partitions [0:64), head 2p+1 on [64:128)
        kts = [wpool.tile([128, seq_len], BF16, tag=f"kt{p}", name=f"kt{p}")
               for p in range(PAIRS)]
        # V in bf16: per (head, k-tile) a [128, 64] stationary block
        v1 = wpool.tile([128, HPC, n_tt, HD], BF16, tag="v1")

        qts_of = {}  # g -> [qt tiles per pair]

        # psum scratch for the projection/transpose/oproj chains. In steady
        # state only the mm bank is free; in the prologue and final drain the
        # scores/AV banks are idle, so rotate through them too (the st slot is
        # [128,2,512]; its first bank is used as a [128,512] scratch).
        # "banked" mode (prologue only, F32) additionally uses the st slots'
        # SECOND psum banks (zero regions are 2KB, so the two banks of one
        # slot hold independent accumulation groups), giving 7 concurrent
        # scratch accumulators for the q+k chain overlap.
        _ps_state = {"wide": False, "banked": False, "i": 0, "st": {}}

        def set_wide_scratch(wide, banked=False):
            _ps_state["wide"] = wide
            _ps_state["banked"] = banked

        def scratch_ps(dtype):
            if not _ps_state["wide"]:
                return mm_ps.tile([128, 512], dtype, tag="mm", name="mm")
            if _ps_state["banked"] and dtype == F32:
                i = _ps_state["i"] = (_ps_state["i"] + 1) % 7
                if i == 0:
                    return mm_ps.tile([128, 512], dtype, tag="mm", name="mm")
                if i in (1, 2, 3, 4):
                    buf, bank = divmod(i - 1, 2)
                    if bank == 0:
                        _ps_state["st"][buf] = st_ps.tile(
                            [128, 2, 512], dtype, tag="st", name="stx")
                    return _ps_state["st"][buf][:, bank, :]
                return av_ps.tile([128, 512], dtype, tag="av", name="avx")
            i = _ps_state["i"] = (_ps_state["i"] + 1) % 5
            if i == 0:
                return mm_ps.tile([128, 512], dtype, tag="mm", name="mm")
            if i in (1, 2):
                return st_ps.tile([128, 512], dtype, tag="st", name="stx")
            return av_ps.tile([128, 512], dtype, tag="av", name="avx")

        def transpose_units(g):
            """x arrives pre-transposed and fp8-hi/lo-split from the host:
            just DMA the group's xT columns. Group 0 (prologue, latency-
            critical) goes on the sync HWDGE queue split in half-chunks;
            later groups ride the then-idle SWDGE ring as one transfer per
            hi/lo half."""
            xt = (xt_pool.tile([128, CH, TG], FP8, tag="xth", name=f"xth{g}"),
                  xt_pool.tile([128, CH, TG], FP8, tag="xtl", name=f"xtl{g}"))
            gsl = slice(g * TG, (g + 1) * TG)

            def u():
                if g == 0:
                    # lo directly after hi per chunk-half: every chain's
                    # third (lo*hi) term needs x-lo early
                    for c0 in (0, 4):
                        for t, src in ((xt[0], xh), (xt[1], xl)):
                            nc.sync.dma_start(
                                out=t[:, c0:c0 + 4, :],
                                in_=src[128 * c0:128 * (c0 + 4), gsl].rearrange(
                                    "(c p) n -> p c n", p=128))
                else:
                    # wait-ts biases the tile scheduler so these preps don't
                    # jump the serial DMA lane ahead of the prologue-critical
                    # weight transfers
                    with tc.tile_wait_until(0.010 + 0.038 * (g - 1)):
                        for t, src in ((xt[0], xh), (xt[1], xl)):
                            nc.gpsimd.dma_start(
                                out=t[:, :, :],
                                in_=src[:, gsl].rearrange(
                                    "(c p) n -> p c n", p=128))
            return xt, [u]

        def qkv_units(g, xt):
            """12 units: 4 qt chains, 4 kT chains, 4 V chains."""
            qts = qts_of.setdefault(g, [])
            units = []

            half = {}

            xth, xtl = xt

            def qk_chain(p, qk, h):
                # split into two half-chains (finer interleave granularity).
                # Each half contracts 4 D-chunks = 2 fp8 DoubleRow chunk-pairs
                # x 3 hi/lo cross terms (lo*lo dropped). Terms are ordered
                # hi*hi first so the prologue chains can start as soon as the
                # hi transfers land (lo halves trickle in later).
                if h == 0:
                    half[(p, qk)] = scratch_ps(F32)
                ps = half[(p, qk)]
                col = qk * 512 + p * 128
                first = True
                for wt, xt_ in ((wh_sb, xth), (wl_sb, xth), (wh_sb, xtl)):
                    for c0 in range(4 * h, 4 * h + 4, 2):
                        cs = slice(c0, c0 + 2)
                        nc.tensor.matmul(
                            ps[:, :TG], wt[:, cs, col:col + 128],
                            xt_[:, cs, :],
                            start=(h == 0 and first),
                            stop=(h == 1 and c0 == CH - 2 and xt_ is xtl
                                  and not with_bias),
                            perf_mode=DR)
                        first = False
                if h == 0:
                    return
                if with_bias:
                    nc.tensor.matmul(
                        ps[:, :TG], b_sb[0:1, col:col + 128],
                        ones_row[0:1, :], start=False, stop=True)
                if qk == 0:
                    qt = qt_pool.tile([128, TG], BF16, tag="qt")
                    nc.vector.tensor_copy(out=qt[:], in_=ps[:, :TG])
                    qts.append(qt)
                else:
                    nc.vector.tensor_copy(
                        out=kts[p][:, g * TG:(g + 1) * TG], in_=ps[:, :TG])

            def v_chain(t4):
                tt = g * (TG // 128) + t4
                ps = scratch_ps(F32)
                first = True
                tsl = slice(128 * t4, 128 * (t4 + 1))
                for xt_, wt in ((xth, wh_sb), (xth, wl_sb), (xtl, wh_sb)):
                    for c0 in range(0, CH, 2):
                        cs = slice(c0, c0 + 2)
                        nc.tensor.matmul(
                            ps[:, :512], xt_[:, cs, tsl],
                            wt[:, cs, 1024:1536],
                            start=first,
                            stop=(c0 == CH - 2 and xt_ is xtl
                                  and not with_bias),
                            perf_mode=DR)
                        first = False
                if with_bias:
                    nc.tensor.matmul(
                        ps[:, :512], ones_row[0:1, 0:128],
                        b_sb[0:1, 1024:1536], start=False, stop=True)
                nc.vector.tensor_copy(
                    out=v1[:, :, tt, :],
                    in_=ps[:, :512].rearrange("p (h d) -> p h d", h=HPC))

            k_units = []
            v_units = []
            # in the prologue the qk chains are gated on their W chunks
            # landing (~1us apart): run all h=0 half-chains (chunks 0-3)
            # before any h=1 (chunks 4-7). Requires 4 concurrent psum
            # accumulators - only legal in the wide-scratch prologue.
            hmajor = _ps_state["wide"]
            for lst, qk in ((units, 0), (k_units, 1)):
                if hmajor:
                    for h in range(2):
                        for p in range(PAIRS):
                            lst.append(lambda p=p, h=h, qk=qk: qk_chain(p, qk, h))
                else:
                    for p in range(PAIRS):
                        lst.append(lambda p=p, qk=qk: qk_chain(p, qk, 0))
                        lst.append(lambda p=p, qk=qk: qk_chain(p, qk, 1))
            for t4 in range(TG // 128):
                v_units.append(lambda t4=t4: v_chain(t4))
            return units, k_units, v_units

        def attention_units(g):
            """Per pair: one unit per k-tile (sT+exp+mask, AV carried by one),
            then a normalization unit; finally the out-projection units."""
            units = []
            pending_finish = []
            an_tiles = [an_pool.tile([128, 512], BF16, tag="an",
                                     name=f"an_g{g}_s{s}")
                        for s in range(TG // 128)]
            # zero this group's sums columns once (all 4 pairs' 8-col slices)
            nc.vector.memset(sums[:, (g % 2) * 32:(g % 2) * 32 + 32], 0.0)
            qts = qts_of[g]
            for p in range(PAIRS):
                nkt = 4 * (g + 1)
                soff = (g % 2) * 32 + p * 8
                state = {}

                def start_pair(p=p, state=state, soff=soff):
                    state["av"] = av_ps.tile([128, 512], F32, tag="av",
                                             name=f"av_g{g}_p{p}")
                    state["carry"] = []
                    state["first"] = True

                def kt_unit(kt, pos, p=p, state=state, nkt=nkt,
                            sp=start_pair):
                    if pos == 0:
                        sp()
                    # pop the carried AV FIRST: it is ready now, and the
                    # scores below may head-block on a busy st slot
                    if len(state["carry"]) >= 9:
                        state["emit_av"](*state["carry"].pop(0))
                    qt = qts[p]
                    rdiag = kt - 4 * g
                    col0 = 128 * rdiag if rdiag > 0 else 0
                    ksl = slice(128 * kt, 128 * (kt + 1))
                    st = st_ps.tile([128, 2, 512], F32, tag="st")
                    nc.tensor.matmul(
                        st[:, 0, col0:], kts[p][0:64, ksl], qt[0:64, col0:])
                    nc.tensor.matmul(
                        st[:, 1, col0:], kts[p][64:128, ksl], qt[64:128, col0:])
                    pt = pt_pool.tile([128, 2, 512], BF16, tag="pt")
                    nc.scalar.activation(
                        pt[:, :, col0:], st[:, :, col0:], AF.Exp,
                        scale=SCALE / (WSCALE * WSCALE))
                    if rdiag >= 0:
                        for h in (0, 1):
                            # keep P^T[k, q] only where q >= k (within-block)
                            nc.gpsimd.affine_select(
                                out=pt[:, h, col0:col0 + 128],
                                in_=pt[:, h, col0:col0 + 128],
                                compare_op=mybir.AluOpType.is_ge,
                                fill=0.0, base=0, pattern=[[1, 128]],
                                channel_multiplier=-1)
                    state["carry"].append((kt, pos, rdiag, pt))

                def emit_av(kt, pos, rdiag, pt, p=p, state=state, nkt=nkt,
                            soff=soff):
                    av = state["av"]
                    s0 = max(rdiag, 0)
                    last = pos == nkt - 1
                    for s in range(s0, TG // 128):
                        for h in (0, 1):
                            nc.tensor.matmul(
                                av[:, 128 * s + 64 * h:128 * s + 64 * h + 64],
                                pt[:, h, 128 * s:128 * (s + 1)],
                                v1[:, 2 * p + h, kt, :],
                                start=state["first"],
                                stop=(last and s == 3 and h == 1))
                            state["first"] = False
                            nc.tensor.matmul(
                                sums[:, soff + 4 * h + s:soff + 4 * h + s + 1],
                                pt[:, h, 128 * s:128 * (s + 1)],
                                ones_bf[:],
                                start=False, stop=False,
                                skip_group_check=True)

                state["emit_av"] = emit_av

                def emit_block(mm, state, p, soff, stop_last):
                    av = state["av"]
                    for i, (isd, kt, pt, s, h) in enumerate(mm):
                        nc.tensor.matmul(
                            av[:, 128 * s + 64 * h:128 * s + 64 * h + 64],
                            pt[:, h, 128 * s:128 * (s + 1)],
                            v1[:, 2 * p + h, kt, :],
                            start=state["first"],
                            stop=(stop_last and i == len(mm) - 1))
                        state["first"] = False
                        nc.tensor.matmul(
                            sums[:, soff + 4 * h + s:soff + 4 * h + s + 1],
                            pt[:, h, 128 * s:128 * (s + 1)],
                            ones_bf[:],
                            start=False, stop=False,
                            skip_group_check=True)

                def flush_unit(p=p, state=state, soff=soff):
                    # flush the carried non-diagonal AVs at pair end; the
                    # affine-masked diagonal blocks wait for Pool latency, so
                    # they are deferred into finish_unit (emitted two k-tile
                    # units into the NEXT pair) to avoid head-blocking the
                    # PE's 4-deep dependency wait queue
                    mm = []
                    for kt, pos, rdiag, pt in state["carry"]:
                        for s in range(max(rdiag, 0), TG // 128):
                            for h in (0, 1):
                                mm.append((s == rdiag, kt, pt, s, h))
                    state["carry"] = []
                    mm.sort(key=lambda t: t[0])
                    ndiag = sum(1 for t in mm if t[0])
                    split = len(mm) - ndiag
                    emit_block(mm[:split], state, p, soff, stop_last=False)
                    state["diag"] = mm[split:]

                def norm_unit(p=p, state=state, soff=soff):
                    emit_block(state.pop("diag"), state, p, soff,
                               stop_last=True)
                    av = state["av"]
                    rc = rc_pool.tile([128, 2, 4], F32, tag="rc")
                    nc.vector.reciprocal(
                        rc[:], sums[:, soff:soff + 8].rearrange(
                            "p (h s) -> p h s", h=2))
                    for s in range(TG // 128):
                        nc.vector.tensor_mul(
                            an_tiles[s][:, 128 * p:128 * (p + 1)].rearrange(
                                "p (h d) -> p h d", h=2),
                            av[:, 128 * s:128 * (s + 1)].rearrange(
                                "p (h d) -> p h d", h=2),
                            rc[:, :, s:s + 1].broadcast_to([128, 2, 64]))

                pair_units = []
                for pos, kt in enumerate(range(nkt)):
                    pair_units.append(
                        lambda kt=kt, pos=pos, f=kt_unit: f(kt, pos))
                if pending_finish:
                    pair_units.insert(min(6, max(2, len(pair_units) - 2)),
                                      pending_finish.pop())
                units += pair_units
                units.append(flush_unit)
                pending_finish.append(norm_unit)

            if pending_finish:
                units.append(pending_finish.pop())

            ats = {}

            def trans_unit(s):
                ps = scratch_ps(BF16)
                for c in range(PAIRS):
                    nc.tensor.transpose(
                        ps[:, 128 * c:128 * (c + 1)],
                        an_tiles[s][:, 128 * c:128 * (c + 1)],
                        ident_bf[:])
                # fp8 hi/lo split of attnT for the DoubleRow out-projection.
                # The hi copy runs on the (mostly idle) Pool engine so the
                # DVE isn't the serial bottleneck of the projection drain.
                ath = at_pool.tile([128, 512], FP8, tag="ath")
                atl = at_pool.tile([128, 512], FP8, tag="atl")
                if _ps_state["wide"]:
                    # final drain: ACT is idle and DVE is the serial
                    # bottleneck, so split the quant pair across both
                    nc.scalar.copy(ath[:], ps[:])
                else:
                    nc.vector.tensor_copy(out=ath[:], in_=ps[:])
                nc.vector.tensor_sub(atl[:], ps[:], ath[:])
                ats[s] = (ath, atl)

            def oproj_unit(s, nh):
                row0 = g * TG + 128 * s
                ath, atl = ats[s]
                ps = scratch_ps(F32)
                nsl = slice(512 * nh, 512 * (nh + 1))
                first = True
                for at_, wo in ((ath, wouth_sb), (ath, woutl_sb),
                                (atl, wouth_sb)):
                    for cp in range(2):
                        nc.tensor.matmul(
                            ps[:, :512],
                            at_[:, 256 * cp:256 * (cp + 1)].rearrange(
                                "p (c m) -> p c m", c=2),
                            wo[cp][:, :, nsl],
                            start=first,
                            stop=(at_ is atl and cp == 1),
                            perf_mode=DR)
                        first = False
                ob = ob_pool.tile([128, 512], BF16, tag="ob")
                if _ps_state["wide"] and (s + nh) % 2 == 0:
                    nc.scalar.copy(ob[:], ps[:, :512])
                else:
                    nc.vector.tensor_copy(out=ob[:], in_=ps[:, :512])
                # in the final drain the sync HWDGE queue serializes the last
                # 8 output stores (~625ns descriptor-gen each); spread them
                # over the idle queues so the tail isn't gated on it
                eng = (nc.sync if not _ps_state["wide"] else
                       (nc.sync, nc.gpsimd, nc.scalar)[(2 * s + nh) % 3])
                eng.dma_start(
                    out=out[row0:row0 + 128, 512 * nh:512 * (nh + 1)],
                    in_=ob[:])

            ounits = []
            for s in range(TG // 128):
                ounits.append(lambda s=s: trans_unit(s))
            for s in range(TG // 128):
                for nh in range(2):
                    ounits.append(lambda s=s, nh=nh: oproj_unit(s, nh))
            return units, ounits

        def interleave(a_units, b_units, pre_b=0):
            # pre_b: emit that many b-units before any a-unit (phase 0 needs
            # the group-0 V chains emitted before the first AV flush so the
            # tile framework sees the writes first)
            for u in b_units[:pre_b]:
                u()
            b_units = b_units[pre_b:]
            na, nb = len(a_units), len(b_units)
            ia = ib = 0
            while ia < na or ib < nb:
                fa = (na - ia) / na if na else 0.0
                fb = (nb - ib) / nb if nb else 0.0
                if ia < na and (fa > fb or ib >= nb):
                    a_units[ia]()
                    ia += 1
                else:
                    b_units[ib]()
                    ib += 1

        # prologue: group 0 projection (weight DMAs after group 0's x loads).
        # The scores/AV banks are idle here, so scratch rotates through them.
        set_wide_scratch(True, banked=True)
        # fill the initial x/w DMA latency with dummy PE work (also completes
        # the tensor engine's p-state ramp before real work lands); plain
        # matmuls on a memset tile need no identity, so they start ~1us in
        dummy = const.tile([128, 256], BF16, tag="dummy")
        nc.vector.memset(dummy[:], 0.0)
        xt0, tunits0 = transpose_units(0)
        for u in tunits0:
            u()
        load_weights()
        make_consts()
        # SWDGE gate: the gpsimd descriptor ring would otherwise eagerly
        # prepare the later-group x / wout transfers at t~1us and their
        # transfers would jump the (serial) DMA lane ahead of the
        # prologue-critical weight transfers. A tiny Pool-engine copy that
        # waits on the first x tile holds the ring back until the prologue
        # transfers are in flight.
        gate = const.tile([1, 1], FP8, tag="gate")
        nc.gpsimd.tensor_copy(out=gate[:], in_=xt0[1][0:1, CH - 1, 0:1])
        for _ in range(NDUMMY):
            wps = scratch_ps(F32)
            nc.tensor.matmul(wps[:, :256], dummy[:, 0:128], dummy[:],
                             start=True, stop=True)
        # prologue runs only the q and k chains (their weights land first);
        # group 0's V chains are deferred into the phase-0 fill, where they
        # interleave with the (V-independent) score units while the V-column
        # weights land. Unit order follows the DMA landing order: all h=0
        # halves (chunks 0-3) before h=1 (chunks 4-7). This holds up to 7
        # open psum accumulations (4 q + 3 k), which the banked scratch
        # rotation provides; k p3 waits until the q psums close.
        _ps_state["i"] = 0
        q0, k0, v0 = qkv_units(0, xt0)
        for u in [q0[0], q0[1], q0[2], q0[3],      # q h0 p0-3
                  k0[0], k0[1], k0[2],             # k h0 p0-2
                  q0[4], q0[5], q0[6], q0[7],      # q h1 p0-3 (close q psums)
                  k0[3],                           # k h0 p3
                  k0[4], k0[5], k0[6], k0[7]]:     # k h1 p0-3
            u()
        set_wide_scratch(False)
        # steady state: attention(g) interleaved with transposes(g+1) +
        # projection(g+1); out-projections are deferred up to two groups so
        # the last (largest) attention group still has dense PE fill
        pending_oproj = []  # deferred out-projection unit lists, oldest first
        fill_carry = v0     # group g's V chains lead the phase-g fill: their
        # only consumer is the AV flush (safely late in emission), and the
        # ACT-bound later phases need the q/k fill to shrink, not grow
        for g in range(n_tg):
            attn, ounits = attention_units(g)
            fill = fill_carry
            fill_carry = []
            if g + 1 < n_tg:
                xt1, tunits = transpose_units(g + 1)
                qu, ku, vu = qkv_units(g + 1, xt1)
                fill += tunits + ([load_wout] if g == 0 else []) + qu + ku
                fill_carry = vu
            if False and pending_oproj:
                # phase 2 is ACT(exp)-bound with PE slack: give it the
                # oldest deferred out-projection group
                fill += pending_oproj.pop(0)
            if g + 1 == n_tg:
                if g == 0:
                    fill.append(load_wout)
                # last group is exp(ACT)-bound and has no next-group
                # projection: feed it the remaining deferred out-projections
                while pending_oproj:
                    fill += pending_oproj.pop(0)
            interleave(attn, fill, pre_b=len(v0) if g == 0 else 0)
            pending_oproj.append(ounits)
        # final drain: attention is done, scores/AV banks are idle again
        set_wide_scratch(True)
        for ou in pending_oproj:
            for u in ou:
                u()


def build_program(with_bias, seq_len=T):
    nc = bacc.Bacc("TRN2", target_bir_lowering=False, debug=False,
                   enable_asserts=False, num_devices=8)
    xh = nc.dram_tensor("xh", [D, seq_len], FP8, kind="ExternalInput").ap()
    xl = nc.dram_tensor("xl", [D, seq_len], FP8, kind="ExternalInput").ap()
    wqh = nc.dram_tensor("wqh", [D, 3 * HPC * HD], FP8,
                         kind="ExternalInput").ap()
    wql = nc.dram_tensor("wql", [D, 3 * HPC * HD], FP8,
                         kind="ExternalInput").ap()
    wouth = nc.dram_tensor("wouth", [HPC * HD, D], FP8,
                           kind="ExternalInput").ap()
    woutl = nc.dram_tensor("woutl", [HPC * HD, D], FP8,
                           kind="ExternalInput").ap()
    out = nc.dram_tensor("out", [seq_len, D], BF16,
                         kind="ExternalOutput").ap()
    bqkv = None
    if with_bias:
        bqkv = nc.dram_tensor("bqkv", [1, 3 * HPC * HD], F32,
                              kind="ExternalInput").ap()
    with tile.TileContext(nc) as tc:
        build_tile_program(tc, xh, xl, wqh, wql, wouth, woutl, out, bqkv,
                           seq_len=seq_len)
    nc.compile()
    return nc


_PROGRAM_CACHE = {}


def _get_program(with_bias):
    if with_bias not in _PROGRAM_CACHE:
        _PROGRAM_CACHE[with_bias] = build_program(with_bias)
    return _PROGRAM_CACHE[with_bias]


def _split8(a):
    """fp8(e4m3) hi/lo split: a ~= hi + lo to ~7 mantissa bits."""
    import ml_dtypes
    hi = a.astype(ml_dtypes.float8_e4m3)
    lo = (a - hi.astype(np.float32)).astype(ml_dtypes.float8_e4m3)
    return hi, lo


def make_core_inputs(x_core, wqkv_core, wout_core, b_core=None):
    """Quantized inputs for ONE core: x_core [T, D], wqkv_core [D, 1536]
    (q|k|v grouped), wout_core [512, D]."""
    xh, xl = _split8(np.ascontiguousarray(x_core.T))
    wqh, wql = _split8(np.ascontiguousarray(wqkv_core) * WSCALE)
    wouth, woutl = _split8(np.ascontiguousarray(wout_core) * WSCALE)
    m = {
        "xh": xh, "xl": xl, "wqh": wqh, "wql": wql,
        "wouth": wouth, "woutl": woutl,
    }
    if b_core is not None:
        m["bqkv"] = np.ascontiguousarray(b_core * WSCALE).reshape(
            1, -1).astype(np.float32)
    return m


def make_in_maps(x, w_qkv, b_qkv, w_out, with_bias):
    """Per-core input dicts: core c -> batch c//2, head group c%2."""
    in_maps = []
    for core in range(8):
        b, gr = divmod(core, 2)
        qc = slice(512 * gr, 512 * (gr + 1))
        kc = slice(D + 512 * gr, D + 512 * (gr + 1))
        vc = slice(2 * D + 512 * gr, 2 * D + 512 * (gr + 1))
        wq = np.concatenate([w_qkv[:, qc], w_qkv[:, kc], w_qkv[:, vc]], axis=1)
        bq = (np.concatenate([b_qkv[qc], b_qkv[kc], b_qkv[vc]])
              if with_bias else None)
        in_maps.append(make_core_inputs(
            x[b], wq, w_out[512 * gr:512 * (gr + 1), :], bq))
    return in_maps


def kernel(x, w_qkv, b_qkv, w_out, b_out):
    x = np.asarray(x, dtype=np.float32)
    w_qkv = np.asarray(w_qkv, dtype=np.float32)
    b_qkv = np.asarray(b_qkv, dtype=np.float32)
    w_out = np.asarray(w_out, dtype=np.float32)
    b_out = np.asarray(b_out, dtype=np.float32)

    with_bias = bool(np.any(b_qkv))
    nc = _get_program(with_bias)
    in_maps = make_in_maps(x, w_qkv, b_qkv, w_out, with_bias)
    res = run_bass_kernel_spmd(nc, in_maps, core_ids=list(range(8))).results

    out = np.empty((B, T, D), dtype=np.float32)
    for b in range(B):
        out[b] = ((res[2 * b]["out"].astype(np.float32)
                   + res[2 * b + 1]["out"].astype(np.float32)) * OUT_SCALE
                  + b_out[None, :])
    return out



# revision 76
# speedup vs baseline: 1.0171x; 1.0027x over previous
"""Causal self-attention (B=4, T=2048, D=1024, H=16, hd=64) on 8 trn2 cores.

Sharding: 4-way data parallel over batch x 2-way tensor parallel over heads.
Core c handles batch c//2 and heads [8*(c%2), 8*(c%2)+8). Each core computes
its heads' partial contribution to the out-projection (a full [T, D] tensor);
the host sums the two head-group partials per batch and adds b_out.

Host-side prep: x is transposed to xT [D, T] and split into fp8(e4m3)
hi + lo residual planes; w_qkv (x32) and w_out (x32) likewise. The QKV and
out-projections then run as fp8 DoubleRow matmuls (2 K-chunks per pass, 0.5
cycles/row = 4x bf16 MAC rate) with three hi/lo cross terms (hi*hi + hi*lo +
lo*hi, the lo*lo term dropped), which is 0.75x the bf16 cost at ~bf16
accuracy. The weight x32 scaling keeps the lo planes out of fp8's subnormal
range; the softmax scale absorbs 1/1024 and the host divides the output
partials by 1024. End-to-end rel err vs the fp32 reference is ~4.4e-3
(gate: 2e-2).

Per-core dataflow (single NeuronCore, Tile framework):
  1. xT hi/lo stream straight from DRAM into SBUF (no on-device work).
  2. qT/kT [hd,T] are computed head-PAIR-stacked ([128,T] per pair) with
     w_qkv chunks as stationary DoubleRow fp8 matmuls; V [T,hd] per head is
     computed naturally and scattered into v1 (64 cols per (head, k-tile),
     bf16).
  3. Scores are computed TRANSPOSED: S^T[k,q] = K @ Q^T via two K=64 matmuls
     per (pair, k-tile) into one [128, 2, 512] two-bank PSUM tile; ONE wide
     exp (ACT, scale folded in) produces P^T for both heads in bf16.
     Causality: k-tiles above the diagonal are skipped, diagonal blocks get
     an affine_select zero-mask after exp, partial k-tiles use narrower
     matmuls.
  4. AV runs NON-transposed: per (head, q-subtile 128) the P^T block is the
     STATIONARY operand and V [128,64] streams, so each matmul moves 64
     rows instead of 512. All 8 (head, subtile) accumulators of a pair pack
     into ONE psum bank (zero-on-first-touch after a single start=True).
     Softmax sums ride as 1-column matmuls (P^T stationary, ones moving)
     into a shared, DVE-zeroed sums bank; AV emission trails its exp by 8
     k-tile units (carry) so the PE never waits on ACT latency.
  5. Normalization is a DVE broadcast multiply (reciprocal of sums, then
     attn * recip -> bf16). The normalized attn [q, 512] is PE-transposed
     (bf16), then quantized to an fp8 hi/lo pair (DVE copy + subtract; the
     hi copy moves to ACT in the drain where DVE is the serial bottleneck)
     as the out-projection's stationary operand; out = attnT.T @ w_out
     accumulates as 6 DoubleRow matmuls over 2 feature-chunk-pairs x 3
     hi/lo terms. All out-projections are deferred into the last q-group's
     phase + final drain, which are otherwise exp(ACT)-bound.

Scheduling: emission-level software pipelining interleaves attention for
q-group g with the transposes + q/k projections of group g+1; each group's
V chains ride at the FRONT of its own phase (their only consumer, the AV
flush, is emitted safely later). PSUM (8 banks) is budgeted as 4 scores +
2 AV + 1 sums + 1 projection scratch; the prologue additionally uses the
score slots' second banks for 7 concurrent q/k accumulators, and the final
drain widens the scratch rotation to 5. The cost model serializes all DMA
transfers through one lane fed by two descriptor-gen queues (HWDGE ~625ns,
SWDGE ~1.2us per dma_start), so the prologue issues transfers in exact
consumption order and gates the eager SWDGE ring (later-group x, w_out)
behind a Pool-engine copy of the last group-0 x tile; dummy matmuls at t=0
cover the first DMA latency and the PE p-state ramp (full clock needs 3us
of continuous execution - any >100ns gap drops it to half rate).
"""

import numpy as np

import concourse.bass as bass
import concourse.mybir as mybir
import concourse.tile as tile
from concourse import bacc
from concourse.bass_utils import run_bass_kernel_spmd
from concourse.masks import make_identity

B, T, D = 4, 2048, 1024
H, HD = 16, 64
HPC = 8              # heads per core
PAIRS = HPC // 2
CH = D // 128        # K-chunks for the QKV projection
TG = 512             # T-group / q-group width
SCALE = 1.0 / 8.0    # 1/sqrt(HD)
NDUMMY = 20          # prologue PE-warm dummy matmuls

F32 = mybir.dt.float32
F32R = mybir.dt.float32r
BF16 = mybir.dt.bfloat16
FP8 = mybir.dt.float8e4
DR = mybir.MatmulPerfMode.DoubleRow
AF = mybir.ActivationFunctionType

# w_qkv and w_out are scaled by 2**5 on the host so their fp8(e4m3) hi/lo
# splits stay out of the subnormal range; q', k', v' come out 32x larger, so
# the softmax scale absorbs 1/(32*32) and the device output is 32*32x the
# true projection (divided back out on the host).
WSCALE = 32.0
OUT_SCALE = 1.0 / (WSCALE * WSCALE)


def _r(ap):
    return ap.bitcast(F32R)


def build_tile_program(tc, xh, xl, wqh, wql, wouth, woutl, out, bqkv=None,
                       seq_len=T):
    """Emit the per-core program. seq_len is parametrized for small-scale
    simulation tests; the real kernel uses seq_len=T=2048.

    Emission is software-pipelined: attention for q-group g (latency-bound
    serial chains sT -> exp -> mask -> AV) is interleaved at emission level
    with the transposes + QKV projection of group g+1 (dense PE work), so the
    in-order PE fills attention's dependency stalls with projection matmuls.
    """
    nc = tc.nc
    n_tg = seq_len // TG
    n_tt = seq_len // 128
    with_bias = bqkv is not None

    from contextlib import ExitStack
    with ExitStack() as ctx:
        const = ctx.enter_context(tc.tile_pool(name="const", bufs=1))
        wpool = ctx.enter_context(tc.tile_pool(name="wpool", bufs=1))
        xt_pool = ctx.enter_context(tc.tile_pool(name="xt", bufs=4))
        qt_pool = ctx.enter_context(tc.tile_pool(name="qt", bufs=2 * PAIRS))
        pt_pool = ctx.enter_context(tc.tile_pool(name="pt", bufs=14))
        an_pool = ctx.enter_context(tc.tile_pool(name="an", bufs=16))
        at_pool = ctx.enter_context(tc.tile_pool(name="at", bufs=16))
        ob_pool = ctx.enter_context(tc.tile_pool(name="ob", bufs=6))
        rc_pool = ctx.enter_context(tc.tile_pool(name="rc", bufs=2))
        mm_ps = ctx.enter_context(tc.tile_pool(name="mmps", bufs=1, space="PSUM"))
        st_ps = ctx.enter_context(tc.tile_pool(name="stps", bufs=2, space="PSUM"))
        av_ps = ctx.enter_context(tc.tile_pool(name="avps", bufs=2, space="PSUM"))
        sums_ps = ctx.enter_context(tc.tile_pool(name="sums", bufs=1, space="PSUM"))
        identity = const.tile([128, 128], F32, tag="ident")
        ident_bf = const.tile([128, 128], BF16, tag="identbf")
        ones_bf = const.tile([128, 1], BF16, tag="ones_bf")

        def make_consts():
            # deferred so the weight SWDGE generation isn't queued behind
            # make_identity on the Pool engine at t=0; the identity is only
            # needed by the (late) attnT transposes
            make_identity(nc, identity[:])
            nc.vector.tensor_copy(out=ident_bf[:], in_=identity[:])
            nc.vector.memset(ones_bf[:], 1.0)
        if with_bias:
            ones_f = const.tile([128, 64], F32, tag="ones_f")
            nc.vector.memset(ones_f[:], 1.0)
            b_sb = const.tile([1, 3 * HPC * HD], F32R, tag="bias")
            nc.sync.dma_start(out=b_sb[:], in_=bqkv.bitcast(F32R)[:])
            ones_row = const.tile([1, TG], F32R, tag="ones_row")
            nc.vector.tensor_copy(
                out=ones_row[:], in_=ones_f[0:1, 0:1].broadcast_to([1, TG]))

        # softmax sums: one persistent bank; cols = parity*32 + pair*8 + h*4+s
        sums = sums_ps.tile([128, 64], F32, tag="sums")

        # resident weights (fp8 hi/lo pairs); w_out pair-packed: pair p rows
        # [128p, 128p+128). The DMAs are deferred until after group 0's x
        # loads so the first transposes aren't queued behind the weight
        # traffic.
        wh_sb = wpool.tile([128, CH, 3 * HPC * HD], FP8, tag="wqh")
        wl_sb = wpool.tile([128, CH, 3 * HPC * HD], FP8, tag="wql")
        # wout fp8 hi/lo, chunk-PAIR packed for DoubleRow: tile cp holds rows
        # [256cp, 256cp+256) as [128, 2, D]
        wouth_sb = [wpool.tile([128, 2, D], FP8, tag=f"wouth{p}",
                               name=f"wouth{p}") for p in range(2)]
        woutl_sb = [wpool.tile([128, 2, D], FP8, tag=f"woutl{p}",
                               name=f"woutl{p}") for p in range(2)]

        def load_weights():
            # The cost model serializes ALL transfers through one DMA_ENGINES
            # lane (~bytes/360GB/s each), fed by two descriptor-gen lanes:
            # HWDGE (shared SP+ACT, 625ns/dma) and SWDGE (gpsimd, ~1.1us/dma).
            # The prologue is transfer-order-bound and the PE is in-order, so
            # transfers are issued in exact consumption order of the prologue
            # units (q chains h0/h1, k chains h0/h1, then V), with q/k weight
            # columns split so q unblocks first. All on the scalar HWDGE
            # queue; x rides the sync HWDGE queue; later-group x and wout
            # ride SWDGE.
            for c0 in (0, 4):
                for w_sb, wsrc in ((wh_sb, wqh), (wl_sb, wql)):
                    nc.scalar.dma_start(
                        out=w_sb[:, c0:c0 + 4, 0:1024],
                        in_=wsrc[128 * c0:128 * (c0 + 4), 0:1024].rearrange(
                            "(c p) n -> p c n", p=128))
            for w_sb, wsrc in ((wh_sb, wqh), (wl_sb, wql)):
                for c0 in range(0, CH, 4):
                    nc.scalar.dma_start(
                        out=w_sb[:, c0:c0 + 4, 1024:1536],
                        in_=wsrc[128 * c0:128 * (c0 + 4), 1024:1536].rearrange(
                            "(c p) n -> p c n", p=128))

        def load_wout():
            # deferred into the phase-0 fill (behind the group-1 x loads on
            # SWDGE): first consumed by the deferred group-0 out-projections
            # in phase 1 (~45us in)
            with tc.tile_wait_until(0.016):
                for cp in range(2):
                    for w_sb, wsrc in ((wouth_sb, wouth), (woutl_sb, woutl)):
                        nc.gpsimd.dma_start(
                            out=w_sb[cp][:],
                            in_=wsrc[256 * cp:256 * (cp + 1), :].rearrange(
                                "(c p) n -> p c n", p=128))
        # kT per pair, head 2p on partitions [0:64), head 2p+1 on [64:128)
        kts = [wpool.tile([128, seq_len], BF16, tag=f"kt{p}", name=f"kt{p}")
               for p in range(PAIRS)]
        # V in bf16: per (head, k-tile) a [128, 64] stationary block
        v1 = wpool.tile([128, HPC, n_tt, HD], BF16, tag="v1")

        qts_of = {}  # g -> [qt tiles per pair]

        # psum scratch for the projection/transpose/oproj chains. In steady
        # state only the mm bank is free; in the prologue and final drain the
        # scores/AV banks are idle, so rotate through them too (the st slot is
        # [128,2,512]; its first bank is used as a [128,512] scratch).
        # "banked" mode (prologue only, F32) additionally uses the st slots'
        # SECOND psum banks (zero regions are 2KB, so the two banks of one
        # slot hold independent accumulation groups), giving 7 concurrent
        # scratch accumulators for the q+k chain overlap.
        _ps_state = {"wide": False, "banked": False, "i": 0, "st": {}}

        def set_wide_scratch(wide, banked=False):
            _ps_state["wide"] = wide
            _ps_state["banked"] = banked

        def scratch_ps(dtype):
            if not _ps_state["wide"]:
                return mm_ps.tile([128, 512], dtype, tag="mm", name="mm")
            if _ps_state["banked"] and dtype == F32:
                i = _ps_state["i"] = (_ps_state["i"] + 1) % 7
                if i == 0:
                    return mm_ps.tile([128, 512], dtype, tag="mm", name="mm")
                if i in (1, 2, 3, 4):
                    buf, bank = divmod(i - 1, 2)
                    if bank == 0:
                        _ps_state["st"][buf] = st_ps.tile(
                            [128, 2, 512], dtype, tag="st", name="stx")
                    return _ps_state["st"][buf][:, bank, :]
                return av_ps.tile([128, 512], dtype, tag="av", name="avx")
            i = _ps_state["i"] = (_ps_state["i"] + 1) % 5
            if i == 0:
                return mm_ps.tile([128, 512], dtype, tag="mm", name="mm")
            if i in (1, 2):
                return st_ps.tile([128, 512], dtype, tag="st", name="stx")
            return av_ps.tile([128, 512], dtype, tag="av", name="avx")

        def transpose_units(g):
            """x arrives pre-transposed and fp8-hi/lo-split from the host:
            just DMA the group's xT columns. Group 0 (prologue, latency-
            critical) goes on the sync HWDGE queue split in half-chunks;
            later groups ride the then-idle SWDGE ring as one transfer per
            hi/lo half."""
            xt = (xt_pool.tile([128, CH, TG], FP8, tag="xth", name=f"xth{g}"),
                  xt_pool.tile([128, CH, TG], FP8, tag="xtl", name=f"xtl{g}"))
            gsl = slice(g * TG, (g + 1) * TG)

            def u():
                if g == 0:
                    # lo directly after hi per chunk-half: every chain's
                    # third (lo*hi) term needs x-lo early
                    for c0 in (0, 4):
                        for t, src in ((xt[0], xh), (xt[1], xl)):
                            nc.sync.dma_start(
                                out=t[:, c0:c0 + 4, :],
                                in_=src[128 * c0:128 * (c0 + 4), gsl].rearrange(
                                    "(c p) n -> p c n", p=128))
                else:
                    # wait-ts biases the tile scheduler so these preps don't
                    # jump the serial DMA lane ahead of the prologue-critical
                    # weight transfers
                    with tc.tile_wait_until(0.010 + 0.038 * (g - 1)):
                        for t, src in ((xt[0], xh), (xt[1], xl)):
                            nc.gpsimd.dma_start(
                                out=t[:, :, :],
                                in_=src[:, gsl].rearrange(
                                    "(c p) n -> p c n", p=128))
            return xt, [u]

        def qkv_units(g, xt):
            """12 units: 4 qt chains, 4 kT chains, 4 V chains."""
            qts = qts_of.setdefault(g, [])
            units = []

            half = {}

            xth, xtl = xt

            def qk_chain(p, qk, h):
                # split into two half-chains (finer interleave granularity).
                # Each half contracts 4 D-chunks = 2 fp8 DoubleRow chunk-pairs
                # x 3 hi/lo cross terms (lo*lo dropped). Terms are ordered
                # hi*hi first so the prologue chains can start as soon as the
                # hi transfers land (lo halves trickle in later).
                if h == 0:
                    half[(p, qk)] = scratch_ps(F32)
                ps = half[(p, qk)]
                col = qk * 512 + p * 128
                first = True
                for wt, xt_ in ((wh_sb, xth), (wl_sb, xth), (wh_sb, xtl)):
                    for c0 in range(4 * h, 4 * h + 4, 2):
                        cs = slice(c0, c0 + 2)
                        nc.tensor.matmul(
                            ps[:, :TG], wt[:, cs, col:col + 128],
                            xt_[:, cs, :],
                            start=(h == 0 and first),
                            stop=(h == 1 and c0 == CH - 2 and xt_ is xtl
                                  and not with_bias),
                            perf_mode=DR)
                        first = False
                if h == 0:
                    return
                if with_bias:
                    nc.tensor.matmul(
                        ps[:, :TG], b_sb[0:1, col:col + 128],
                        ones_row[0:1, :], start=False, stop=True)
                if qk == 0:
                    qt = qt_pool.tile([128, TG], BF16, tag="qt")
                    nc.vector.tensor_copy(out=qt[:], in_=ps[:, :TG])
                    qts.append(qt)
                else:
                    nc.vector.tensor_copy(
                        out=kts[p][:, g * TG:(g + 1) * TG], in_=ps[:, :TG])

            def v_chain(t4):
                tt = g * (TG // 128) + t4
                ps = scratch_ps(F32)
                first = True
                tsl = slice(128 * t4, 128 * (t4 + 1))
                for xt_, wt in ((xth, wh_sb), (xth, wl_sb), (xtl, wh_sb)):
                    for c0 in range(0, CH, 2):
                        cs = slice(c0, c0 + 2)
                        nc.tensor.matmul(
                            ps[:, :512], xt_[:, cs, tsl],
                            wt[:, cs, 1024:1536],
                            start=first,
                            stop=(c0 == CH - 2 and xt_ is xtl
                                  and not with_bias),
                            perf_mode=DR)
                        first = False
                if with_bias:
                    nc.tensor.matmul(
                        ps[:, :512], ones_row[0:1, 0:128],
                        b_sb[0:1, 1024:1536], start=False, stop=True)
                nc.vector.tensor_copy(
                    out=v1[:, :, tt, :],
                    in_=ps[:, :512].rearrange("p (h d) -> p h d", h=HPC))

            k_units = []
            v_units = []
            # in the prologue the qk chains are gated on their W chunks
            # landing (~1us apart): run all h=0 half-chains (chunks 0-3)
            # before any h=1 (chunks 4-7). Requires 4 concurrent psum
            # accumulators - only legal in the wide-scratch prologue.
            hmajor = _ps_state["wide"]
            for lst, qk in ((units, 0), (k_units, 1)):
                if hmajor:
                    for h in range(2):
                        for p in range(PAIRS):
                            lst.append(lambda p=p, h=h, qk=qk: qk_chain(p, qk, h))
                else:
                    for p in range(PAIRS):
                        lst.append(lambda p=p, qk=qk: qk_chain(p, qk, 0))
                        lst.append(lambda p=p, qk=qk: qk_chain(p, qk, 1))
            for t4 in range(TG // 128):
                v_units.append(lambda t4=t4: v_chain(t4))
            return units, k_units, v_units

        def attention_units(g):
            """Per pair: one unit per k-tile (sT+exp+mask, AV carried by one),
            then a normalization unit; finally the out-projection units."""
            units = []
            pending_finish = []
            an_tiles = [an_pool.tile([128, 512], BF16, tag="an",
                                     name=f"an_g{g}_s{s}")
                        for s in range(TG // 128)]
            # zero this group's sums columns once (all 4 pairs' 8-col slices)
            nc.vector.memset(sums[:, (g % 2) * 32:(g % 2) * 32 + 32], 0.0)
            qts = qts_of[g]
            for p in range(PAIRS):
                nkt = 4 * (g + 1)
                soff = (g % 2) * 32 + p * 8
                state = {}

                def start_pair(p=p, state=state, soff=soff):
                    state["av"] = av_ps.tile([128, 512], F32, tag="av",
                                             name=f"av_g{g}_p{p}")
                    state["carry"] = []
                    state["first"] = True

                def kt_unit(kt, pos, p=p, state=state, nkt=nkt,
                            sp=start_pair):
                    if pos == 0:
                        sp()
                    # pop the carried AV FIRST: it is ready now, and the
                    # scores below may head-block on a busy st slot
                    if len(state["carry"]) >= 9:
                        state["emit_av"](*state["carry"].pop(0))
                    qt = qts[p]
                    rdiag = kt - 4 * g
                    col0 = 128 * rdiag if rdiag > 0 else 0
                    ksl = slice(128 * kt, 128 * (kt + 1))
                    st = st_ps.tile([128, 2, 512], F32, tag="st")
                    nc.tensor.matmul(
                        st[:, 0, col0:], kts[p][0:64, ksl], qt[0:64, col0:])
                    nc.tensor.matmul(
                        st[:, 1, col0:], kts[p][64:128, ksl], qt[64:128, col0:])
                    pt = pt_pool.tile([128, 2, 512], BF16, tag="pt")
                    nc.scalar.activation(
                        pt[:, :, col0:], st[:, :, col0:], AF.Exp,
                        scale=SCALE / (WSCALE * WSCALE))
                    if rdiag >= 0:
                        # keep P^T[k, q] only where q >= k (within-block);
                        # one select covers both heads via a stride-0 dim
                        nc.gpsimd.affine_select(
                            out=pt[:, :, col0:col0 + 128],
                            in_=pt[:, :, col0:col0 + 128],
                            compare_op=mybir.AluOpType.is_ge,
                            fill=0.0, base=0, pattern=[[0, 2], [1, 128]],
                            channel_multiplier=-1)
                    state["carry"].append((kt, pos, rdiag, pt))

                def emit_av(kt, pos, rdiag, pt, p=p, state=state, nkt=nkt,
                            soff=soff):
                    av = state["av"]
                    s0 = max(rdiag, 0)
                    last = pos == nkt - 1
                    for s in range(s0, TG // 128):
                        for h in (0, 1):
                            nc.tensor.matmul(
                                av[:, 128 * s + 64 * h:128 * s + 64 * h + 64],
                                pt[:, h, 128 * s:128 * (s + 1)],
                                v1[:, 2 * p + h, kt, :],
                                start=state["first"],
                                stop=(last and s == 3 and h == 1))
                            state["first"] = False
                            nc.tensor.matmul(
                                sums[:, soff + 4 * h + s:soff + 4 * h + s + 1],
                                pt[:, h, 128 * s:128 * (s + 1)],
                                ones_bf[:],
                                start=False, stop=False,
                                skip_group_check=True)

                state["emit_av"] = emit_av

                def emit_block(mm, state, p, soff, stop_last):
                    av = state["av"]
                    for i, (isd, kt, pt, s, h) in enumerate(mm):
                        nc.tensor.matmul(
                            av[:, 128 * s + 64 * h:128 * s + 64 * h + 64],
                            pt[:, h, 128 * s:128 * (s + 1)],
                            v1[:, 2 * p + h, kt, :],
                            start=state["first"],
                            stop=(stop_last and i == len(mm) - 1))
                        state["first"] = False
                        nc.tensor.matmul(
                            sums[:, soff + 4 * h + s:soff + 4 * h + s + 1],
                            pt[:, h, 128 * s:128 * (s + 1)],
                            ones_bf[:],
                            start=False, stop=False,
                            skip_group_check=True)

                def flush_unit(p=p, state=state, soff=soff):
                    # flush the carried non-diagonal AVs at pair end; the
                    # affine-masked diagonal blocks wait for Pool latency, so
                    # they are deferred into finish_unit (emitted two k-tile
                    # units into the NEXT pair) to avoid head-blocking the
                    # PE's 4-deep dependency wait queue
                    mm = []
                    for kt, pos, rdiag, pt in state["carry"]:
                        for s in range(max(rdiag, 0), TG // 128):
                            for h in (0, 1):
                                mm.append((s == rdiag, kt, pt, s, h))
                    state["carry"] = []
                    mm.sort(key=lambda t: t[0])
                    ndiag = sum(1 for t in mm if t[0])
                    split = len(mm) - ndiag
                    emit_block(mm[:split], state, p, soff, stop_last=False)
                    state["diag"] = mm[split:]

                def norm_unit(p=p, state=state, soff=soff):
                    emit_block(state.pop("diag"), state, p, soff,
                               stop_last=True)
                    av = state["av"]
                    rc = rc_pool.tile([128, 2, 4], F32, tag="rc")
                    nc.vector.reciprocal(
                        rc[:], sums[:, soff:soff + 8].rearrange(
                            "p (h s) -> p h s", h=2))
                    for s in range(TG // 128):
                        nc.vector.tensor_mul(
                            an_tiles[s][:, 128 * p:128 * (p + 1)].rearrange(
                                "p (h d) -> p h d", h=2),
                            av[:, 128 * s:128 * (s + 1)].rearrange(
                                "p (h d) -> p h d", h=2),
                            rc[:, :, s:s + 1].broadcast_to([128, 2, 64]))

                pair_units = []
                for pos, kt in enumerate(range(nkt)):
                    pair_units.append(
                        lambda kt=kt, pos=pos, f=kt_unit: f(kt, pos))
                if pending_finish:
                    pair_units.insert(min(6, max(2, len(pair_units) - 2)),
                                      pending_finish.pop())
                units += pair_units
                units.append(flush_unit)
                pending_finish.append(norm_unit)

            if pending_finish:
                units.append(pending_finish.pop())

            ats = {}

            def trans_unit(s):
                ps = scratch_ps(BF16)
                for c in range(PAIRS):
                    nc.tensor.transpose(
                        ps[:, 128 * c:128 * (c + 1)],
                        an_tiles[s][:, 128 * c:128 * (c + 1)],
                        ident_bf[:])
                # fp8 hi/lo split of attnT for the DoubleRow out-projection.
                # The hi copy runs on the (mostly idle) Pool engine so the
                # DVE isn't the serial bottleneck of the projection drain.
                ath = at_pool.tile([128, 512], FP8, tag="ath")
                atl = at_pool.tile([128, 512], FP8, tag="atl")
                if _ps_state["wide"]:
                    # final drain: ACT is idle and DVE is the serial
                    # bottleneck, so split the quant pair across both
                    nc.scalar.copy(ath[:], ps[:])
                else:
                    nc.vector.tensor_copy(out=ath[:], in_=ps[:])
                nc.vector.tensor_sub(atl[:], ps[:], ath[:])
                ats[s] = (ath, atl)

            def oproj_unit(s, nh):
                row0 = g * TG + 128 * s
                ath, atl = ats[s]
                ps = scratch_ps(F32)
                nsl = slice(512 * nh, 512 * (nh + 1))
                first = True
                for at_, wo in ((ath, wouth_sb), (ath, woutl_sb),
                                (atl, wouth_sb)):
                    for cp in range(2):
                        nc.tensor.matmul(
                            ps[:, :512],
                            at_[:, 256 * cp:256 * (cp + 1)].rearrange(
                                "p (c m) -> p c m", c=2),
                            wo[cp][:, :, nsl],
                            start=first,
                            stop=(at_ is atl and cp == 1),
                            perf_mode=DR)
                        first = False
                ob = ob_pool.tile([128, 512], BF16, tag="ob")
                if _ps_state["wide"] and (s + nh) % 2 == 0:
                    nc.scalar.copy(ob[:], ps[:, :512])
                else:
                    nc.vector.tensor_copy(out=ob[:], in_=ps[:, :512])
                # in the final drain the sync HWDGE queue serializes the last
                # 8 output stores (~625ns descriptor-gen each); spread them
                # over the idle queues so the tail isn't gated on it
                eng = (nc.sync if not _ps_state["wide"] else
                       (nc.sync, nc.scalar, nc.gpsimd)[(3 - s + nh) % 3])
                eng.dma_start(
                    out=out[row0:row0 + 128, 512 * nh:512 * (nh + 1)],
                    in_=ob[:])

            ounits = []
            for s in range(TG // 128):
                ounits.append(lambda s=s: trans_unit(s))
            for s in range(TG // 128):
                for nh in range(2):
                    ounits.append(lambda s=s, nh=nh: oproj_unit(s, nh))
            return units, ounits

        def interleave(a_units, b_units, pre_b=0):
            # pre_b: emit that many b-units before any a-unit (phase 0 needs
            # the group-0 V chains emitted before the first AV flush so the
            # tile framework sees the writes first)
            for u in b_units[:pre_b]:
                u()
            b_units = b_units[pre_b:]
            na, nb = len(a_units), len(b_units)
            ia = ib = 0
            while ia < na or ib < nb:
                fa = (na - ia) / na if na else 0.0
                fb = (nb - ib) / nb if nb else 0.0
                if ia < na and (fa > fb or ib >= nb):
                    a_units[ia]()
                    ia += 1
                else:
                    b_units[ib]()
                    ib += 1

        # prologue: group 0 projection (weight DMAs after group 0's x loads).
        # The scores/AV banks are idle here, so scratch rotates through them.
        set_wide_scratch(True, banked=True)
        # fill the initial x/w DMA latency with dummy PE work (also completes
        # the tensor engine's p-state ramp before real work lands); plain
        # matmuls on a memset tile need no identity, so they start ~1us in
        dummy = const.tile([128, 256], BF16, tag="dummy")
        nc.vector.memset(dummy[:], 0.0)
        xt0, tunits0 = transpose_units(0)
        for u in tunits0:
            u()
        load_weights()
        make_consts()
        # SWDGE gate: the gpsimd descriptor ring would otherwise eagerly
        # prepare the later-group x / wout transfers at t~1us and their
        # transfers would jump the (serial) DMA lane ahead of the
        # prologue-critical weight transfers. A tiny Pool-engine copy that
        # waits on the first x tile holds the ring back until the prologue
        # transfers are in flight.
        gate = const.tile([1, 1], FP8, tag="gate")
        nc.gpsimd.tensor_copy(out=gate[:], in_=xt0[1][0:1, CH - 1, 0:1])
        for _ in range(NDUMMY):
            wps = scratch_ps(F32)
            nc.tensor.matmul(wps[:, :256], dummy[:, 0:128], dummy[:],
                             start=True, stop=True)
        # prologue runs only the q and k chains (their weights land first);
        # group 0's V chains are deferred into the phase-0 fill, where they
        # interleave with the (V-independent) score units while the V-column
        # weights land. Unit order follows the DMA landing order: all h=0
        # halves (chunks 0-3) before h=1 (chunks 4-7). This holds up to 7
        # open psum accumulations (4 q + 3 k), which the banked scratch
        # rotation provides; k p3 waits until the q psums close.
        _ps_state["i"] = 0
        q0, k0, v0 = qkv_units(0, xt0)
        for u in [q0[0], q0[1], q0[2], q0[3],      # q h0 p0-3
                  k0[0], k0[1], k0[2],             # k h0 p0-2
                  q0[4], q0[5], q0[6], q0[7],      # q h1 p0-3 (close q psums)
                  k0[3],                           # k h0 p3
                  k0[4], k0[5], k0[6], k0[7]]:     # k h1 p0-3
            u()
        set_wide_scratch(False)
        # steady state: attention(g) interleaved with transposes(g+1) +
        # projection(g+1); out-projections are deferred up to two groups so
        # the last (largest) attention group still has dense PE fill
        pending_oproj = []  # deferred out-projection unit lists, oldest first
        fill_carry = v0     # group g's V chains lead the phase-g fill: their
        # only consumer is the AV flush (safely late in emission), and the
        # ACT-bound later phases need the q/k fill to shrink, not grow
        for g in range(n_tg):
            attn, ounits = attention_units(g)
            fill = fill_carry
            fill_carry = []
            if g + 1 < n_tg:
                xt1, tunits = transpose_units(g + 1)
                qu, ku, vu = qkv_units(g + 1, xt1)
                fill += tunits + ([load_wout] if g == 0 else []) + qu + ku
                fill_carry = vu
            if False and pending_oproj:
                # phase 2 is ACT(exp)-bound with PE slack: give it the
                # oldest deferred out-projection group
                fill += pending_oproj.pop(0)
            if g + 1 == n_tg:
                if g == 0:
                    fill.append(load_wout)
                # last group is exp(ACT)-bound and has no next-group
                # projection: feed it the remaining deferred out-projections
                while pending_oproj:
                    fill += pending_oproj.pop(0)
            interleave(attn, fill, pre_b=len(v0) if g == 0 else 0)
            pending_oproj.append(ounits)
        # final drain: attention is done, scores/AV banks are idle again
        set_wide_scratch(True)
        for ou in pending_oproj:
            for u in ou:
                u()


def build_program(with_bias, seq_len=T):
    nc = bacc.Bacc("TRN2", target_bir_lowering=False, debug=False,
                   enable_asserts=False, num_devices=8)
    xh = nc.dram_tensor("xh", [D, seq_len], FP8, kind="ExternalInput").ap()
    xl = nc.dram_tensor("xl", [D, seq_len], FP8, kind="ExternalInput").ap()
    wqh = nc.dram_tensor("wqh", [D, 3 * HPC * HD], FP8,
                         kind="ExternalInput").ap()
    wql = nc.dram_tensor("wql", [D, 3 * HPC * HD], FP8,
                         kind="ExternalInput").ap()
    wouth = nc.dram_tensor("wouth", [HPC * HD, D], FP8,
                           kind="ExternalInput").ap()
    woutl = nc.dram_tensor("woutl", [HPC * HD, D], FP8,
                           kind="ExternalInput").ap()
    out = nc.dram_tensor("out", [seq_len, D], BF16,
                         kind="ExternalOutput").ap()
    bqkv = None
    if with_bias:
        bqkv = nc.dram_tensor("bqkv", [1, 3 * HPC * HD], F32,
                              kind="ExternalInput").ap()
    with tile.TileContext(nc) as tc:
        build_tile_program(tc, xh, xl, wqh, wql, wouth, woutl, out, bqkv,
                           seq_len=seq_len)
    nc.compile()
    return nc


_PROGRAM_CACHE = {}


def _get_program(with_bias):
    if with_bias not in _PROGRAM_CACHE:
        _PROGRAM_CACHE[with_bias] = build_program(with_bias)
    return _PROGRAM_CACHE[with_bias]


def _split8(a):
    """fp8(e4m3) hi/lo split: a ~= hi + lo to ~7 mantissa bits."""
    import ml_dtypes
    hi = a.astype(ml_dtypes.float8_e4m3)
    lo = (a - hi.astype(np.float32)).astype(ml_dtypes.float8_e4m3)
    return hi, lo


def make_core_inputs(x_core, wqkv_core, wout_core, b_core=None):
    """Quantized inputs for ONE core: x_core [T, D], wqkv_core [D, 1536]
    (q|k|v grouped), wout_core [512, D]."""
    xh, xl = _split8(np.ascontiguousarray(x_core.T))
    wqh, wql = _split8(np.ascontiguousarray(wqkv_core) * WSCALE)
    wouth, woutl = _split8(np.ascontiguousarray(wout_core) * WSCALE)
    m = {
        "xh": xh, "xl": xl, "wqh": wqh, "wql": wql,
        "wouth": wouth, "woutl": woutl,
    }
    if b_core is not None:
        m["bqkv"] = np.ascontiguousarray(b_core * WSCALE).reshape(
            1, -1).astype(np.float32)
    return m


def make_in_maps(x, w_qkv, b_qkv, w_out, with_bias):
    """Per-core input dicts: core c -> batch c//2, head group c%2."""
    in_maps = []
    for core in range(8):
        b, gr = divmod(core, 2)
        qc = slice(512 * gr, 512 * (gr + 1))
        kc = slice(D + 512 * gr, D + 512 * (gr + 1))
        vc = slice(2 * D + 512 * gr, 2 * D + 512 * (gr + 1))
        wq = np.concatenate([w_qkv[:, qc], w_qkv[:, kc], w_qkv[:, vc]], axis=1)
        bq = (np.concatenate([b_qkv[qc], b_qkv[kc], b_qkv[vc]])
              if with_bias else None)
        in_maps.append(make_core_inputs(
            x[b], wq, w_out[512 * gr:512 * (gr + 1), :], bq))
    return in_maps


def kernel(x, w_qkv, b_qkv, w_out, b_out):
    x = np.asarray(x, dtype=np.float32)
    w_qkv = np.asarray(w_qkv, dtype=np.float32)
    b_qkv = np.asarray(b_qkv, dtype=np.float32)
    w_out = np.asarray(w_out, dtype=np.float32)
    b_out = np.asarray(b_out, dtype=np.float32)

    with_bias = bool(np.any(b_qkv))
    nc = _get_program(with_bias)
    in_maps = make_in_maps(x, w_qkv, b_qkv, w_out, with_bias)
    res = run_bass_kernel_spmd(nc, in_maps, core_ids=list(range(8))).results

    out = np.empty((B, T, D), dtype=np.float32)
    for b in range(B):
        out[b] = ((res[2 * b]["out"].astype(np.float32)
                   + res[2 * b + 1]["out"].astype(np.float32)) * OUT_SCALE
                  + b_out[None, :])
    return out



# revision 77
# speedup vs baseline: 1.0176x; 1.0004x over previous
"""Causal self-attention (B=4, T=2048, D=1024, H=16, hd=64) on 8 trn2 cores.

Sharding: 4-way data parallel over batch x 2-way tensor parallel over heads.
Core c handles batch c//2 and heads [8*(c%2), 8*(c%2)+8). Each core computes
its heads' partial contribution to the out-projection (a full [T, D] tensor);
the host sums the two head-group partials per batch and adds b_out.

Host-side prep: x is transposed to xT [D, T] and split into fp8(e4m3)
hi + lo residual planes; w_qkv (x32) and w_out (x32) likewise. The QKV and
out-projections then run as fp8 DoubleRow matmuls (2 K-chunks per pass, 0.5
cycles/row = 4x bf16 MAC rate) with three hi/lo cross terms (hi*hi + hi*lo +
lo*hi, the lo*lo term dropped), which is 0.75x the bf16 cost at ~bf16
accuracy. The weight x32 scaling keeps the lo planes out of fp8's subnormal
range; the softmax scale absorbs 1/1024 and the host divides the output
partials by 1024. End-to-end rel err vs the fp32 reference is ~4.4e-3
(gate: 2e-2).

Per-core dataflow (single NeuronCore, Tile framework):
  1. xT hi/lo stream straight from DRAM into SBUF (no on-device work).
  2. qT/kT [hd,T] are computed head-PAIR-stacked ([128,T] per pair) with
     w_qkv chunks as stationary DoubleRow fp8 matmuls; V [T,hd] per head is
     computed naturally and scattered into v1 (64 cols per (head, k-tile),
     bf16).
  3. Scores are computed TRANSPOSED: S^T[k,q] = K @ Q^T via two K=64 matmuls
     per (pair, k-tile) into one [128, 2, 512] two-bank PSUM tile; ONE wide
     exp (ACT, scale folded in) produces P^T for both heads in bf16.
     Causality: k-tiles above the diagonal are skipped, diagonal blocks get
     an affine_select zero-mask after exp, partial k-tiles use narrower
     matmuls.
  4. AV runs NON-transposed: per (head, q-subtile 128) the P^T block is the
     STATIONARY operand and V [128,64] streams, so each matmul moves 64
     rows instead of 512. All 8 (head, subtile) accumulators of a pair pack
     into ONE psum bank (zero-on-first-touch after a single start=True).
     Softmax sums ride as 1-column matmuls (P^T stationary, ones moving)
     into a shared, DVE-zeroed sums bank; AV emission trails its exp by 8
     k-tile units (carry) so the PE never waits on ACT latency.
  5. Normalization is a DVE broadcast multiply (reciprocal of sums, then
     attn * recip -> bf16). The normalized attn [q, 512] is PE-transposed
     (bf16), then quantized to an fp8 hi/lo pair (DVE copy + subtract; the
     hi copy moves to ACT in the drain where DVE is the serial bottleneck)
     as the out-projection's stationary operand; out = attnT.T @ w_out
     accumulates as 6 DoubleRow matmuls over 2 feature-chunk-pairs x 3
     hi/lo terms. All out-projections are deferred into the last q-group's
     phase + final drain, which are otherwise exp(ACT)-bound.

Scheduling: emission-level software pipelining interleaves attention for
q-group g with the transposes + q/k projections of group g+1; each group's
V chains ride at the FRONT of its own phase (their only consumer, the AV
flush, is emitted safely later). PSUM (8 banks) is budgeted as 4 scores +
2 AV + 1 sums + 1 projection scratch; the prologue additionally uses the
score slots' second banks for 7 concurrent q/k accumulators, and the final
drain widens the scratch rotation to 5. The cost model serializes all DMA
transfers through one lane fed by two descriptor-gen queues (HWDGE ~625ns,
SWDGE ~1.2us per dma_start), so the prologue issues transfers in exact
consumption order and gates the eager SWDGE ring (later-group x, w_out)
behind a Pool-engine copy of the last group-0 x tile; dummy matmuls at t=0
cover the first DMA latency and the PE p-state ramp (full clock needs 3us
of continuous execution - any >100ns gap drops it to half rate).
"""

import numpy as np

import concourse.bass as bass
import concourse.mybir as mybir
import concourse.tile as tile
from concourse import bacc
from concourse.bass_utils import run_bass_kernel_spmd
from concourse.masks import make_identity

B, T, D = 4, 2048, 1024
H, HD = 16, 64
HPC = 8              # heads per core
PAIRS = HPC // 2
CH = D // 128        # K-chunks for the QKV projection
TG = 512             # T-group / q-group width
SCALE = 1.0 / 8.0    # 1/sqrt(HD)
NDUMMY = 20          # prologue PE-warm dummy matmuls

F32 = mybir.dt.float32
F32R = mybir.dt.float32r
BF16 = mybir.dt.bfloat16
FP8 = mybir.dt.float8e4
DR = mybir.MatmulPerfMode.DoubleRow
AF = mybir.ActivationFunctionType

# w_qkv and w_out are scaled by 2**5 on the host so their fp8(e4m3) hi/lo
# splits stay out of the subnormal range; q', k', v' come out 32x larger, so
# the softmax scale absorbs 1/(32*32) and the device output is 32*32x the
# true projection (divided back out on the host).
WSCALE = 32.0
OUT_SCALE = 1.0 / (WSCALE * WSCALE)


def _r(ap):
    return ap.bitcast(F32R)


def build_tile_program(tc, xh, xl, wqh, wql, wouth, woutl, out, bqkv=None,
                       seq_len=T):
    """Emit the per-core program. seq_len is parametrized for small-scale
    simulation tests; the real kernel uses seq_len=T=2048.

    Emission is software-pipelined: attention for q-group g (latency-bound
    serial chains sT -> exp -> mask -> AV) is interleaved at emission level
    with the transposes + QKV projection of group g+1 (dense PE work), so the
    in-order PE fills attention's dependency stalls with projection matmuls.
    """
    nc = tc.nc
    n_tg = seq_len // TG
    n_tt = seq_len // 128
    with_bias = bqkv is not None

    from contextlib import ExitStack
    with ExitStack() as ctx:
        const = ctx.enter_context(tc.tile_pool(name="const", bufs=1))
        wpool = ctx.enter_context(tc.tile_pool(name="wpool", bufs=1))
        xt_pool = ctx.enter_context(tc.tile_pool(name="xt", bufs=4))
        qt_pool = ctx.enter_context(tc.tile_pool(name="qt", bufs=2 * PAIRS))
        pt_pool = ctx.enter_context(tc.tile_pool(name="pt", bufs=14))
        an_pool = ctx.enter_context(tc.tile_pool(name="an", bufs=16))
        at_pool = ctx.enter_context(tc.tile_pool(name="at", bufs=16))
        ob_pool = ctx.enter_context(tc.tile_pool(name="ob", bufs=6))
        rc_pool = ctx.enter_context(tc.tile_pool(name="rc", bufs=2))
        mm_ps = ctx.enter_context(tc.tile_pool(name="mmps", bufs=1, space="PSUM"))
        st_ps = ctx.enter_context(tc.tile_pool(name="stps", bufs=2, space="PSUM"))
        av_ps = ctx.enter_context(tc.tile_pool(name="avps", bufs=2, space="PSUM"))
        sums_ps = ctx.enter_context(tc.tile_pool(name="sums", bufs=1, space="PSUM"))
        identity = const.tile([128, 128], F32, tag="ident")
        ident_bf = const.tile([128, 128], BF16, tag="identbf")
        ones_bf = const.tile([128, 1], BF16, tag="ones_bf")

        def make_consts():
            # deferred so the weight SWDGE generation isn't queued behind
            # make_identity on the Pool engine at t=0; the identity is only
            # needed by the (late) attnT transposes
            make_identity(nc, identity[:])
            nc.vector.tensor_copy(out=ident_bf[:], in_=identity[:])
            nc.vector.memset(ones_bf[:], 1.0)
        if with_bias:
            ones_f = const.tile([128, 64], F32, tag="ones_f")
            nc.vector.memset(ones_f[:], 1.0)
            b_sb = const.tile([1, 3 * HPC * HD], F32R, tag="bias")
            nc.sync.dma_start(out=b_sb[:], in_=bqkv.bitcast(F32R)[:])
            ones_row = const.tile([1, TG], F32R, tag="ones_row")
            nc.vector.tensor_copy(
                out=ones_row[:], in_=ones_f[0:1, 0:1].broadcast_to([1, TG]))

        # softmax sums: one persistent bank; cols = parity*32 + pair*8 + h*4+s
        sums = sums_ps.tile([128, 64], F32, tag="sums")

        # resident weights (fp8 hi/lo pairs); w_out pair-packed: pair p rows
        # [128p, 128p+128). The DMAs are deferred until after group 0's x
        # loads so the first transposes aren't queued behind the weight
        # traffic.
        wh_sb = wpool.tile([128, CH, 3 * HPC * HD], FP8, tag="wqh")
        wl_sb = wpool.tile([128, CH, 3 * HPC * HD], FP8, tag="wql")
        # wout fp8 hi/lo, chunk-PAIR packed for DoubleRow: tile cp holds rows
        # [256cp, 256cp+256) as [128, 2, D]
        wouth_sb = [wpool.tile([128, 2, D], FP8, tag=f"wouth{p}",
                               name=f"wouth{p}") for p in range(2)]
        woutl_sb = [wpool.tile([128, 2, D], FP8, tag=f"woutl{p}",
                               name=f"woutl{p}") for p in range(2)]

        def load_weights():
            # The cost model serializes ALL transfers through one DMA_ENGINES
            # lane (~bytes/360GB/s each), fed by two descriptor-gen lanes:
            # HWDGE (shared SP+ACT, 625ns/dma) and SWDGE (gpsimd, ~1.1us/dma).
            # The prologue is transfer-order-bound and the PE is in-order, so
            # transfers are issued in exact consumption order of the prologue
            # units (q chains h0/h1, k chains h0/h1, then V), with q/k weight
            # columns split so q unblocks first. All on the scalar HWDGE
            # queue; x rides the sync HWDGE queue; later-group x and wout
            # ride SWDGE.
            for c0 in (0, 4):
                for w_sb, wsrc in ((wh_sb, wqh), (wl_sb, wql)):
                    nc.scalar.dma_start(
                        out=w_sb[:, c0:c0 + 4, 0:1024],
                        in_=wsrc[128 * c0:128 * (c0 + 4), 0:1024].rearrange(
                            "(c p) n -> p c n", p=128))
            for w_sb, wsrc in ((wh_sb, wqh), (wl_sb, wql)):
                for c0 in range(0, CH, 4):
                    nc.scalar.dma_start(
                        out=w_sb[:, c0:c0 + 4, 1024:1536],
                        in_=wsrc[128 * c0:128 * (c0 + 4), 1024:1536].rearrange(
                            "(c p) n -> p c n", p=128))

        def load_wout():
            # deferred into the phase-0 fill (behind the group-1 x loads on
            # SWDGE): first consumed by the deferred group-0 out-projections
            # in phase 1 (~45us in)
            with tc.tile_wait_until(0.016):
                for cp in range(2):
                    for w_sb, wsrc in ((wouth_sb, wouth), (woutl_sb, woutl)):
                        nc.gpsimd.dma_start(
                            out=w_sb[cp][:],
                            in_=wsrc[256 * cp:256 * (cp + 1), :].rearrange(
                                "(c p) n -> p c n", p=128))
        # kT per pair, head 2p on partitions [0:64), head 2p+1 on [64:128)
        kts = [wpool.tile([128, seq_len], BF16, tag=f"kt{p}", name=f"kt{p}")
               for p in range(PAIRS)]
        # V in bf16: per (head, k-tile) a [128, 64] stationary block
        v1 = wpool.tile([128, HPC, n_tt, HD], BF16, tag="v1")

        qts_of = {}  # g -> [qt tiles per pair]

        # psum scratch for the projection/transpose/oproj chains. In steady
        # state only the mm bank is free; in the prologue and final drain the
        # scores/AV banks are idle, so rotate through them too (the st slot is
        # [128,2,512]; its first bank is used as a [128,512] scratch).
        # "banked" mode (prologue only, F32) additionally uses the st slots'
        # SECOND psum banks (zero regions are 2KB, so the two banks of one
        # slot hold independent accumulation groups), giving 7 concurrent
        # scratch accumulators for the q+k chain overlap.
        _ps_state = {"wide": False, "banked": False, "i": 0, "st": {}}

        def set_wide_scratch(wide, banked=False):
            _ps_state["wide"] = wide
            _ps_state["banked"] = banked

        def scratch_ps(dtype):
            if not _ps_state["wide"]:
                return mm_ps.tile([128, 512], dtype, tag="mm", name="mm")
            if _ps_state["banked"] and dtype == F32:
                i = _ps_state["i"] = (_ps_state["i"] + 1) % 7
                if i == 0:
                    return mm_ps.tile([128, 512], dtype, tag="mm", name="mm")
                if i in (1, 2, 3, 4):
                    buf, bank = divmod(i - 1, 2)
                    if bank == 0:
                        _ps_state["st"][buf] = st_ps.tile(
                            [128, 2, 512], dtype, tag="st", name="stx")
                    return _ps_state["st"][buf][:, bank, :]
                return av_ps.tile([128, 512], dtype, tag="av", name="avx")
            i = _ps_state["i"] = (_ps_state["i"] + 1) % 5
            if i == 0:
                return mm_ps.tile([128, 512], dtype, tag="mm", name="mm")
            if i in (1, 2):
                return st_ps.tile([128, 512], dtype, tag="st", name="stx")
            return av_ps.tile([128, 512], dtype, tag="av", name="avx")

        def transpose_units(g):
            """x arrives pre-transposed and fp8-hi/lo-split from the host:
            just DMA the group's xT columns. Group 0 (prologue, latency-
            critical) goes on the sync HWDGE queue split in half-chunks;
            later groups ride the then-idle SWDGE ring as one transfer per
            hi/lo half."""
            xt = (xt_pool.tile([128, CH, TG], FP8, tag="xth", name=f"xth{g}"),
                  xt_pool.tile([128, CH, TG], FP8, tag="xtl", name=f"xtl{g}"))
            gsl = slice(g * TG, (g + 1) * TG)

            def u():
                if g == 0:
                    # lo directly after hi per chunk-half: every chain's
                    # third (lo*hi) term needs x-lo early
                    for c0 in (0, 4):
                        for t, src in ((xt[0], xh), (xt[1], xl)):
                            nc.sync.dma_start(
                                out=t[:, c0:c0 + 4, :],
                                in_=src[128 * c0:128 * (c0 + 4), gsl].rearrange(
                                    "(c p) n -> p c n", p=128))
                else:
                    # wait-ts biases the tile scheduler so these preps don't
                    # jump the serial DMA lane ahead of the prologue-critical
                    # weight transfers
                    with tc.tile_wait_until(0.010 + 0.038 * (g - 1)):
                        for t, src in ((xt[0], xh), (xt[1], xl)):
                            nc.gpsimd.dma_start(
                                out=t[:, :, :],
                                in_=src[:, gsl].rearrange(
                                    "(c p) n -> p c n", p=128))
            return xt, [u]

        def qkv_units(g, xt):
            """12 units: 4 qt chains, 4 kT chains, 4 V chains."""
            qts = qts_of.setdefault(g, [])
            units = []

            half = {}

            xth, xtl = xt

            def qk_chain(p, qk, h):
                # split into two half-chains (finer interleave granularity).
                # Each half contracts 4 D-chunks = 2 fp8 DoubleRow chunk-pairs
                # x 3 hi/lo cross terms (lo*lo dropped). Terms are ordered
                # hi*hi first so the prologue chains can start as soon as the
                # hi transfers land (lo halves trickle in later).
                if h == 0:
                    half[(p, qk)] = scratch_ps(F32)
                ps = half[(p, qk)]
                col = qk * 512 + p * 128
                first = True
                for wt, xt_ in ((wh_sb, xth), (wl_sb, xth), (wh_sb, xtl)):
                    for c0 in range(4 * h, 4 * h + 4, 2):
                        cs = slice(c0, c0 + 2)
                        nc.tensor.matmul(
                            ps[:, :TG], wt[:, cs, col:col + 128],
                            xt_[:, cs, :],
                            start=(h == 0 and first),
                            stop=(h == 1 and c0 == CH - 2 and xt_ is xtl
                                  and not with_bias),
                            perf_mode=DR)
                        first = False
                if h == 0:
                    return
                if with_bias:
                    nc.tensor.matmul(
                        ps[:, :TG], b_sb[0:1, col:col + 128],
                        ones_row[0:1, :], start=False, stop=True)
                if qk == 0:
                    qt = qt_pool.tile([128, TG], BF16, tag="qt")
                    nc.vector.tensor_copy(out=qt[:], in_=ps[:, :TG])
                    qts.append(qt)
                else:
                    nc.vector.tensor_copy(
                        out=kts[p][:, g * TG:(g + 1) * TG], in_=ps[:, :TG])

            def v_chain(t4):
                tt = g * (TG // 128) + t4
                ps = scratch_ps(F32)
                first = True
                tsl = slice(128 * t4, 128 * (t4 + 1))
                for xt_, wt in ((xth, wh_sb), (xth, wl_sb), (xtl, wh_sb)):
                    for c0 in range(0, CH, 2):
                        cs = slice(c0, c0 + 2)
                        nc.tensor.matmul(
                            ps[:, :512], xt_[:, cs, tsl],
                            wt[:, cs, 1024:1536],
                            start=first,
                            stop=(c0 == CH - 2 and xt_ is xtl
                                  and not with_bias),
                            perf_mode=DR)
                        first = False
                if with_bias:
                    nc.tensor.matmul(
                        ps[:, :512], ones_row[0:1, 0:128],
                        b_sb[0:1, 1024:1536], start=False, stop=True)
                nc.vector.tensor_copy(
                    out=v1[:, :, tt, :],
                    in_=ps[:, :512].rearrange("p (h d) -> p h d", h=HPC))

            k_units = []
            v_units = []
            # in the prologue the qk chains are gated on their W chunks
            # landing (~1us apart): run all h=0 half-chains (chunks 0-3)
            # before any h=1 (chunks 4-7). Requires 4 concurrent psum
            # accumulators - only legal in the wide-scratch prologue.
            hmajor = _ps_state["wide"]
            for lst, qk in ((units, 0), (k_units, 1)):
                if hmajor:
                    for h in range(2):
                        for p in range(PAIRS):
                            lst.append(lambda p=p, h=h, qk=qk: qk_chain(p, qk, h))
                else:
                    for p in range(PAIRS):
                        lst.append(lambda p=p, qk=qk: qk_chain(p, qk, 0))
                        lst.append(lambda p=p, qk=qk: qk_chain(p, qk, 1))
            for t4 in range(TG // 128):
                v_units.append(lambda t4=t4: v_chain(t4))
            return units, k_units, v_units

        def attention_units(g):
            """Per pair: one unit per k-tile (sT+exp+mask, AV carried by one),
            then a normalization unit; finally the out-projection units."""
            units = []
            pending_finish = []
            an_tiles = [an_pool.tile([128, 512], BF16, tag="an",
                                     name=f"an_g{g}_s{s}")
                        for s in range(TG // 128)]
            # zero this group's sums columns once (all 4 pairs' 8-col slices)
            nc.vector.memset(sums[:, (g % 2) * 32:(g % 2) * 32 + 32], 0.0)
            qts = qts_of[g]
            for p in range(PAIRS):
                nkt = 4 * (g + 1)
                soff = (g % 2) * 32 + p * 8
                state = {}

                def start_pair(p=p, state=state, soff=soff):
                    state["av"] = av_ps.tile([128, 512], F32, tag="av",
                                             name=f"av_g{g}_p{p}")
                    state["carry"] = []
                    state["first"] = True

                def kt_unit(kt, pos, p=p, state=state, nkt=nkt,
                            sp=start_pair):
                    if pos == 0:
                        sp()
                    # pop the carried AV FIRST: it is ready now, and the
                    # scores below may head-block on a busy st slot
                    if len(state["carry"]) >= 9:
                        state["emit_av"](*state["carry"].pop(0))
                    qt = qts[p]
                    rdiag = kt - 4 * g
                    col0 = 128 * rdiag if rdiag > 0 else 0
                    ksl = slice(128 * kt, 128 * (kt + 1))
                    st = st_ps.tile([128, 2, 512], F32, tag="st")
                    nc.tensor.matmul(
                        st[:, 0, col0:], kts[p][0:64, ksl], qt[0:64, col0:])
                    nc.tensor.matmul(
                        st[:, 1, col0:], kts[p][64:128, ksl], qt[64:128, col0:])
                    pt = pt_pool.tile([128, 2, 512], BF16, tag="pt")
                    nc.scalar.activation(
                        pt[:, :, col0:], st[:, :, col0:], AF.Exp,
                        scale=SCALE / (WSCALE * WSCALE))
                    if rdiag >= 0:
                        # keep P^T[k, q] only where q >= k (within-block);
                        # one select covers both heads via a stride-0 dim
                        nc.gpsimd.affine_select(
                            out=pt[:, :, col0:col0 + 128],
                            in_=pt[:, :, col0:col0 + 128],
                            compare_op=mybir.AluOpType.is_ge,
                            fill=0.0, base=0, pattern=[[0, 2], [1, 128]],
                            channel_multiplier=-1)
                    state["carry"].append((kt, pos, rdiag, pt))

                def emit_av(kt, pos, rdiag, pt, p=p, state=state, nkt=nkt,
                            soff=soff):
                    av = state["av"]
                    s0 = max(rdiag, 0)
                    last = pos == nkt - 1
                    for s in range(s0, TG // 128):
                        for h in (0, 1):
                            nc.tensor.matmul(
                                av[:, 128 * s + 64 * h:128 * s + 64 * h + 64],
                                pt[:, h, 128 * s:128 * (s + 1)],
                                v1[:, 2 * p + h, kt, :],
                                start=state["first"],
                                stop=(last and s == 3 and h == 1))
                            state["first"] = False
                            nc.tensor.matmul(
                                sums[:, soff + 4 * h + s:soff + 4 * h + s + 1],
                                pt[:, h, 128 * s:128 * (s + 1)],
                                ones_bf[:],
                                start=False, stop=False,
                                skip_group_check=True)

                state["emit_av"] = emit_av

                def emit_block(mm, state, p, soff, stop_last):
                    av = state["av"]
                    for i, (isd, kt, pt, s, h) in enumerate(mm):
                        nc.tensor.matmul(
                            av[:, 128 * s + 64 * h:128 * s + 64 * h + 64],
                            pt[:, h, 128 * s:128 * (s + 1)],
                            v1[:, 2 * p + h, kt, :],
                            start=state["first"],
                            stop=(stop_last and i == len(mm) - 1))
                        state["first"] = False
                        nc.tensor.matmul(
                            sums[:, soff + 4 * h + s:soff + 4 * h + s + 1],
                            pt[:, h, 128 * s:128 * (s + 1)],
                            ones_bf[:],
                            start=False, stop=False,
                            skip_group_check=True)

                def flush_unit(p=p, state=state, soff=soff):
                    # flush the carried non-diagonal AVs at pair end; the
                    # affine-masked diagonal blocks wait for Pool latency, so
                    # they are deferred into finish_unit (emitted two k-tile
                    # units into the NEXT pair) to avoid head-blocking the
                    # PE's 4-deep dependency wait queue
                    mm = []
                    for kt, pos, rdiag, pt in state["carry"]:
                        for s in range(max(rdiag, 0), TG // 128):
                            for h in (0, 1):
                                mm.append((s == rdiag, kt, pt, s, h))
                    state["carry"] = []
                    mm.sort(key=lambda t: t[0])
                    ndiag = sum(1 for t in mm if t[0])
                    split = len(mm) - ndiag
                    emit_block(mm[:split], state, p, soff, stop_last=False)
                    state["diag"] = mm[split:]

                def norm_unit(p=p, state=state, soff=soff):
                    emit_block(state.pop("diag"), state, p, soff,
                               stop_last=True)
                    av = state["av"]
                    rc = rc_pool.tile([128, 2, 4], F32, tag="rc")
                    nc.vector.reciprocal(
                        rc[:], sums[:, soff:soff + 8].rearrange(
                            "p (h s) -> p h s", h=2))
                    for s in range(TG // 128):
                        nc.vector.tensor_mul(
                            an_tiles[s][:, 128 * p:128 * (p + 1)].rearrange(
                                "p (h d) -> p h d", h=2),
                            av[:, 128 * s:128 * (s + 1)].rearrange(
                                "p (h d) -> p h d", h=2),
                            rc[:, :, s:s + 1].broadcast_to([128, 2, 64]))

                pair_units = []
                kt_order = list(range(nkt))
                if g == n_tg - 1 and p == PAIRS - 1:
                    # last pair of the last group: diagonal k-tiles first so
                    # the wind-down tail has no Pool(affine) round-trip on
                    # its critical path
                    kt_order = kt_order[4 * g:] + kt_order[:4 * g]
                for pos, kt in enumerate(kt_order):
                    pair_units.append(
                        lambda kt=kt, pos=pos, f=kt_unit: f(kt, pos))
                if pending_finish:
                    pair_units.insert(min(6, max(2, len(pair_units) - 2)),
                                      pending_finish.pop())
                units += pair_units
                units.append(flush_unit)
                pending_finish.append(norm_unit)

            if pending_finish:
                units.append(pending_finish.pop())

            ats = {}

            def trans_unit(s):
                ps = scratch_ps(BF16)
                for c in range(PAIRS):
                    nc.tensor.transpose(
                        ps[:, 128 * c:128 * (c + 1)],
                        an_tiles[s][:, 128 * c:128 * (c + 1)],
                        ident_bf[:])
                # fp8 hi/lo split of attnT for the DoubleRow out-projection.
                # The hi copy runs on the (mostly idle) Pool engine so the
                # DVE isn't the serial bottleneck of the projection drain.
                ath = at_pool.tile([128, 512], FP8, tag="ath")
                atl = at_pool.tile([128, 512], FP8, tag="atl")
                if _ps_state["wide"]:
                    # final drain: ACT is idle and DVE is the serial
                    # bottleneck, so split the quant pair across both
                    nc.scalar.copy(ath[:], ps[:])
                else:
                    nc.vector.tensor_copy(out=ath[:], in_=ps[:])
                nc.vector.tensor_sub(atl[:], ps[:], ath[:])
                ats[s] = (ath, atl)

            def oproj_unit(s, nh):
                row0 = g * TG + 128 * s
                ath, atl = ats[s]
                ps = scratch_ps(F32)
                nsl = slice(512 * nh, 512 * (nh + 1))
                first = True
                for at_, wo in ((ath, wouth_sb), (ath, woutl_sb),
                                (atl, wouth_sb)):
                    for cp in range(2):
                        nc.tensor.matmul(
                            ps[:, :512],
                            at_[:, 256 * cp:256 * (cp + 1)].rearrange(
                                "p (c m) -> p c m", c=2),
                            wo[cp][:, :, nsl],
                            start=first,
                            stop=(at_ is atl and cp == 1),
                            perf_mode=DR)
                        first = False
                ob = ob_pool.tile([128, 512], BF16, tag="ob")
                if _ps_state["wide"] and (s + nh) % 2 == 0:
                    nc.scalar.copy(ob[:], ps[:, :512])
                else:
                    nc.vector.tensor_copy(out=ob[:], in_=ps[:, :512])
                # in the final drain the sync HWDGE queue serializes the last
                # 8 output stores (~625ns descriptor-gen each); spread them
                # over the idle queues so the tail isn't gated on it
                eng = (nc.sync if not _ps_state["wide"] else
                       (nc.sync, nc.scalar, nc.gpsimd)[(3 - s + nh) % 3])
                eng.dma_start(
                    out=out[row0:row0 + 128, 512 * nh:512 * (nh + 1)],
                    in_=ob[:])

            ounits = []
            for s in range(TG // 128):
                ounits.append(lambda s=s: trans_unit(s))
            for s in range(TG // 128):
                for nh in range(2):
                    ounits.append(lambda s=s, nh=nh: oproj_unit(s, nh))
            return units, ounits

        def interleave(a_units, b_units, pre_b=0):
            # pre_b: emit that many b-units before any a-unit (phase 0 needs
            # the group-0 V chains emitted before the first AV flush so the
            # tile framework sees the writes first)
            for u in b_units[:pre_b]:
                u()
            b_units = b_units[pre_b:]
            na, nb = len(a_units), len(b_units)
            ia = ib = 0
            while ia < na or ib < nb:
                fa = (na - ia) / na if na else 0.0
                fb = (nb - ib) / nb if nb else 0.0
                if ia < na and (fa > fb or ib >= nb):
                    a_units[ia]()
                    ia += 1
                else:
                    b_units[ib]()
                    ib += 1

        # prologue: group 0 projection (weight DMAs after group 0's x loads).
        # The scores/AV banks are idle here, so scratch rotates through them.
        set_wide_scratch(True, banked=True)
        # fill the initial x/w DMA latency with dummy PE work (also completes
        # the tensor engine's p-state ramp before real work lands); plain
        # matmuls on a memset tile need no identity, so they start ~1us in
        dummy = const.tile([128, 256], BF16, tag="dummy")
        nc.vector.memset(dummy[:], 0.0)
        xt0, tunits0 = transpose_units(0)
        for u in tunits0:
            u()
        load_weights()
        make_consts()
        # SWDGE gate: the gpsimd descriptor ring would otherwise eagerly
        # prepare the later-group x / wout transfers at t~1us and their
        # transfers would jump the (serial) DMA lane ahead of the
        # prologue-critical weight transfers. A tiny Pool-engine copy that
        # waits on the first x tile holds the ring back until the prologue
        # transfers are in flight.
        gate = const.tile([1, 1], FP8, tag="gate")
        nc.gpsimd.tensor_copy(out=gate[:], in_=xt0[1][0:1, CH - 1, 0:1])
        for _ in range(NDUMMY):
            wps = scratch_ps(F32)
            nc.tensor.matmul(wps[:, :256], dummy[:, 0:128], dummy[:],
                             start=True, stop=True)
        # prologue runs only the q and k chains (their weights land first);
        # group 0's V chains are deferred into the phase-0 fill, where they
        # interleave with the (V-independent) score units while the V-column
        # weights land. Unit order follows the DMA landing order: all h=0
        # halves (chunks 0-3) before h=1 (chunks 4-7). This holds up to 7
        # open psum accumulations (4 q + 3 k), which the banked scratch
        # rotation provides; k p3 waits until the q psums close.
        _ps_state["i"] = 0
        q0, k0, v0 = qkv_units(0, xt0)
        for u in [q0[0], q0[1], q0[2], q0[3],      # q h0 p0-3
                  k0[0], k0[1], k0[2],             # k h0 p0-2
                  q0[4], q0[5], q0[6], q0[7],      # q h1 p0-3 (close q psums)
                  k0[3],                           # k h0 p3
                  k0[4], k0[5], k0[6], k0[7]]:     # k h1 p0-3
            u()
        set_wide_scratch(False)
        # steady state: attention(g) interleaved with transposes(g+1) +
        # projection(g+1); out-projections are deferred up to two groups so
        # the last (largest) attention group still has dense PE fill
        pending_oproj = []  # deferred out-projection unit lists, oldest first
        fill_carry = v0     # group g's V chains lead the phase-g fill: their
        # only consumer is the AV flush (safely late in emission), and the
        # ACT-bound later phases need the q/k fill to shrink, not grow
        for g in range(n_tg):
            attn, ounits = attention_units(g)
            fill = fill_carry
            fill_carry = []
            if g + 1 < n_tg:
                xt1, tunits = transpose_units(g + 1)
                qu, ku, vu = qkv_units(g + 1, xt1)
                fill += tunits + ([load_wout] if g == 0 else []) + qu + ku
                fill_carry = vu
            if False and pending_oproj:
                # phase 2 is ACT(exp)-bound with PE slack: give it the
                # oldest deferred out-projection group
                fill += pending_oproj.pop(0)
            if g + 1 == n_tg:
                if g == 0:
                    fill.append(load_wout)
                # last group is exp(ACT)-bound and has no next-group
                # projection: feed it the remaining deferred out-projections
                while pending_oproj:
                    fill += pending_oproj.pop(0)
            interleave(attn, fill, pre_b=len(v0) if g == 0 else 0)
            pending_oproj.append(ounits)
        # final drain: attention is done, scores/AV banks are idle again
        set_wide_scratch(True)
        for ou in pending_oproj:
            for u in ou:
                u()


def build_program(with_bias, seq_len=T):
    nc = bacc.Bacc("TRN2", target_bir_lowering=False, debug=False,
                   enable_asserts=False, num_devices=8)
    xh = nc.dram_tensor("xh", [D, seq_len], FP8, kind="ExternalInput").ap()
    xl = nc.dram_tensor("xl", [D, seq_len], FP8, kind="ExternalInput").ap()
    wqh = nc.dram_tensor("wqh", [D, 3 * HPC * HD], FP8,
                         kind="ExternalInput").ap()
    wql = nc.dram_tensor("wql", [D, 3 * HPC * HD], FP8,
                         kind="ExternalInput").ap()
    wouth = nc.dram_tensor("wouth", [HPC * HD, D], FP8,
                           kind="ExternalInput").ap()
    woutl = nc.dram_tensor("woutl", [HPC * HD, D], FP8,
                           kind="ExternalInput").ap()
    out = nc.dram_tensor("out", [seq_len, D], BF16,
                         kind="ExternalOutput").ap()
    bqkv = None
    if with_bias:
        bqkv = nc.dram_tensor("bqkv", [1, 3 * HPC * HD], F32,
                              kind="ExternalInput").ap()
    with tile.TileContext(nc) as tc:
        build_tile_program(tc, xh, xl, wqh, wql, wouth, woutl, out, bqkv,
                           seq_len=seq_len)
    nc.compile()
    return nc


_PROGRAM_CACHE = {}


def _get_program(with_bias):
    if with_bias not in _PROGRAM_CACHE:
        _PROGRAM_CACHE[with_bias] = build_program(with_bias)
    return _PROGRAM_CACHE[with_bias]


def _split8(a):
    """fp8(e4m3) hi/lo split: a ~= hi + lo to ~7 mantissa bits."""
    import ml_dtypes
    hi = a.astype(ml_dtypes.float8_e4m3)
    lo = (a - hi.astype(np.float32)).astype(ml_dtypes.float8_e4m3)
    return hi, lo


def make_core_inputs(x_core, wqkv_core, wout_core, b_core=None):
    """Quantized inputs for ONE core: x_core [T, D], wqkv_core [D, 1536]
    (q|k|v grouped), wout_core [512, D]."""
    xh, xl = _split8(np.ascontiguousarray(x_core.T))
    wqh, wql = _split8(np.ascontiguousarray(wqkv_core) * WSCALE)
    wouth, woutl = _split8(np.ascontiguousarray(wout_core) * WSCALE)
    m = {
        "xh": xh, "xl": xl, "wqh": wqh, "wql": wql,
        "wouth": wouth, "woutl": woutl,
    }
    if b_core is not None:
        m["bqkv"] = np.ascontiguousarray(b_core * WSCALE).reshape(
            1, -1).astype(np.float32)
    return m


def make_in_maps(x, w_qkv, b_qkv, w_out, with_bias):
    """Per-core input dicts: core c -> batch c//2, head group c%2."""
    in_maps = []
    for core in range(8):
        b, gr = divmod(core, 2)
        qc = slice(512 * gr, 512 * (gr + 1))
        kc = slice(D + 512 * gr, D + 512 * (gr + 1))
        vc = slice(2 * D + 512 * gr, 2 * D + 512 * (gr + 1))
        wq = np.concatenate([w_qkv[:, qc], w_qkv[:, kc], w_qkv[:, vc]], axis=1)
        bq = (np.concatenate([b_qkv[qc], b_qkv[kc], b_qkv[vc]])
              if with_bias else None)
        in_maps.append(make_core_inputs(
            x[b], wq, w_out[512 * gr:512 * (gr + 1), :], bq))
    return in_maps


def kernel(x, w_qkv, b_qkv, w_out, b_out):
    x = np.asarray(x, dtype=np.float32)
    w_qkv = np.asarray(w_qkv, dtype=np.float32)
    b_qkv = np.asarray(b_qkv, dtype=np.float32)
    w_out = np.asarray(w_out, dtype=np.float32)
    b_out = np.asarray(b_out, dtype=np.float32)

    with_bias = bool(np.any(b_qkv))
    nc = _get_program(with_bias)
    in_maps = make_in_maps(x, w_qkv, b_qkv, w_out, with_bias)
    res = run_bass_kernel_spmd(nc, in_maps, core_ids=list(range(8))).results

    out = np.empty((B, T, D), dtype=np.float32)
    for b in range(B):
        out[b] = ((res[2 * b]["out"].astype(np.float32)
                   + res[2 * b + 1]["out"].astype(np.float32)) * OUT_SCALE
                  + b_out[None, :])
    return out



# revision 78
# speedup vs baseline: 1.0180x; 1.0004x over previous
"""Causal self-attention (B=4, T=2048, D=1024, H=16, hd=64) on 8 trn2 cores.

Sharding: 4-way data parallel over batch x 2-way tensor parallel over heads.
Core c handles batch c//2 and heads [8*(c%2), 8*(c%2)+8). Each core computes
its heads' partial contribution to the out-projection (a full [T, D] tensor);
the host sums the two head-group partials per batch and adds b_out.

Host-side prep: x is transposed to xT [D, T] and split into fp8(e4m3)
hi + lo residual planes; w_qkv (x32) and w_out (x32) likewise. The QKV and
out-projections then run as fp8 DoubleRow matmuls (2 K-chunks per pass, 0.5
cycles/row = 4x bf16 MAC rate) with three hi/lo cross terms (hi*hi + hi*lo +
lo*hi, the lo*lo term dropped), which is 0.75x the bf16 cost at ~bf16
accuracy. The weight x32 scaling keeps the lo planes out of fp8's subnormal
range; the softmax scale absorbs 1/1024 and the host divides the output
partials by 1024. End-to-end rel err vs the fp32 reference is ~4.4e-3
(gate: 2e-2).

Per-core dataflow (single NeuronCore, Tile framework):
  1. xT hi/lo stream straight from DRAM into SBUF (no on-device work).
  2. qT/kT [hd,T] are computed head-PAIR-stacked ([128,T] per pair) with
     w_qkv chunks as stationary DoubleRow fp8 matmuls; V [T,hd] per head is
     computed naturally and scattered into v1 (64 cols per (head, k-tile),
     bf16).
  3. Scores are computed TRANSPOSED: S^T[k,q] = K @ Q^T via two K=64 matmuls
     per (pair, k-tile) into one [128, 2, 512] two-bank PSUM tile; ONE wide
     exp (ACT, scale folded in) produces P^T for both heads in bf16.
     Causality: k-tiles above the diagonal are skipped, diagonal blocks get
     an affine_select zero-mask after exp, partial k-tiles use narrower
     matmuls.
  4. AV runs NON-transposed: per (head, q-subtile 128) the P^T block is the
     STATIONARY operand and V [128,64] streams, so each matmul moves 64
     rows instead of 512. All 8 (head, subtile) accumulators of a pair pack
     into ONE psum bank (zero-on-first-touch after a single start=True).
     Softmax sums ride as 1-column matmuls (P^T stationary, ones moving)
     into a shared, DVE-zeroed sums bank; AV emission trails its exp by 8
     k-tile units (carry) so the PE never waits on ACT latency.
  5. Normalization is a DVE broadcast multiply (reciprocal of sums, then
     attn * recip -> bf16). The normalized attn [q, 512] is PE-transposed
     (bf16), then quantized to an fp8 hi/lo pair (DVE copy + subtract; the
     hi copy moves to ACT in the drain where DVE is the serial bottleneck)
     as the out-projection's stationary operand; out = attnT.T @ w_out
     accumulates as 6 DoubleRow matmuls over 2 feature-chunk-pairs x 3
     hi/lo terms. All out-projections are deferred into the last q-group's
     phase + final drain, which are otherwise exp(ACT)-bound.

Scheduling: emission-level software pipelining interleaves attention for
q-group g with the transposes + q/k projections of group g+1; each group's
V chains ride at the FRONT of its own phase (their only consumer, the AV
flush, is emitted safely later). PSUM (8 banks) is budgeted as 4 scores +
2 AV + 1 sums + 1 projection scratch; the prologue additionally uses the
score slots' second banks for 7 concurrent q/k accumulators, and the final
drain widens the scratch rotation to 5. The cost model serializes all DMA
transfers through one lane fed by two descriptor-gen queues (HWDGE ~625ns,
SWDGE ~1.2us per dma_start), so the prologue issues transfers in exact
consumption order and gates the eager SWDGE ring (later-group x, w_out)
behind a Pool-engine copy of the last group-0 x tile; dummy matmuls at t=0
cover the first DMA latency and the PE p-state ramp (full clock needs 3us
of continuous execution - any >100ns gap drops it to half rate).
"""

import numpy as np

import concourse.bass as bass
import concourse.mybir as mybir
import concourse.tile as tile
from concourse import bacc
from concourse.bass_utils import run_bass_kernel_spmd
from concourse.masks import make_identity

B, T, D = 4, 2048, 1024
H, HD = 16, 64
HPC = 8              # heads per core
PAIRS = HPC // 2
CH = D // 128        # K-chunks for the QKV projection
TG = 512             # T-group / q-group width
SCALE = 1.0 / 8.0    # 1/sqrt(HD)
NDUMMY = 20          # prologue PE-warm dummy matmuls

F32 = mybir.dt.float32
F32R = mybir.dt.float32r
BF16 = mybir.dt.bfloat16
FP8 = mybir.dt.float8e4
DR = mybir.MatmulPerfMode.DoubleRow
AF = mybir.ActivationFunctionType

# w_qkv and w_out are scaled by 2**5 on the host so their fp8(e4m3) hi/lo
# splits stay out of the subnormal range; q', k', v' come out 32x larger, so
# the softmax scale absorbs 1/(32*32) and the device output is 32*32x the
# true projection (divided back out on the host).
WSCALE = 32.0
OUT_SCALE = 1.0 / (WSCALE * WSCALE)


def _r(ap):
    return ap.bitcast(F32R)


def build_tile_program(tc, xh, xl, wqh, wql, wouth, woutl, out, bqkv=None,
                       seq_len=T):
    """Emit the per-core program. seq_len is parametrized for small-scale
    simulation tests; the real kernel uses seq_len=T=2048.

    Emission is software-pipelined: attention for q-group g (latency-bound
    serial chains sT -> exp -> mask -> AV) is interleaved at emission level
    with the transposes + QKV projection of group g+1 (dense PE work), so the
    in-order PE fills attention's dependency stalls with projection matmuls.
    """
    nc = tc.nc
    n_tg = seq_len // TG
    n_tt = seq_len // 128
    with_bias = bqkv is not None

    from contextlib import ExitStack
    with ExitStack() as ctx:
        const = ctx.enter_context(tc.tile_pool(name="const", bufs=1))
        wpool = ctx.enter_context(tc.tile_pool(name="wpool", bufs=1))
        xt_pool = ctx.enter_context(tc.tile_pool(name="xt", bufs=4))
        qt_pool = ctx.enter_context(tc.tile_pool(name="qt", bufs=2 * PAIRS))
        pt_pool = ctx.enter_context(tc.tile_pool(name="pt", bufs=14))
        an_pool = ctx.enter_context(tc.tile_pool(name="an", bufs=16))
        at_pool = ctx.enter_context(tc.tile_pool(name="at", bufs=16))
        ob_pool = ctx.enter_context(tc.tile_pool(name="ob", bufs=6))
        rc_pool = ctx.enter_context(tc.tile_pool(name="rc", bufs=2))
        mm_ps = ctx.enter_context(tc.tile_pool(name="mmps", bufs=1, space="PSUM"))
        st_ps = ctx.enter_context(tc.tile_pool(name="stps", bufs=2, space="PSUM"))
        av_ps = ctx.enter_context(tc.tile_pool(name="avps", bufs=2, space="PSUM"))
        sums_ps = ctx.enter_context(tc.tile_pool(name="sums", bufs=1, space="PSUM"))
        identity = const.tile([128, 128], F32, tag="ident")
        ident_bf = const.tile([128, 128], BF16, tag="identbf")
        ones_bf = const.tile([128, 1], BF16, tag="ones_bf")

        def make_consts():
            # deferred so the weight SWDGE generation isn't queued behind
            # make_identity on the Pool engine at t=0; the identity is only
            # needed by the (late) attnT transposes
            make_identity(nc, identity[:])
            nc.vector.tensor_copy(out=ident_bf[:], in_=identity[:])
            nc.vector.memset(ones_bf[:], 1.0)
        if with_bias:
            ones_f = const.tile([128, 64], F32, tag="ones_f")
            nc.vector.memset(ones_f[:], 1.0)
            b_sb = const.tile([1, 3 * HPC * HD], F32R, tag="bias")
            nc.sync.dma_start(out=b_sb[:], in_=bqkv.bitcast(F32R)[:])
            ones_row = const.tile([1, TG], F32R, tag="ones_row")
            nc.vector.tensor_copy(
                out=ones_row[:], in_=ones_f[0:1, 0:1].broadcast_to([1, TG]))

        # softmax sums: one persistent bank; cols = parity*32 + pair*8 + h*4+s
        sums = sums_ps.tile([128, 64], F32, tag="sums")

        # resident weights (fp8 hi/lo pairs); w_out pair-packed: pair p rows
        # [128p, 128p+128). The DMAs are deferred until after group 0's x
        # loads so the first transposes aren't queued behind the weight
        # traffic.
        wh_sb = wpool.tile([128, CH, 3 * HPC * HD], FP8, tag="wqh")
        wl_sb = wpool.tile([128, CH, 3 * HPC * HD], FP8, tag="wql")
        # wout fp8 hi/lo, chunk-PAIR packed for DoubleRow: tile cp holds rows
        # [256cp, 256cp+256) as [128, 2, D]
        wouth_sb = [wpool.tile([128, 2, D], FP8, tag=f"wouth{p}",
                               name=f"wouth{p}") for p in range(2)]
        woutl_sb = [wpool.tile([128, 2, D], FP8, tag=f"woutl{p}",
                               name=f"woutl{p}") for p in range(2)]

        def load_weights():
            # The cost model serializes ALL transfers through one DMA_ENGINES
            # lane (~bytes/360GB/s each), fed by two descriptor-gen lanes:
            # HWDGE (shared SP+ACT, 625ns/dma) and SWDGE (gpsimd, ~1.1us/dma).
            # The prologue is transfer-order-bound and the PE is in-order, so
            # transfers are issued in exact consumption order of the prologue
            # units (q chains h0/h1, k chains h0/h1, then V), with q/k weight
            # columns split so q unblocks first. All on the scalar HWDGE
            # queue; x rides the sync HWDGE queue; later-group x and wout
            # ride SWDGE.
            # q|k lo chunks 0-3 ride the otherwise-idle SWDGE lane (ahead
            # of the gate) so the first chains' hi*lo terms land earlier and
            # the HWDGE stream shortens
            nc.gpsimd.dma_start(
                out=wl_sb[:, 0:4, 0:1024],
                in_=wql[0:512, 0:1024].rearrange("(c p) n -> p c n", p=128))
            for c0 in (0, 4):
                for w_sb, wsrc in (((wh_sb, wqh),) if c0 == 0 else
                                   ((wh_sb, wqh), (wl_sb, wql))):
                    nc.scalar.dma_start(
                        out=w_sb[:, c0:c0 + 4, 0:1024],
                        in_=wsrc[128 * c0:128 * (c0 + 4), 0:1024].rearrange(
                            "(c p) n -> p c n", p=128))
            for w_sb, wsrc in ((wh_sb, wqh), (wl_sb, wql)):
                for c0 in range(0, CH, 4):
                    nc.scalar.dma_start(
                        out=w_sb[:, c0:c0 + 4, 1024:1536],
                        in_=wsrc[128 * c0:128 * (c0 + 4), 1024:1536].rearrange(
                            "(c p) n -> p c n", p=128))

        def load_wout():
            # deferred into the phase-0 fill (behind the group-1 x loads on
            # SWDGE): first consumed by the deferred group-0 out-projections
            # in phase 1 (~45us in)
            with tc.tile_wait_until(0.016):
                for cp in range(2):
                    for w_sb, wsrc in ((wouth_sb, wouth), (woutl_sb, woutl)):
                        nc.gpsimd.dma_start(
                            out=w_sb[cp][:],
                            in_=wsrc[256 * cp:256 * (cp + 1), :].rearrange(
                                "(c p) n -> p c n", p=128))
        # kT per pair, head 2p on partitions [0:64), head 2p+1 on [64:128)
        kts = [wpool.tile([128, seq_len], BF16, tag=f"kt{p}", name=f"kt{p}")
               for p in range(PAIRS)]
        # V in bf16: per (head, k-tile) a [128, 64] stationary block
        v1 = wpool.tile([128, HPC, n_tt, HD], BF16, tag="v1")

        qts_of = {}  # g -> [qt tiles per pair]

        # psum scratch for the projection/transpose/oproj chains. In steady
        # state only the mm bank is free; in the prologue and final drain the
        # scores/AV banks are idle, so rotate through them too (the st slot is
        # [128,2,512]; its first bank is used as a [128,512] scratch).
        # "banked" mode (prologue only, F32) additionally uses the st slots'
        # SECOND psum banks (zero regions are 2KB, so the two banks of one
        # slot hold independent accumulation groups), giving 7 concurrent
        # scratch accumulators for the q+k chain overlap.
        _ps_state = {"wide": False, "banked": False, "i": 0, "st": {}}

        def set_wide_scratch(wide, banked=False):
            _ps_state["wide"] = wide
            _ps_state["banked"] = banked

        def scratch_ps(dtype):
            if not _ps_state["wide"]:
                return mm_ps.tile([128, 512], dtype, tag="mm", name="mm")
            if _ps_state["banked"] and dtype == F32:
                i = _ps_state["i"] = (_ps_state["i"] + 1) % 7
                if i == 0:
                    return mm_ps.tile([128, 512], dtype, tag="mm", name="mm")
                if i in (1, 2, 3, 4):
                    buf, bank = divmod(i - 1, 2)
                    if bank == 0:
                        _ps_state["st"][buf] = st_ps.tile(
                            [128, 2, 512], dtype, tag="st", name="stx")
                    return _ps_state["st"][buf][:, bank, :]
                return av_ps.tile([128, 512], dtype, tag="av", name="avx")
            i = _ps_state["i"] = (_ps_state["i"] + 1) % 5
            if i == 0:
                return mm_ps.tile([128, 512], dtype, tag="mm", name="mm")
            if i in (1, 2):
                return st_ps.tile([128, 512], dtype, tag="st", name="stx")
            return av_ps.tile([128, 512], dtype, tag="av", name="avx")

        def transpose_units(g):
            """x arrives pre-transposed and fp8-hi/lo-split from the host:
            just DMA the group's xT columns. Group 0 (prologue, latency-
            critical) goes on the sync HWDGE queue split in half-chunks;
            later groups ride the then-idle SWDGE ring as one transfer per
            hi/lo half."""
            xt = (xt_pool.tile([128, CH, TG], FP8, tag="xth", name=f"xth{g}"),
                  xt_pool.tile([128, CH, TG], FP8, tag="xtl", name=f"xtl{g}"))
            gsl = slice(g * TG, (g + 1) * TG)

            def u():
                if g == 0:
                    # lo directly after hi per chunk-half: every chain's
                    # third (lo*hi) term needs x-lo early
                    for c0 in (0, 4):
                        for t, src in ((xt[0], xh), (xt[1], xl)):
                            nc.sync.dma_start(
                                out=t[:, c0:c0 + 4, :],
                                in_=src[128 * c0:128 * (c0 + 4), gsl].rearrange(
                                    "(c p) n -> p c n", p=128))
                else:
                    # wait-ts biases the tile scheduler so these preps don't
                    # jump the serial DMA lane ahead of the prologue-critical
                    # weight transfers
                    with tc.tile_wait_until(0.010 + 0.038 * (g - 1)):
                        for t, src in ((xt[0], xh), (xt[1], xl)):
                            nc.gpsimd.dma_start(
                                out=t[:, :, :],
                                in_=src[:, gsl].rearrange(
                                    "(c p) n -> p c n", p=128))
            return xt, [u]

        def qkv_units(g, xt):
            """12 units: 4 qt chains, 4 kT chains, 4 V chains."""
            qts = qts_of.setdefault(g, [])
            units = []

            half = {}

            xth, xtl = xt

            def qk_chain(p, qk, h):
                # split into two half-chains (finer interleave granularity).
                # Each half contracts 4 D-chunks = 2 fp8 DoubleRow chunk-pairs
                # x 3 hi/lo cross terms (lo*lo dropped). Terms are ordered
                # hi*hi first so the prologue chains can start as soon as the
                # hi transfers land (lo halves trickle in later).
                if h == 0:
                    half[(p, qk)] = scratch_ps(F32)
                ps = half[(p, qk)]
                col = qk * 512 + p * 128
                first = True
                for wt, xt_ in ((wh_sb, xth), (wl_sb, xth), (wh_sb, xtl)):
                    for c0 in range(4 * h, 4 * h + 4, 2):
                        cs = slice(c0, c0 + 2)
                        nc.tensor.matmul(
                            ps[:, :TG], wt[:, cs, col:col + 128],
                            xt_[:, cs, :],
                            start=(h == 0 and first),
                            stop=(h == 1 and c0 == CH - 2 and xt_ is xtl
                                  and not with_bias),
                            perf_mode=DR)
                        first = False
                if h == 0:
                    return
                if with_bias:
                    nc.tensor.matmul(
                        ps[:, :TG], b_sb[0:1, col:col + 128],
                        ones_row[0:1, :], start=False, stop=True)
                if qk == 0:
                    qt = qt_pool.tile([128, TG], BF16, tag="qt")
                    nc.vector.tensor_copy(out=qt[:], in_=ps[:, :TG])
                    qts.append(qt)
                else:
                    nc.vector.tensor_copy(
                        out=kts[p][:, g * TG:(g + 1) * TG], in_=ps[:, :TG])

            def v_chain(t4):
                tt = g * (TG // 128) + t4
                ps = scratch_ps(F32)
                first = True
                tsl = slice(128 * t4, 128 * (t4 + 1))
                for xt_, wt in ((xth, wh_sb), (xth, wl_sb), (xtl, wh_sb)):
                    for c0 in range(0, CH, 2):
                        cs = slice(c0, c0 + 2)
                        nc.tensor.matmul(
                            ps[:, :512], xt_[:, cs, tsl],
                            wt[:, cs, 1024:1536],
                            start=first,
                            stop=(c0 == CH - 2 and xt_ is xtl
                                  and not with_bias),
                            perf_mode=DR)
                        first = False
                if with_bias:
                    nc.tensor.matmul(
                        ps[:, :512], ones_row[0:1, 0:128],
                        b_sb[0:1, 1024:1536], start=False, stop=True)
                nc.vector.tensor_copy(
                    out=v1[:, :, tt, :],
                    in_=ps[:, :512].rearrange("p (h d) -> p h d", h=HPC))

            k_units = []
            v_units = []
            # in the prologue the qk chains are gated on their W chunks
            # landing (~1us apart): run all h=0 half-chains (chunks 0-3)
            # before any h=1 (chunks 4-7). Requires 4 concurrent psum
            # accumulators - only legal in the wide-scratch prologue.
            hmajor = _ps_state["wide"]
            for lst, qk in ((units, 0), (k_units, 1)):
                if hmajor:
                    for h in range(2):
                        for p in range(PAIRS):
                            lst.append(lambda p=p, h=h, qk=qk: qk_chain(p, qk, h))
                else:
                    for p in range(PAIRS):
                        lst.append(lambda p=p, qk=qk: qk_chain(p, qk, 0))
                        lst.append(lambda p=p, qk=qk: qk_chain(p, qk, 1))
            for t4 in range(TG // 128):
                v_units.append(lambda t4=t4: v_chain(t4))
            return units, k_units, v_units

        def attention_units(g):
            """Per pair: one unit per k-tile (sT+exp+mask, AV carried by one),
            then a normalization unit; finally the out-projection units."""
            units = []
            pending_finish = []
            an_tiles = [an_pool.tile([128, 512], BF16, tag="an",
                                     name=f"an_g{g}_s{s}")
                        for s in range(TG // 128)]
            # zero this group's sums columns once (all 4 pairs' 8-col slices)
            nc.vector.memset(sums[:, (g % 2) * 32:(g % 2) * 32 + 32], 0.0)
            qts = qts_of[g]
            for p in range(PAIRS):
                nkt = 4 * (g + 1)
                soff = (g % 2) * 32 + p * 8
                state = {}

                def start_pair(p=p, state=state, soff=soff):
                    state["av"] = av_ps.tile([128, 512], F32, tag="av",
                                             name=f"av_g{g}_p{p}")
                    state["carry"] = []
                    state["first"] = True

                def kt_unit(kt, pos, p=p, state=state, nkt=nkt,
                            sp=start_pair):
                    if pos == 0:
                        sp()
                    # pop the carried AV FIRST: it is ready now, and the
                    # scores below may head-block on a busy st slot
                    if len(state["carry"]) >= 9:
                        state["emit_av"](*state["carry"].pop(0))
                    qt = qts[p]
                    rdiag = kt - 4 * g
                    col0 = 128 * rdiag if rdiag > 0 else 0
                    ksl = slice(128 * kt, 128 * (kt + 1))
                    st = st_ps.tile([128, 2, 512], F32, tag="st")
                    nc.tensor.matmul(
                        st[:, 0, col0:], kts[p][0:64, ksl], qt[0:64, col0:])
                    nc.tensor.matmul(
                        st[:, 1, col0:], kts[p][64:128, ksl], qt[64:128, col0:])
                    pt = pt_pool.tile([128, 2, 512], BF16, tag="pt")
                    nc.scalar.activation(
                        pt[:, :, col0:], st[:, :, col0:], AF.Exp,
                        scale=SCALE / (WSCALE * WSCALE))
                    if rdiag >= 0:
                        # keep P^T[k, q] only where q >= k (within-block);
                        # one select covers both heads via a stride-0 dim
                        nc.gpsimd.affine_select(
                            out=pt[:, :, col0:col0 + 128],
                            in_=pt[:, :, col0:col0 + 128],
                            compare_op=mybir.AluOpType.is_ge,
                            fill=0.0, base=0, pattern=[[0, 2], [1, 128]],
                            channel_multiplier=-1)
                    state["carry"].append((kt, pos, rdiag, pt))

                def emit_av(kt, pos, rdiag, pt, p=p, state=state, nkt=nkt,
                            soff=soff):
                    av = state["av"]
                    s0 = max(rdiag, 0)
                    last = pos == nkt - 1
                    for s in range(s0, TG // 128):
                        for h in (0, 1):
                            nc.tensor.matmul(
                                av[:, 128 * s + 64 * h:128 * s + 64 * h + 64],
                                pt[:, h, 128 * s:128 * (s + 1)],
                                v1[:, 2 * p + h, kt, :],
                                start=state["first"],
                                stop=(last and s == 3 and h == 1))
                            state["first"] = False
                            nc.tensor.matmul(
                                sums[:, soff + 4 * h + s:soff + 4 * h + s + 1],
                                pt[:, h, 128 * s:128 * (s + 1)],
                                ones_bf[:],
                                start=False, stop=False,
                                skip_group_check=True)

                state["emit_av"] = emit_av

                def emit_block(mm, state, p, soff, stop_last):
                    av = state["av"]
                    for i, (isd, kt, pt, s, h) in enumerate(mm):
                        nc.tensor.matmul(
                            av[:, 128 * s + 64 * h:128 * s + 64 * h + 64],
                            pt[:, h, 128 * s:128 * (s + 1)],
                            v1[:, 2 * p + h, kt, :],
                            start=state["first"],
                            stop=(stop_last and i == len(mm) - 1))
                        state["first"] = False
                        nc.tensor.matmul(
                            sums[:, soff + 4 * h + s:soff + 4 * h + s + 1],
                            pt[:, h, 128 * s:128 * (s + 1)],
                            ones_bf[:],
                            start=False, stop=False,
                            skip_group_check=True)

                def flush_unit(p=p, state=state, soff=soff):
                    # flush the carried non-diagonal AVs at pair end; the
                    # affine-masked diagonal blocks wait for Pool latency, so
                    # they are deferred into finish_unit (emitted two k-tile
                    # units into the NEXT pair) to avoid head-blocking the
                    # PE's 4-deep dependency wait queue
                    mm = []
                    for kt, pos, rdiag, pt in state["carry"]:
                        for s in range(max(rdiag, 0), TG // 128):
                            for h in (0, 1):
                                mm.append((s == rdiag, kt, pt, s, h))
                    state["carry"] = []
                    mm.sort(key=lambda t: t[0])
                    ndiag = sum(1 for t in mm if t[0])
                    split = len(mm) - ndiag
                    emit_block(mm[:split], state, p, soff, stop_last=False)
                    state["diag"] = mm[split:]

                def norm_unit(p=p, state=state, soff=soff):
                    emit_block(state.pop("diag"), state, p, soff,
                               stop_last=True)
                    av = state["av"]
                    rc = rc_pool.tile([128, 2, 4], F32, tag="rc")
                    nc.vector.reciprocal(
                        rc[:], sums[:, soff:soff + 8].rearrange(
                            "p (h s) -> p h s", h=2))
                    for s in range(TG // 128):
                        nc.vector.tensor_mul(
                            an_tiles[s][:, 128 * p:128 * (p + 1)].rearrange(
                                "p (h d) -> p h d", h=2),
                            av[:, 128 * s:128 * (s + 1)].rearrange(
                                "p (h d) -> p h d", h=2),
                            rc[:, :, s:s + 1].broadcast_to([128, 2, 64]))

                pair_units = []
                kt_order = list(range(nkt))
                if g == n_tg - 1 and p == PAIRS - 1:
                    # last pair of the last group: diagonal k-tiles first so
                    # the wind-down tail has no Pool(affine) round-trip on
                    # its critical path
                    kt_order = kt_order[4 * g:] + kt_order[:4 * g]
                for pos, kt in enumerate(kt_order):
                    pair_units.append(
                        lambda kt=kt, pos=pos, f=kt_unit: f(kt, pos))
                if pending_finish:
                    pair_units.insert(min(6, max(2, len(pair_units) - 2)),
                                      pending_finish.pop())
                units += pair_units
                units.append(flush_unit)
                pending_finish.append(norm_unit)

            if pending_finish:
                units.append(pending_finish.pop())

            ats = {}

            def trans_unit(s):
                ps = scratch_ps(BF16)
                for c in range(PAIRS):
                    nc.tensor.transpose(
                        ps[:, 128 * c:128 * (c + 1)],
                        an_tiles[s][:, 128 * c:128 * (c + 1)],
                        ident_bf[:])
                # fp8 hi/lo split of attnT for the DoubleRow out-projection.
                # The hi copy runs on the (mostly idle) Pool engine so the
                # DVE isn't the serial bottleneck of the projection drain.
                ath = at_pool.tile([128, 512], FP8, tag="ath")
                atl = at_pool.tile([128, 512], FP8, tag="atl")
                if _ps_state["wide"]:
                    # final drain: ACT is idle and DVE is the serial
                    # bottleneck, so split the quant pair across both
                    nc.scalar.copy(ath[:], ps[:])
                else:
                    nc.vector.tensor_copy(out=ath[:], in_=ps[:])
                nc.vector.tensor_sub(atl[:], ps[:], ath[:])
                ats[s] = (ath, atl)

            def oproj_unit(s, nh):
                row0 = g * TG + 128 * s
                ath, atl = ats[s]
                ps = scratch_ps(F32)
                nsl = slice(512 * nh, 512 * (nh + 1))
                first = True
                for at_, wo in ((ath, wouth_sb), (ath, woutl_sb),
                                (atl, wouth_sb)):
                    for cp in range(2):
                        nc.tensor.matmul(
                            ps[:, :512],
                            at_[:, 256 * cp:256 * (cp + 1)].rearrange(
                                "p (c m) -> p c m", c=2),
                            wo[cp][:, :, nsl],
                            start=first,
                            stop=(at_ is atl and cp == 1),
                            perf_mode=DR)
                        first = False
                ob = ob_pool.tile([128, 512], BF16, tag="ob")
                if _ps_state["wide"] and (s + nh) % 2 == 0:
                    nc.scalar.copy(ob[:], ps[:, :512])
                else:
                    nc.vector.tensor_copy(out=ob[:], in_=ps[:, :512])
                # in the final drain the sync HWDGE queue serializes the last
                # 8 output stores (~625ns descriptor-gen each); spread them
                # over the idle queues so the tail isn't gated on it
                eng = (nc.sync if not _ps_state["wide"] else
                       (nc.sync, nc.scalar, nc.gpsimd)[(3 - s + nh) % 3])
                eng.dma_start(
                    out=out[row0:row0 + 128, 512 * nh:512 * (nh + 1)],
                    in_=ob[:])

            ounits = []
            for s in range(TG // 128):
                ounits.append(lambda s=s: trans_unit(s))
            for s in range(TG // 128):
                for nh in range(2):
                    ounits.append(lambda s=s, nh=nh: oproj_unit(s, nh))
            return units, ounits

        def interleave(a_units, b_units, pre_b=0):
            # pre_b: emit that many b-units before any a-unit (phase 0 needs
            # the group-0 V chains emitted before the first AV flush so the
            # tile framework sees the writes first)
            for u in b_units[:pre_b]:
                u()
            b_units = b_units[pre_b:]
            na, nb = len(a_units), len(b_units)
            ia = ib = 0
            while ia < na or ib < nb:
                fa = (na - ia) / na if na else 0.0
                fb = (nb - ib) / nb if nb else 0.0
                if ia < na and (fa > fb or ib >= nb):
                    a_units[ia]()
                    ia += 1
                else:
                    b_units[ib]()
                    ib += 1

        # prologue: group 0 projection (weight DMAs after group 0's x loads).
        # The scores/AV banks are idle here, so scratch rotates through them.
        set_wide_scratch(True, banked=True)
        # fill the initial x/w DMA latency with dummy PE work (also completes
        # the tensor engine's p-state ramp before real work lands); plain
        # matmuls on a memset tile need no identity, so they start ~1us in
        dummy = const.tile([128, 256], BF16, tag="dummy")
        nc.vector.memset(dummy[:], 0.0)
        xt0, tunits0 = transpose_units(0)
        for u in tunits0:
            u()
        load_weights()
        make_consts()
        # SWDGE gate: the gpsimd descriptor ring would otherwise eagerly
        # prepare the later-group x / wout transfers at t~1us and their
        # transfers would jump the (serial) DMA lane ahead of the
        # prologue-critical weight transfers. A tiny Pool-engine copy that
        # waits on the first x tile holds the ring back until the prologue
        # transfers are in flight.
        gate = const.tile([1, 1], FP8, tag="gate")
        nc.gpsimd.tensor_copy(out=gate[:], in_=xt0[1][0:1, CH - 1, 0:1])
        for _ in range(NDUMMY):
            wps = scratch_ps(F32)
            nc.tensor.matmul(wps[:, :256], dummy[:, 0:128], dummy[:],
                             start=True, stop=True)
        # prologue runs only the q and k chains (their weights land first);
        # group 0's V chains are deferred into the phase-0 fill, where they
        # interleave with the (V-independent) score units while the V-column
        # weights land. Unit order follows the DMA landing order: all h=0
        # halves (chunks 0-3) before h=1 (chunks 4-7). This holds up to 7
        # open psum accumulations (4 q + 3 k), which the banked scratch
        # rotation provides; k p3 waits until the q psums close.
        _ps_state["i"] = 0
        q0, k0, v0 = qkv_units(0, xt0)
        for u in [q0[0], q0[1], q0[2], q0[3],      # q h0 p0-3
                  k0[0], k0[1], k0[2],             # k h0 p0-2
                  q0[4], q0[5], q0[6], q0[7],      # q h1 p0-3 (close q psums)
                  k0[3],                           # k h0 p3
                  k0[4], k0[5], k0[6], k0[7]]:     # k h1 p0-3
            u()
        set_wide_scratch(False)
        # steady state: attention(g) interleaved with transposes(g+1) +
        # projection(g+1); out-projections are deferred up to two groups so
        # the last (largest) attention group still has dense PE fill
        pending_oproj = []  # deferred out-projection unit lists, oldest first
        fill_carry = v0     # group g's V chains lead the phase-g fill: their
        # only consumer is the AV flush (safely late in emission), and the
        # ACT-bound later phases need the q/k fill to shrink, not grow
        for g in range(n_tg):
            attn, ounits = attention_units(g)
            fill = fill_carry
            fill_carry = []
            if g + 1 < n_tg:
                xt1, tunits = transpose_units(g + 1)
                qu, ku, vu = qkv_units(g + 1, xt1)
                fill += tunits + ([load_wout] if g == 0 else []) + qu + ku
                fill_carry = vu
            if False and pending_oproj:
                # phase 2 is ACT(exp)-bound with PE slack: give it the
                # oldest deferred out-projection group
                fill += pending_oproj.pop(0)
            if g + 1 == n_tg:
                if g == 0:
                    fill.append(load_wout)
                # last group is exp(ACT)-bound and has no next-group
                # projection: feed it the remaining deferred out-projections
                while pending_oproj:
                    fill += pending_oproj.pop(0)
            interleave(attn, fill, pre_b=len(v0) if g == 0 else 0)
            pending_oproj.append(ounits)
        # final drain: attention is done, scores/AV banks are idle again
        set_wide_scratch(True)
        for ou in pending_oproj:
            for u in ou:
                u()


def build_program(with_bias, seq_len=T):
    nc = bacc.Bacc("TRN2", target_bir_lowering=False, debug=False,
                   enable_asserts=False, num_devices=8)
    xh = nc.dram_tensor("xh", [D, seq_len], FP8, kind="ExternalInput").ap()
    xl = nc.dram_tensor("xl", [D, seq_len], FP8, kind="ExternalInput").ap()
    wqh = nc.dram_tensor("wqh", [D, 3 * HPC * HD], FP8,
                         kind="ExternalInput").ap()
    wql = nc.dram_tensor("wql", [D, 3 * HPC * HD], FP8,
                         kind="ExternalInput").ap()
    wouth = nc.dram_tensor("wouth", [HPC * HD, D], FP8,
                           kind="ExternalInput").ap()
    woutl = nc.dram_tensor("woutl", [HPC * HD, D], FP8,
                           kind="ExternalInput").ap()
    out = nc.dram_tensor("out", [seq_len, D], BF16,
                         kind="ExternalOutput").ap()
    bqkv = None
    if with_bias:
        bqkv = nc.dram_tensor("bqkv", [1, 3 * HPC * HD], F32,
                              kind="ExternalInput").ap()
    with tile.TileContext(nc) as tc:
        build_tile_program(tc, xh, xl, wqh, wql, wouth, woutl, out, bqkv,
                           seq_len=seq_len)
    nc.compile()
    return nc


_PROGRAM_CACHE = {}


def _get_program(with_bias):
    if with_bias not in _PROGRAM_CACHE:
        _PROGRAM_CACHE[with_bias] = build_program(with_bias)
    return _PROGRAM_CACHE[with_bias]


def _split8(a):
    """fp8(e4m3) hi/lo split: a ~= hi + lo to ~7 mantissa bits."""
    import ml_dtypes
    hi = a.astype(ml_dtypes.float8_e4m3)
    lo = (a - hi.astype(np.float32)).astype(ml_dtypes.float8_e4m3)
    return hi, lo


def make_core_inputs(x_core, wqkv_core, wout_core, b_core=None):
    """Quantized inputs for ONE core: x_core [T, D], wqkv_core [D, 1536]
    (q|k|v grouped), wout_core [512, D]."""
    xh, xl = _split8(np.ascontiguousarray(x_core.T))
    wqh, wql = _split8(np.ascontiguousarray(wqkv_core) * WSCALE)
    wouth, woutl = _split8(np.ascontiguousarray(wout_core) * WSCALE)
    m = {
        "xh": xh, "xl": xl, "wqh": wqh, "wql": wql,
        "wouth": wouth, "woutl": woutl,
    }
    if b_core is not None:
        m["bqkv"] = np.ascontiguousarray(b_core * WSCALE).reshape(
            1, -1).astype(np.float32)
    return m


def make_in_maps(x, w_qkv, b_qkv, w_out, with_bias):
    """Per-core input dicts: core c -> batch c//2, head group c%2."""
    in_maps = []
    for core in range(8):
        b, gr = divmod(core, 2)
        qc = slice(512 * gr, 512 * (gr + 1))
        kc = slice(D + 512 * gr, D + 512 * (gr + 1))
        vc = slice(2 * D + 512 * gr, 2 * D + 512 * (gr + 1))
        wq = np.concatenate([w_qkv[:, qc], w_qkv[:, kc], w_qkv[:, vc]], axis=1)
        bq = (np.concatenate([b_qkv[qc], b_qkv[kc], b_qkv[vc]])
              if with_bias else None)
        in_maps.append(make_core_inputs(
            x[b], wq, w_out[512 * gr:512 * (gr + 1), :], bq))
    return in_maps


def kernel(x, w_qkv, b_qkv, w_out, b_out):
    x = np.asarray(x, dtype=np.float32)
    w_qkv = np.asarray(w_qkv, dtype=np.float32)
    b_qkv = np.asarray(b_qkv, dtype=np.float32)
    w_out = np.asarray(w_out, dtype=np.float32)
    b_out = np.asarray(b_out, dtype=np.float32)

    with_bias = bool(np.any(b_qkv))
    nc = _get_program(with_bias)
    in_maps = make_in_maps(x, w_qkv, b_qkv, w_out, with_bias)
    res = run_bass_kernel_spmd(nc, in_maps, core_ids=list(range(8))).results

    out = np.empty((B, T, D), dtype=np.float32)
    for b in range(B):
        out[b] = ((res[2 * b]["out"].astype(np.float32)
                   + res[2 * b + 1]["out"].astype(np.float32)) * OUT_SCALE
                  + b_out[None, :])
    return out



# revision 79
# speedup vs baseline: 1.0211x; 1.0031x over previous
"""Causal self-attention (B=4, T=2048, D=1024, H=16, hd=64) on 8 trn2 cores.

Sharding: 4-way data parallel over batch x 2-way tensor parallel over heads.
Core c handles batch c//2 and heads [8*(c%2), 8*(c%2)+8). Each core computes
its heads' partial contribution to the out-projection (a full [T, D] tensor);
the host sums the two head-group partials per batch and adds b_out.

Host-side prep: x is transposed to xT [D, T] and split into fp8(e4m3)
hi + lo residual planes; w_qkv (x32) and w_out (x32) likewise. The QKV and
out-projections then run as fp8 DoubleRow matmuls (2 K-chunks per pass, 0.5
cycles/row = 4x bf16 MAC rate) with three hi/lo cross terms (hi*hi + hi*lo +
lo*hi, the lo*lo term dropped), which is 0.75x the bf16 cost at ~bf16
accuracy. The weight x32 scaling keeps the lo planes out of fp8's subnormal
range; the softmax scale absorbs 1/1024 and the host divides the output
partials by 1024. End-to-end rel err vs the fp32 reference is ~4.4e-3
(gate: 2e-2).

Per-core dataflow (single NeuronCore, Tile framework):
  1. xT hi/lo stream straight from DRAM into SBUF (no on-device work).
  2. qT/kT [hd,T] are computed head-PAIR-stacked ([128,T] per pair) with
     w_qkv chunks as stationary DoubleRow fp8 matmuls; V [T,hd] per head is
     computed naturally and scattered into v1 (64 cols per (head, k-tile),
     bf16).
  3. Scores are computed TRANSPOSED: S^T[k,q] = K @ Q^T via two K=64 matmuls
     per (pair, k-tile) into one [128, 2, 512] two-bank PSUM tile; ONE wide
     exp (ACT, scale folded in) produces P^T for both heads in bf16.
     Causality: k-tiles above the diagonal are skipped, diagonal blocks get
     an affine_select zero-mask after exp, partial k-tiles use narrower
     matmuls.
  4. AV runs NON-transposed: per (head, q-subtile 128) the P^T block is the
     STATIONARY operand and V [128,64] streams, so each matmul moves 64
     rows instead of 512. All 8 (head, subtile) accumulators of a pair pack
     into ONE psum bank (zero-on-first-touch after a single start=True).
     Softmax sums ride as 1-column matmuls (P^T stationary, ones moving)
     into a shared, DVE-zeroed sums bank; AV emission trails its exp by 8
     k-tile units (carry) so the PE never waits on ACT latency.
  5. Normalization is a DVE broadcast multiply (reciprocal of sums, then
     attn * recip -> bf16). The normalized attn [q, 512] is PE-transposed
     (bf16), then quantized to an fp8 hi/lo pair (DVE copy + subtract; the
     hi copy moves to ACT in the drain where DVE is the serial bottleneck)
     as the out-projection's stationary operand; out = attnT.T @ w_out
     accumulates as 6 DoubleRow matmuls over 2 feature-chunk-pairs x 3
     hi/lo terms. All out-projections are deferred into the last q-group's
     phase + final drain, which are otherwise exp(ACT)-bound.

Scheduling: emission-level software pipelining interleaves attention for
q-group g with the transposes + q/k projections of group g+1; each group's
V chains ride at the FRONT of its own phase (their only consumer, the AV
flush, is emitted safely later). PSUM (8 banks) is budgeted as 4 scores +
2 AV + 1 sums + 1 projection scratch; the prologue additionally uses the
score slots' second banks for 7 concurrent q/k accumulators, and the final
drain widens the scratch rotation to 5. The cost model serializes all DMA
transfers through one lane fed by two descriptor-gen queues (HWDGE ~625ns,
SWDGE ~1.2us per dma_start), so the prologue issues transfers in exact
consumption order and gates the eager SWDGE ring (later-group x, w_out)
behind a Pool-engine copy of the last group-0 x tile; dummy matmuls at t=0
cover the first DMA latency and the PE p-state ramp (full clock needs 3us
of continuous execution - any >100ns gap drops it to half rate).
"""

import numpy as np

import concourse.bass as bass
import concourse.mybir as mybir
import concourse.tile as tile
from concourse import bacc
from concourse.bass_utils import run_bass_kernel_spmd
from concourse.masks import make_identity

B, T, D = 4, 2048, 1024
H, HD = 16, 64
HPC = 8              # heads per core
PAIRS = HPC // 2
CH = D // 128        # K-chunks for the QKV projection
TG = 512             # T-group / q-group width
SCALE = 1.0 / 8.0    # 1/sqrt(HD)
NDUMMY = 20          # prologue PE-warm dummy matmuls

F32 = mybir.dt.float32
F32R = mybir.dt.float32r
BF16 = mybir.dt.bfloat16
FP8 = mybir.dt.float8e4
DR = mybir.MatmulPerfMode.DoubleRow
AF = mybir.ActivationFunctionType

# w_qkv and w_out are scaled by 2**5 on the host so their fp8(e4m3) hi/lo
# splits stay out of the subnormal range; q', k', v' come out 32x larger, so
# the softmax scale absorbs 1/(32*32) and the device output is 32*32x the
# true projection (divided back out on the host).
WSCALE = 32.0
OUT_SCALE = 1.0 / (WSCALE * WSCALE)


def _r(ap):
    return ap.bitcast(F32R)


def build_tile_program(tc, xh, xl, wqh, wql, wouth, woutl, out, bqkv=None,
                       seq_len=T):
    """Emit the per-core program. seq_len is parametrized for small-scale
    simulation tests; the real kernel uses seq_len=T=2048.

    Emission is software-pipelined: attention for q-group g (latency-bound
    serial chains sT -> exp -> mask -> AV) is interleaved at emission level
    with the transposes + QKV projection of group g+1 (dense PE work), so the
    in-order PE fills attention's dependency stalls with projection matmuls.
    """
    nc = tc.nc
    n_tg = seq_len // TG
    n_tt = seq_len // 128
    with_bias = bqkv is not None

    from contextlib import ExitStack
    with ExitStack() as ctx:
        const = ctx.enter_context(tc.tile_pool(name="const", bufs=1))
        wpool = ctx.enter_context(tc.tile_pool(name="wpool", bufs=1))
        xt_pool = ctx.enter_context(tc.tile_pool(name="xt", bufs=4))
        qt_pool = ctx.enter_context(tc.tile_pool(name="qt", bufs=2 * PAIRS))
        pt_pool = ctx.enter_context(tc.tile_pool(name="pt", bufs=14))
        an_pool = ctx.enter_context(tc.tile_pool(name="an", bufs=16))
        at_pool = ctx.enter_context(tc.tile_pool(name="at", bufs=16))
        ob_pool = ctx.enter_context(tc.tile_pool(name="ob", bufs=6))
        rc_pool = ctx.enter_context(tc.tile_pool(name="rc", bufs=2))
        mm_ps = ctx.enter_context(tc.tile_pool(name="mmps", bufs=1, space="PSUM"))
        st_ps = ctx.enter_context(tc.tile_pool(name="stps", bufs=2, space="PSUM"))
        av_ps = ctx.enter_context(tc.tile_pool(name="avps", bufs=2, space="PSUM"))
        sums_ps = ctx.enter_context(tc.tile_pool(name="sums", bufs=1, space="PSUM"))
        identity = const.tile([128, 128], F32, tag="ident")
        ident_bf = const.tile([128, 128], BF16, tag="identbf")
        ones_bf = const.tile([128, 1], BF16, tag="ones_bf")

        def make_consts():
            # deferred so the weight SWDGE generation isn't queued behind
            # make_identity on the Pool engine at t=0; the identity is only
            # needed by the (late) attnT transposes
            make_identity(nc, identity[:])
            nc.vector.tensor_copy(out=ident_bf[:], in_=identity[:])
            nc.vector.memset(ones_bf[:], 1.0)
        if with_bias:
            ones_f = const.tile([128, 64], F32, tag="ones_f")
            nc.vector.memset(ones_f[:], 1.0)
            b_sb = const.tile([1, 3 * HPC * HD], F32R, tag="bias")
            nc.sync.dma_start(out=b_sb[:], in_=bqkv.bitcast(F32R)[:])
            ones_row = const.tile([1, TG], F32R, tag="ones_row")
            nc.vector.tensor_copy(
                out=ones_row[:], in_=ones_f[0:1, 0:1].broadcast_to([1, TG]))

        # softmax sums: one persistent bank; cols = parity*32 + pair*8 + h*4+s
        sums = sums_ps.tile([128, 64], F32, tag="sums")

        # resident weights (fp8 hi/lo pairs); w_out pair-packed: pair p rows
        # [128p, 128p+128). The DMAs are deferred until after group 0's x
        # loads so the first transposes aren't queued behind the weight
        # traffic.
        wh_sb = wpool.tile([128, CH, 3 * HPC * HD], FP8, tag="wqh")
        wl_sb = wpool.tile([128, CH, 3 * HPC * HD], FP8, tag="wql")
        # wout fp8 hi/lo, chunk-PAIR packed for DoubleRow: tile cp holds rows
        # [256cp, 256cp+256) as [128, 2, D]
        wouth_sb = [wpool.tile([128, 2, D], FP8, tag=f"wouth{p}",
                               name=f"wouth{p}") for p in range(2)]
        woutl_sb = [wpool.tile([128, 2, D], FP8, tag=f"woutl{p}",
                               name=f"woutl{p}") for p in range(2)]

        def load_weights():
            # The cost model serializes ALL transfers through one DMA_ENGINES
            # lane (~bytes/360GB/s each), fed by two descriptor-gen lanes:
            # HWDGE (shared SP+ACT, 625ns/dma) and SWDGE (gpsimd, ~1.1us/dma).
            # The prologue is transfer-order-bound and the PE is in-order, so
            # transfers are issued in exact consumption order of the prologue
            # units (q chains h0/h1, k chains h0/h1, then V), with q/k weight
            # columns split so q unblocks first. All on the scalar HWDGE
            # queue; x rides the sync HWDGE queue; later-group x and wout
            # ride SWDGE.
            # q|k lo chunks 0-3 ride the otherwise-idle SWDGE lane (ahead
            # of the gate) so the first chains' hi*lo terms land earlier and
            # the HWDGE stream shortens
            nc.gpsimd.dma_start(
                out=wl_sb[:, 0:4, 0:1024],
                in_=wql[0:512, 0:1024].rearrange("(c p) n -> p c n", p=128))
            for c0 in (0, 4):
                for w_sb, wsrc in (((wh_sb, wqh),) if c0 == 0 else
                                   ((wh_sb, wqh), (wl_sb, wql))):
                    nc.scalar.dma_start(
                        out=w_sb[:, c0:c0 + 4, 0:1024],
                        in_=wsrc[128 * c0:128 * (c0 + 4), 0:1024].rearrange(
                            "(c p) n -> p c n", p=128))
            for w_sb, wsrc in ((wh_sb, wqh), (wl_sb, wql)):
                for c0 in range(0, CH, 4):
                    nc.scalar.dma_start(
                        out=w_sb[:, c0:c0 + 4, 1024:1536],
                        in_=wsrc[128 * c0:128 * (c0 + 4), 1024:1536].rearrange(
                            "(c p) n -> p c n", p=128))

        def load_wout():
            # deferred into the phase-0 fill (behind the group-1 x loads on
            # SWDGE): first consumed by the deferred group-0 out-projections
            # in phase 1 (~45us in)
            with tc.tile_wait_until(0.016):
                for cp in range(2):
                    for w_sb, wsrc in ((wouth_sb, wouth), (woutl_sb, woutl)):
                        nc.gpsimd.dma_start(
                            out=w_sb[cp][:],
                            in_=wsrc[256 * cp:256 * (cp + 1), :].rearrange(
                                "(c p) n -> p c n", p=128))
        # kT per pair, head 2p on partitions [0:64), head 2p+1 on [64:128)
        kts = [wpool.tile([128, seq_len], BF16, tag=f"kt{p}", name=f"kt{p}")
               for p in range(PAIRS)]
        # V in bf16: per (head, k-tile) a [128, 64] stationary block
        v1 = wpool.tile([128, HPC, n_tt, HD], BF16, tag="v1")

        qts_of = {}  # g -> [qt tiles per pair]

        # psum scratch for the projection/transpose/oproj chains. In steady
        # state only the mm bank is free; in the prologue and final drain the
        # scores/AV banks are idle, so rotate through them too (the st slot is
        # [128,2,512]; its first bank is used as a [128,512] scratch).
        # "banked" mode (prologue only, F32) additionally uses the st slots'
        # SECOND psum banks (zero regions are 2KB, so the two banks of one
        # slot hold independent accumulation groups), giving 7 concurrent
        # scratch accumulators for the q+k chain overlap.
        _ps_state = {"wide": False, "banked": False, "i": 0, "st": {}}

        def set_wide_scratch(wide, banked=False):
            _ps_state["wide"] = wide
            _ps_state["banked"] = banked

        def scratch_ps(dtype):
            if not _ps_state["wide"]:
                return mm_ps.tile([128, 512], dtype, tag="mm", name="mm")
            if _ps_state["banked"] and dtype == F32:
                i = _ps_state["i"] = (_ps_state["i"] + 1) % 7
                if i == 0:
                    return mm_ps.tile([128, 512], dtype, tag="mm", name="mm")
                if i in (1, 2, 3, 4):
                    buf, bank = divmod(i - 1, 2)
                    if bank == 0:
                        _ps_state["st"][buf] = st_ps.tile(
                            [128, 2, 512], dtype, tag="st", name="stx")
                    return _ps_state["st"][buf][:, bank, :]
                return av_ps.tile([128, 512], dtype, tag="av", name="avx")
            i = _ps_state["i"] = (_ps_state["i"] + 1) % 5
            if i == 0:
                return mm_ps.tile([128, 512], dtype, tag="mm", name="mm")
            if i in (1, 2):
                return st_ps.tile([128, 512], dtype, tag="st", name="stx")
            return av_ps.tile([128, 512], dtype, tag="av", name="avx")

        def transpose_units(g):
            """x arrives pre-transposed and fp8-hi/lo-split from the host:
            just DMA the group's xT columns. Group 0 (prologue, latency-
            critical) goes on the sync HWDGE queue split in half-chunks;
            later groups ride the then-idle SWDGE ring as one transfer per
            hi/lo half."""
            xt = (xt_pool.tile([128, CH, TG], FP8, tag="xth", name=f"xth{g}"),
                  xt_pool.tile([128, CH, TG], FP8, tag="xtl", name=f"xtl{g}"))
            gsl = slice(g * TG, (g + 1) * TG)

            def u():
                if g == 0:
                    # lo directly after hi per chunk-half: every chain's
                    # third (lo*hi) term needs x-lo early
                    for c0 in (0, 4):
                        for t, src in ((xt[0], xh), (xt[1], xl)):
                            nc.sync.dma_start(
                                out=t[:, c0:c0 + 4, :],
                                in_=src[128 * c0:128 * (c0 + 4), gsl].rearrange(
                                    "(c p) n -> p c n", p=128))
                else:
                    # wait-ts biases the tile scheduler so these preps don't
                    # jump the serial DMA lane ahead of the prologue-critical
                    # weight transfers
                    with tc.tile_wait_until(0.010 + 0.038 * (g - 1)):
                        for t, src in ((xt[0], xh), (xt[1], xl)):
                            nc.gpsimd.dma_start(
                                out=t[:, :, :],
                                in_=src[:, gsl].rearrange(
                                    "(c p) n -> p c n", p=128))
            return xt, [u]

        def qkv_units(g, xt):
            """12 units: 4 qt chains, 4 kT chains, 4 V chains."""
            qts = qts_of.setdefault(g, [])
            units = []

            half = {}

            xth, xtl = xt

            def qk_chain(p, qk, h):
                # split into two half-chains (finer interleave granularity).
                # Each half contracts 4 D-chunks = 2 fp8 DoubleRow chunk-pairs
                # x 3 hi/lo cross terms (lo*lo dropped). Terms are ordered
                # hi*hi first so the prologue chains can start as soon as the
                # hi transfers land (lo halves trickle in later).
                if h == 0:
                    half[(p, qk)] = scratch_ps(F32)
                ps = half[(p, qk)]
                col = qk * 512 + p * 128
                first = True
                for wt, xt_ in ((wh_sb, xth), (wl_sb, xth), (wh_sb, xtl)):
                    for c0 in range(4 * h, 4 * h + 4, 2):
                        cs = slice(c0, c0 + 2)
                        nc.tensor.matmul(
                            ps[:, :TG], wt[:, cs, col:col + 128],
                            xt_[:, cs, :],
                            start=(h == 0 and first),
                            stop=(h == 1 and c0 == CH - 2 and xt_ is xtl
                                  and not with_bias),
                            perf_mode=DR)
                        first = False
                if h == 0:
                    return
                if with_bias:
                    nc.tensor.matmul(
                        ps[:, :TG], b_sb[0:1, col:col + 128],
                        ones_row[0:1, :], start=False, stop=True)
                if qk == 0:
                    qt = qt_pool.tile([128, TG], BF16, tag="qt")
                    nc.vector.tensor_copy(out=qt[:], in_=ps[:, :TG])
                    qts.append(qt)
                else:
                    nc.vector.tensor_copy(
                        out=kts[p][:, g * TG:(g + 1) * TG], in_=ps[:, :TG])

            def v_chain(t4):
                tt = g * (TG // 128) + t4
                ps = scratch_ps(F32)
                first = True
                tsl = slice(128 * t4, 128 * (t4 + 1))
                for xt_, wt in ((xth, wh_sb), (xth, wl_sb), (xtl, wh_sb)):
                    for c0 in range(0, CH, 2):
                        cs = slice(c0, c0 + 2)
                        nc.tensor.matmul(
                            ps[:, :512], xt_[:, cs, tsl],
                            wt[:, cs, 1024:1536],
                            start=first,
                            stop=(c0 == CH - 2 and xt_ is xtl
                                  and not with_bias),
                            perf_mode=DR)
                        first = False
                if with_bias:
                    nc.tensor.matmul(
                        ps[:, :512], ones_row[0:1, 0:128],
                        b_sb[0:1, 1024:1536], start=False, stop=True)
                nc.vector.tensor_copy(
                    out=v1[:, :, tt, :],
                    in_=ps[:, :512].rearrange("p (h d) -> p h d", h=HPC))

            k_units = []
            v_units = []
            # in the prologue the qk chains are gated on their W chunks
            # landing (~1us apart): run all h=0 half-chains (chunks 0-3)
            # before any h=1 (chunks 4-7). Requires 4 concurrent psum
            # accumulators - only legal in the wide-scratch prologue.
            hmajor = _ps_state["wide"]
            for lst, qk in ((units, 0), (k_units, 1)):
                if hmajor:
                    for h in range(2):
                        for p in range(PAIRS):
                            lst.append(lambda p=p, h=h, qk=qk: qk_chain(p, qk, h))
                else:
                    for p in range(PAIRS):
                        lst.append(lambda p=p, qk=qk: qk_chain(p, qk, 0))
                        lst.append(lambda p=p, qk=qk: qk_chain(p, qk, 1))
            for t4 in range(TG // 128):
                v_units.append(lambda t4=t4: v_chain(t4))
            return units, k_units, v_units

        def attention_units(g):
            """Per pair: one unit per k-tile (sT+exp+mask, AV carried by one),
            then a normalization unit; finally the out-projection units."""
            units = []
            pending_finish = []
            an_tiles = [an_pool.tile([128, 512], BF16, tag="an",
                                     name=f"an_g{g}_s{s}")
                        for s in range(TG // 128)]
            # zero this group's sums columns once (all 4 pairs' 8-col slices)
            nc.vector.memset(sums[:, (g % 2) * 32:(g % 2) * 32 + 32], 0.0)
            qts = qts_of[g]
            for p in range(PAIRS):
                nkt = 4 * (g + 1)
                soff = (g % 2) * 32 + p * 8
                state = {}

                def start_pair(p=p, state=state, soff=soff):
                    state["av"] = av_ps.tile([128, 512], F32, tag="av",
                                             name=f"av_g{g}_p{p}")
                    state["carry"] = []
                    state["first"] = True

                def kt_unit(kt, pos, p=p, state=state, nkt=nkt,
                            sp=start_pair):
                    if pos == 0:
                        sp()
                    # pop the carried AV FIRST: it is ready now, and the
                    # scores below may head-block on a busy st slot
                    if len(state["carry"]) >= 9:
                        state["emit_av"](*state["carry"].pop(0))
                    qt = qts[p]
                    rdiag = kt - 4 * g
                    col0 = 128 * rdiag if rdiag > 0 else 0
                    ksl = slice(128 * kt, 128 * (kt + 1))
                    st = st_ps.tile([128, 2, 512], F32, tag="st")
                    nc.tensor.matmul(
                        st[:, 0, col0:], kts[p][0:64, ksl], qt[0:64, col0:])
                    nc.tensor.matmul(
                        st[:, 1, col0:], kts[p][64:128, ksl], qt[64:128, col0:])
                    pt = pt_pool.tile([128, 2, 512], BF16, tag="pt")
                    nc.scalar.activation(
                        pt[:, :, col0:], st[:, :, col0:], AF.Exp,
                        scale=SCALE / (WSCALE * WSCALE))
                    if rdiag >= 0:
                        # keep P^T[k, q] only where q >= k (within-block);
                        # one select covers both heads via a stride-0 dim
                        nc.gpsimd.affine_select(
                            out=pt[:, :, col0:col0 + 128],
                            in_=pt[:, :, col0:col0 + 128],
                            compare_op=mybir.AluOpType.is_ge,
                            fill=0.0, base=0, pattern=[[0, 2], [1, 128]],
                            channel_multiplier=-1)
                    state["carry"].append((kt, pos, rdiag, pt))

                def emit_av(kt, pos, rdiag, pt, p=p, state=state, nkt=nkt,
                            soff=soff):
                    av = state["av"]
                    s0 = max(rdiag, 0)
                    last = pos == nkt - 1
                    for s in range(s0, TG // 128):
                        for h in (0, 1):
                            nc.tensor.matmul(
                                av[:, 128 * s + 64 * h:128 * s + 64 * h + 64],
                                pt[:, h, 128 * s:128 * (s + 1)],
                                v1[:, 2 * p + h, kt, :],
                                start=state["first"],
                                stop=(last and s == 3 and h == 1))
                            state["first"] = False
                            nc.tensor.matmul(
                                sums[:, soff + 4 * h + s:soff + 4 * h + s + 1],
                                pt[:, h, 128 * s:128 * (s + 1)],
                                ones_bf[:],
                                start=False, stop=False,
                                skip_group_check=True)

                state["emit_av"] = emit_av

                def emit_block(mm, state, p, soff, stop_last):
                    av = state["av"]
                    for i, (isd, kt, pt, s, h) in enumerate(mm):
                        nc.tensor.matmul(
                            av[:, 128 * s + 64 * h:128 * s + 64 * h + 64],
                            pt[:, h, 128 * s:128 * (s + 1)],
                            v1[:, 2 * p + h, kt, :],
                            start=state["first"],
                            stop=(stop_last and i == len(mm) - 1))
                        state["first"] = False
                        nc.tensor.matmul(
                            sums[:, soff + 4 * h + s:soff + 4 * h + s + 1],
                            pt[:, h, 128 * s:128 * (s + 1)],
                            ones_bf[:],
                            start=False, stop=False,
                            skip_group_check=True)

                def flush_unit(p=p, state=state, soff=soff):
                    # flush the carried non-diagonal AVs at pair end; the
                    # affine-masked diagonal blocks wait for Pool latency, so
                    # they are deferred into finish_unit (emitted two k-tile
                    # units into the NEXT pair) to avoid head-blocking the
                    # PE's 4-deep dependency wait queue
                    mm = []
                    for kt, pos, rdiag, pt in state["carry"]:
                        for s in range(max(rdiag, 0), TG // 128):
                            for h in (0, 1):
                                mm.append((s == rdiag, kt, pt, s, h))
                    state["carry"] = []
                    mm.sort(key=lambda t: t[0])
                    ndiag = sum(1 for t in mm if t[0])
                    split = len(mm) - ndiag
                    emit_block(mm[:split], state, p, soff, stop_last=False)
                    state["diag"] = mm[split:]

                def norm_unit(p=p, state=state, soff=soff):
                    emit_block(state.pop("diag"), state, p, soff,
                               stop_last=True)
                    av = state["av"]
                    rc = rc_pool.tile([128, 2, 4], F32, tag="rc")
                    nc.vector.reciprocal(
                        rc[:], sums[:, soff:soff + 8].rearrange(
                            "p (h s) -> p h s", h=2))
                    for s in range(TG // 128):
                        nc.vector.tensor_mul(
                            an_tiles[s][:, 128 * p:128 * (p + 1)].rearrange(
                                "p (h d) -> p h d", h=2),
                            av[:, 128 * s:128 * (s + 1)].rearrange(
                                "p (h d) -> p h d", h=2),
                            rc[:, :, s:s + 1].broadcast_to([128, 2, 64]))

                pair_units = []
                kt_order = list(range(nkt))
                if g == n_tg - 1 and p == PAIRS - 1:
                    # last pair of the last group: diagonal k-tiles first so
                    # the wind-down tail has no Pool(affine) round-trip on
                    # its critical path
                    kt_order = kt_order[4 * g:] + kt_order[:4 * g]
                for pos, kt in enumerate(kt_order):
                    pair_units.append(
                        lambda kt=kt, pos=pos, f=kt_unit: f(kt, pos))
                if pending_finish:
                    pair_units.insert(min(6, max(2, len(pair_units) - 2)),
                                      pending_finish.pop())
                units += pair_units
                units.append(flush_unit)
                pending_finish.append(norm_unit)

            if pending_finish:
                units.append(pending_finish.pop())

            ats = {}

            def trans_unit(s):
                ps = scratch_ps(BF16)
                for c in range(PAIRS):
                    nc.tensor.transpose(
                        ps[:, 128 * c:128 * (c + 1)],
                        an_tiles[s][:, 128 * c:128 * (c + 1)],
                        ident_bf[:])
                # fp8 hi/lo split of attnT for the DoubleRow out-projection.
                # The hi copy runs on the (mostly idle) Pool engine so the
                # DVE isn't the serial bottleneck of the projection drain.
                ath = at_pool.tile([128, 512], FP8, tag="ath")
                atl = at_pool.tile([128, 512], FP8, tag="atl")
                if _ps_state["wide"]:
                    # final drain: ACT is idle and DVE is the serial
                    # bottleneck, so split the quant pair across both
                    nc.scalar.copy(ath[:], ps[:])
                else:
                    nc.vector.tensor_copy(out=ath[:], in_=ps[:])
                nc.vector.tensor_sub(atl[:], ps[:], ath[:])
                ats[s] = (ath, atl)

            def oproj_unit(s, nh):
                row0 = g * TG + 128 * s
                ath, atl = ats[s]
                ps = scratch_ps(F32)
                nsl = slice(512 * nh, 512 * (nh + 1))
                first = True
                for at_, wo in ((ath, wouth_sb), (ath, woutl_sb),
                                (atl, wouth_sb)):
                    for cp in range(2):
                        nc.tensor.matmul(
                            ps[:, :512],
                            at_[:, 256 * cp:256 * (cp + 1)].rearrange(
                                "p (c m) -> p c m", c=2),
                            wo[cp][:, :, nsl],
                            start=first,
                            stop=(at_ is atl and cp == 1),
                            perf_mode=DR)
                        first = False
                ob = ob_pool.tile([128, 512], BF16, tag="ob")
                if _ps_state["wide"] and (s + nh) % 2 == 0:
                    nc.scalar.copy(ob[:], ps[:, :512])
                else:
                    nc.vector.tensor_copy(out=ob[:], in_=ps[:, :512])
                # in the final drain the sync HWDGE queue serializes the last
                # 8 output stores (~625ns descriptor-gen each); spread them
                # over the idle queues so the tail isn't gated on it
                eng = (nc.sync if not _ps_state["wide"] else
                       (nc.sync, nc.scalar, nc.gpsimd)[(3 - s + nh) % 3])
                eng.dma_start(
                    out=out[row0:row0 + 128, 512 * nh:512 * (nh + 1)],
                    in_=ob[:])

            ounits = []
            for s in range(TG // 128):
                ounits.append(lambda s=s: trans_unit(s))
            for s in range(TG // 128):
                for nh in range(2):
                    ounits.append(lambda s=s, nh=nh: oproj_unit(s, nh))
            return units, ounits

        def interleave(a_units, b_units, pre_b=0):
            # pre_b: emit that many b-units before any a-unit (phase 0 needs
            # the group-0 V chains emitted before the first AV flush so the
            # tile framework sees the writes first)
            for u in b_units[:pre_b]:
                u()
            b_units = b_units[pre_b:]
            na, nb = len(a_units), len(b_units)
            ia = ib = 0
            while ia < na or ib < nb:
                fa = (na - ia) / na if na else 0.0
                fb = (nb - ib) / nb if nb else 0.0
                if ia < na and (fa > fb or ib >= nb):
                    a_units[ia]()
                    ia += 1
                else:
                    b_units[ib]()
                    ib += 1

        # prologue: group 0 projection (weight DMAs after group 0's x loads).
        # The scores/AV banks are idle here, so scratch rotates through them.
        set_wide_scratch(True, banked=True)
        # fill the initial x/w DMA latency with dummy PE work (also completes
        # the tensor engine's p-state ramp before real work lands); plain
        # matmuls on a memset tile need no identity, so they start ~1us in
        dummy = const.tile([128, 256], BF16, tag="dummy")
        nc.vector.memset(dummy[:], 0.0)
        xt0, tunits0 = transpose_units(0)
        for u in tunits0:
            u()
        load_weights()
        make_consts()
        # SWDGE gate: the gpsimd descriptor ring would otherwise eagerly
        # prepare the later-group x / wout transfers at t~1us and their
        # transfers would jump the (serial) DMA lane ahead of the
        # prologue-critical weight transfers. A tiny Pool-engine copy that
        # waits on the first x tile holds the ring back until the prologue
        # transfers are in flight.
        gate = const.tile([1, 1], FP8, tag="gate")
        nc.gpsimd.tensor_copy(out=gate[:], in_=xt0[1][0:1, CH - 1, 0:1])
        for _ in range(NDUMMY):
            wps = scratch_ps(F32)
            nc.tensor.matmul(wps[:, :256], dummy[:, 0:128], dummy[:],
                             start=True, stop=True)
        # prologue runs only the q and k chains (their weights land first);
        # group 0's V chains are deferred into the phase-0 fill, where they
        # interleave with the (V-independent) score units while the V-column
        # weights land. Unit order follows the DMA landing order: all h=0
        # halves (chunks 0-3) before h=1 (chunks 4-7). This holds up to 7
        # open psum accumulations (4 q + 3 k), which the banked scratch
        # rotation provides; k p3 waits until the q psums close.
        _ps_state["i"] = 0
        q0, k0, v0 = qkv_units(0, xt0)
        for u in [q0[0], q0[1], q0[2], q0[3],      # q h0 p0-3
                  k0[0], k0[1], k0[2],             # k h0 p0-2
                  q0[4], q0[5], q0[6], q0[7],      # q h1 p0-3 (close q psums)
                  k0[3],                           # k h0 p3
                  k0[4], k0[5], k0[6], k0[7]]:     # k h1 p0-3
            u()
        set_wide_scratch(False)
        # steady state: attention(g) interleaved with transposes(g+1) +
        # projection(g+1); out-projections are deferred up to two groups so
        # the last (largest) attention group still has dense PE fill
        pending_oproj = []  # deferred out-projection unit lists, oldest first
        fill_carry = v0     # group g's V chains lead the phase-g fill: their
        # only consumer is the AV flush (safely late in emission), and the
        # ACT-bound later phases need the q/k fill to shrink, not grow
        for g in range(n_tg):
            attn, ounits = attention_units(g)
            fill = fill_carry
            fill_carry = []
            if g + 1 < n_tg:
                xt1, tunits = transpose_units(g + 1)
                qu, ku, vu = qkv_units(g + 1, xt1)
                fill += tunits + ([load_wout] if g == 0 else []) + qu + ku
                fill_carry = vu
            if False and pending_oproj:
                # phase 2 is ACT(exp)-bound with PE slack: give it the
                # oldest deferred out-projection group
                fill += pending_oproj.pop(0)
            if g + 1 == n_tg:
                if g == 0:
                    fill.append(load_wout)
                # last group is exp(ACT)-bound and has no next-group
                # projection: feed it the remaining deferred out-projections,
                # round-robin across groups so each projection trails its
                # transpose's fp8-quantize chain by 3x more slack
                lists = [list(o) for o in pending_oproj]
                pending_oproj.clear()
                while any(lists):
                    for o in lists:
                        if o:
                            fill.append(o.pop(0))
            interleave(attn, fill, pre_b=len(v0) if g == 0 else 0)
            pending_oproj.append(ounits)
        # final drain: attention is done, scores/AV banks are idle again
        set_wide_scratch(True)
        for ou in pending_oproj:
            for u in ou:
                u()


def build_program(with_bias, seq_len=T):
    nc = bacc.Bacc("TRN2", target_bir_lowering=False, debug=False,
                   enable_asserts=False, num_devices=8)
    xh = nc.dram_tensor("xh", [D, seq_len], FP8, kind="ExternalInput").ap()
    xl = nc.dram_tensor("xl", [D, seq_len], FP8, kind="ExternalInput").ap()
    wqh = nc.dram_tensor("wqh", [D, 3 * HPC * HD], FP8,
                         kind="ExternalInput").ap()
    wql = nc.dram_tensor("wql", [D, 3 * HPC * HD], FP8,
                         kind="ExternalInput").ap()
    wouth = nc.dram_tensor("wouth", [HPC * HD, D], FP8,
                           kind="ExternalInput").ap()
    woutl = nc.dram_tensor("woutl", [HPC * HD, D], FP8,
                           kind="ExternalInput").ap()
    out = nc.dram_tensor("out", [seq_len, D], BF16,
                         kind="ExternalOutput").ap()
    bqkv = None
    if with_bias:
        bqkv = nc.dram_tensor("bqkv", [1, 3 * HPC * HD], F32,
                              kind="ExternalInput").ap()
    with tile.TileContext(nc) as tc:
        build_tile_program(tc, xh, xl, wqh, wql, wouth, woutl, out, bqkv,
                           seq_len=seq_len)
    nc.compile()
    return nc


_PROGRAM_CACHE = {}


def _get_program(with_bias):
    if with_bias not in _PROGRAM_CACHE:
        _PROGRAM_CACHE[with_bias] = build_program(with_bias)
    return _PROGRAM_CACHE[with_bias]


def _split8(a):
    """fp8(e4m3) hi/lo split: a ~= hi + lo to ~7 mantissa bits."""
    import ml_dtypes
    hi = a.astype(ml_dtypes.float8_e4m3)
    lo = (a - hi.astype(np.float32)).astype(ml_dtypes.float8_e4m3)
    return hi, lo


def make_core_inputs(x_core, wqkv_core, wout_core, b_core=None):
    """Quantized inputs for ONE core: x_core [T, D], wqkv_core [D, 1536]
    (q|k|v grouped), wout_core [512, D]."""
    xh, xl = _split8(np.ascontiguousarray(x_core.T))
    wqh, wql = _split8(np.ascontiguousarray(wqkv_core) * WSCALE)
    wouth, woutl = _split8(np.ascontiguousarray(wout_core) * WSCALE)
    m = {
        "xh": xh, "xl": xl, "wqh": wqh, "wql": wql,
        "wouth": wouth, "woutl": woutl,
    }
    if b_core is not None:
        m["bqkv"] = np.ascontiguousarray(b_core * WSCALE).reshape(
            1, -1).astype(np.float32)
    return m


def make_in_maps(x, w_qkv, b_qkv, w_out, with_bias):
    """Per-core input dicts: core c -> batch c//2, head group c%2."""
    in_maps = []
    for core in range(8):
        b, gr = divmod(core, 2)
        qc = slice(512 * gr, 512 * (gr + 1))
        kc = slice(D + 512 * gr, D + 512 * (gr + 1))
        vc = slice(2 * D + 512 * gr, 2 * D + 512 * (gr + 1))
        wq = np.concatenate([w_qkv[:, qc], w_qkv[:, kc], w_qkv[:, vc]], axis=1)
        bq = (np.concatenate([b_qkv[qc], b_qkv[kc], b_qkv[vc]])
              if with_bias else None)
        in_maps.append(make_core_inputs(
            x[b], wq, w_out[512 * gr:512 * (gr + 1), :], bq))
    return in_maps


def kernel(x, w_qkv, b_qkv, w_out, b_out):
    x = np.asarray(x, dtype=np.float32)
    w_qkv = np.asarray(w_qkv, dtype=np.float32)
    b_qkv = np.asarray(b_qkv, dtype=np.float32)
    w_out = np.asarray(w_out, dtype=np.float32)
    b_out = np.asarray(b_out, dtype=np.float32)

    with_bias = bool(np.any(b_qkv))
    nc = _get_program(with_bias)
    in_maps = make_in_maps(x, w_qkv, b_qkv, w_out, with_bias)
    res = run_bass_kernel_spmd(nc, in_maps, core_ids=list(range(8))).results

    out = np.empty((B, T, D), dtype=np.float32)
    for b in range(B):
        out[b] = ((res[2 * b]["out"].astype(np.float32)
                   + res[2 * b + 1]["out"].astype(np.float32)) * OUT_SCALE
                  + b_out[None, :])
    return out



# revision 80
# speedup vs baseline: 1.0267x; 1.0055x over previous
"""Causal self-attention (B=4, T=2048, D=1024, H=16, hd=64) on 8 trn2 cores.

Sharding: 4-way data parallel over batch x 2-way tensor parallel over heads.
Core c handles batch c//2 and heads [8*(c%2), 8*(c%2)+8). Each core computes
its heads' partial contribution to the out-projection (a full [T, D] tensor);
the host sums the two head-group partials per batch and adds b_out.

Host-side prep: x is transposed to xT [D, T] and split into fp8(e4m3)
hi + lo residual planes; w_qkv (x32) and w_out (x32) likewise. The QKV and
out-projections then run as fp8 DoubleRow matmuls (2 K-chunks per pass, 0.5
cycles/row = 4x bf16 MAC rate) with three hi/lo cross terms (hi*hi + hi*lo +
lo*hi, the lo*lo term dropped), which is 0.75x the bf16 cost at ~bf16
accuracy. The weight x32 scaling keeps the lo planes out of fp8's subnormal
range; the softmax scale absorbs 1/1024 and the host divides the output
partials by 1024. End-to-end rel err vs the fp32 reference is ~4.4e-3
(gate: 2e-2).

Per-core dataflow (single NeuronCore, Tile framework):
  1. xT hi/lo stream straight from DRAM into SBUF (no on-device work).
  2. qT/kT [hd,T] are computed head-PAIR-stacked ([128,T] per pair) with
     w_qkv chunks as stationary DoubleRow fp8 matmuls; V [T,hd] per head is
     computed naturally and scattered into v1 (64 cols per (head, k-tile),
     bf16).
  3. Scores are computed TRANSPOSED: S^T[k,q] = K @ Q^T via two K=64 matmuls
     per (pair, k-tile) into one [128, 2, 512] two-bank PSUM tile; ONE wide
     exp (ACT, scale folded in) produces P^T for both heads in bf16.
     Causality: k-tiles above the diagonal are skipped, diagonal blocks get
     an affine_select zero-mask after exp, partial k-tiles use narrower
     matmuls.
  4. AV runs NON-transposed: per (head, q-subtile 128) the P^T block is the
     STATIONARY operand and V [128,64] streams, so each matmul moves 64
     rows instead of 512. All 8 (head, subtile) accumulators of a pair pack
     into ONE psum bank (zero-on-first-touch after a single start=True).
     Softmax sums ride as 1-column matmuls (P^T stationary, ones moving)
     into a shared, DVE-zeroed sums bank; AV emission trails its exp by 8
     k-tile units (carry) so the PE never waits on ACT latency.
  5. Normalization is a DVE broadcast multiply (reciprocal of sums, then
     attn * recip -> bf16). The normalized attn [q, 512] is PE-transposed
     (bf16), then quantized to an fp8 hi/lo pair (DVE copy + subtract; the
     hi copy moves to ACT in the drain where DVE is the serial bottleneck)
     as the out-projection's stationary operand; out = attnT.T @ w_out
     accumulates as 6 DoubleRow matmuls over 2 feature-chunk-pairs x 3
     hi/lo terms. All out-projections are deferred into the last q-group's
     phase + final drain, which are otherwise exp(ACT)-bound.

Scheduling: emission-level software pipelining interleaves attention for
q-group g with the transposes + q/k projections of group g+1; each group's
V chains ride at the FRONT of its own phase (their only consumer, the AV
flush, is emitted safely later). PSUM (8 banks) is budgeted as 4 scores +
2 AV + 1 sums + 1 projection scratch; the prologue additionally uses the
score slots' second banks for 7 concurrent q/k accumulators, and the final
drain widens the scratch rotation to 5. The cost model serializes all DMA
transfers through one lane fed by two descriptor-gen queues (HWDGE ~625ns,
SWDGE ~1.2us per dma_start), so the prologue issues transfers in exact
consumption order and gates the eager SWDGE ring (later-group x, w_out)
behind a Pool-engine copy of the last group-0 x tile; dummy matmuls at t=0
cover the first DMA latency and the PE p-state ramp (full clock needs 3us
of continuous execution - any >100ns gap drops it to half rate).
"""

import numpy as np

import concourse.bass as bass
import concourse.mybir as mybir
import concourse.tile as tile
from concourse import bacc
from concourse.bass_utils import run_bass_kernel_spmd
from concourse.masks import make_identity

B, T, D = 4, 2048, 1024
H, HD = 16, 64
HPC = 8              # heads per core
PAIRS = HPC // 2
CH = D // 128        # K-chunks for the QKV projection
TG = 512             # T-group / q-group width
SCALE = 1.0 / 8.0    # 1/sqrt(HD)
NDUMMY = 20          # prologue PE-warm dummy matmuls

F32 = mybir.dt.float32
F32R = mybir.dt.float32r
BF16 = mybir.dt.bfloat16
FP8 = mybir.dt.float8e4
DR = mybir.MatmulPerfMode.DoubleRow
AF = mybir.ActivationFunctionType

# w_qkv and w_out are scaled by 2**5 on the host so their fp8(e4m3) hi/lo
# splits stay out of the subnormal range; q', k', v' come out 32x larger, so
# the softmax scale absorbs 1/(32*32) and the device output is 32*32x the
# true projection (divided back out on the host).
WSCALE = 32.0
OUT_SCALE = 1.0 / (WSCALE * WSCALE)


def _r(ap):
    return ap.bitcast(F32R)


def build_tile_program(tc, xh, xl, wqh, wql, wouth, woutl, out, bqkv=None,
                       seq_len=T):
    """Emit the per-core program. seq_len is parametrized for small-scale
    simulation tests; the real kernel uses seq_len=T=2048.

    Emission is software-pipelined: attention for q-group g (latency-bound
    serial chains sT -> exp -> mask -> AV) is interleaved at emission level
    with the transposes + QKV projection of group g+1 (dense PE work), so the
    in-order PE fills attention's dependency stalls with projection matmuls.
    """
    nc = tc.nc
    n_tg = seq_len // TG
    n_tt = seq_len // 128
    with_bias = bqkv is not None

    from contextlib import ExitStack
    with ExitStack() as ctx:
        const = ctx.enter_context(tc.tile_pool(name="const", bufs=1))
        wpool = ctx.enter_context(tc.tile_pool(name="wpool", bufs=1))
        xt_pool = ctx.enter_context(tc.tile_pool(name="xt", bufs=4))
        qt_pool = ctx.enter_context(tc.tile_pool(name="qt", bufs=2 * PAIRS))
        pt_pool = ctx.enter_context(tc.tile_pool(name="pt", bufs=14))
        an_pool = ctx.enter_context(tc.tile_pool(name="an", bufs=16))
        at_pool = ctx.enter_context(tc.tile_pool(name="at", bufs=16))
        ob_pool = ctx.enter_context(tc.tile_pool(name="ob", bufs=6))
        rc_pool = ctx.enter_context(tc.tile_pool(name="rc", bufs=2))
        mm_ps = ctx.enter_context(tc.tile_pool(name="mmps", bufs=1, space="PSUM"))
        st_ps = ctx.enter_context(tc.tile_pool(name="stps", bufs=2, space="PSUM"))
        av_ps = ctx.enter_context(tc.tile_pool(name="avps", bufs=2, space="PSUM"))
        sums_ps = ctx.enter_context(tc.tile_pool(name="sums", bufs=1, space="PSUM"))
        identity = const.tile([128, 128], F32, tag="ident")
        ident_bf = const.tile([128, 128], BF16, tag="identbf")
        ones_bf = const.tile([128, 1], BF16, tag="ones_bf")

        def make_consts():
            # deferred so the weight SWDGE generation isn't queued behind
            # make_identity on the Pool engine at t=0; the identity is only
            # needed by the (late) attnT transposes
            make_identity(nc, identity[:])
            nc.vector.tensor_copy(out=ident_bf[:], in_=identity[:])
            nc.vector.memset(ones_bf[:], 1.0)
        if with_bias:
            ones_f = const.tile([128, 64], F32, tag="ones_f")
            nc.vector.memset(ones_f[:], 1.0)
            b_sb = const.tile([1, 3 * HPC * HD], F32R, tag="bias")
            nc.sync.dma_start(out=b_sb[:], in_=bqkv.bitcast(F32R)[:])
            ones_row = const.tile([1, TG], F32R, tag="ones_row")
            nc.vector.tensor_copy(
                out=ones_row[:], in_=ones_f[0:1, 0:1].broadcast_to([1, TG]))

        # softmax sums: one persistent bank; cols = parity*32 + pair*8 + h*4+s
        sums = sums_ps.tile([128, 64], F32, tag="sums")

        # resident weights (fp8 hi/lo pairs); w_out pair-packed: pair p rows
        # [128p, 128p+128). The DMAs are deferred until after group 0's x
        # loads so the first transposes aren't queued behind the weight
        # traffic.
        wh_sb = wpool.tile([128, CH, 3 * HPC * HD], FP8, tag="wqh")
        wl_sb = wpool.tile([128, CH, 3 * HPC * HD], FP8, tag="wql")
        # wout fp8 hi/lo, chunk-PAIR packed for DoubleRow: tile cp holds rows
        # [256cp, 256cp+256) as [128, 2, D]
        wouth_sb = [wpool.tile([128, 2, D], FP8, tag=f"wouth{p}",
                               name=f"wouth{p}") for p in range(2)]
        woutl_sb = [wpool.tile([128, 2, D], FP8, tag=f"woutl{p}",
                               name=f"woutl{p}") for p in range(2)]

        def load_weights():
            # The cost model serializes ALL transfers through one DMA_ENGINES
            # lane (~bytes/360GB/s each), fed by two descriptor-gen lanes:
            # HWDGE (shared SP+ACT, 625ns/dma) and SWDGE (gpsimd, ~1.1us/dma).
            # The prologue is transfer-order-bound and the PE is in-order, so
            # transfers are issued in exact consumption order of the prologue
            # units (q chains h0/h1, k chains h0/h1, then V), with q/k weight
            # columns split so q unblocks first. All on the scalar HWDGE
            # queue; x rides the sync HWDGE queue; later-group x and wout
            # ride SWDGE.
            # q|k lo chunks 0-3 ride the otherwise-idle SWDGE lane (ahead
            # of the gate) so the first chains' hi*lo terms land earlier and
            # the HWDGE stream shortens
            nc.gpsimd.dma_start(
                out=wl_sb[:, 0:4, 0:1024],
                in_=wql[0:512, 0:1024].rearrange("(c p) n -> p c n", p=128))
            for c0 in (0, 4):
                for w_sb, wsrc in (((wh_sb, wqh),) if c0 == 0 else
                                   ((wh_sb, wqh), (wl_sb, wql))):
                    nc.scalar.dma_start(
                        out=w_sb[:, c0:c0 + 4, 0:1024],
                        in_=wsrc[128 * c0:128 * (c0 + 4), 0:1024].rearrange(
                            "(c p) n -> p c n", p=128))
            for w_sb, wsrc in ((wh_sb, wqh), (wl_sb, wql)):
                for c0 in range(0, CH, 4):
                    nc.scalar.dma_start(
                        out=w_sb[:, c0:c0 + 4, 1024:1536],
                        in_=wsrc[128 * c0:128 * (c0 + 4), 1024:1536].rearrange(
                            "(c p) n -> p c n", p=128))

        def load_wout():
            # deferred into the phase-0 fill (behind the group-1 x loads on
            # SWDGE): first consumed by the deferred group-0 out-projections
            # in phase 1 (~45us in)
            with tc.tile_wait_until(0.016):
                for cp in range(2):
                    for w_sb, wsrc in ((wouth_sb, wouth), (woutl_sb, woutl)):
                        nc.gpsimd.dma_start(
                            out=w_sb[cp][:],
                            in_=wsrc[256 * cp:256 * (cp + 1), :].rearrange(
                                "(c p) n -> p c n", p=128))
        # kT per pair, head 2p on partitions [0:64), head 2p+1 on [64:128)
        kts = [wpool.tile([128, seq_len], BF16, tag=f"kt{p}", name=f"kt{p}")
               for p in range(PAIRS)]
        # V in bf16: per (head, k-tile) a [128, 64] stationary block
        v1 = wpool.tile([128, HPC, n_tt, HD], BF16, tag="v1")

        qts_of = {}  # g -> [qt tiles per pair]

        # psum scratch for the projection/transpose/oproj chains. In steady
        # state only the mm bank is free; in the prologue and final drain the
        # scores/AV banks are idle, so rotate through them too (the st slot is
        # [128,2,512]; its first bank is used as a [128,512] scratch).
        # "banked" mode (prologue only, F32) additionally uses the st slots'
        # SECOND psum banks (zero regions are 2KB, so the two banks of one
        # slot hold independent accumulation groups), giving 7 concurrent
        # scratch accumulators for the q+k chain overlap.
        _ps_state = {"wide": False, "banked": False, "i": 0, "st": {}}

        def set_wide_scratch(wide, banked=False):
            _ps_state["wide"] = wide
            _ps_state["banked"] = banked

        def scratch_ps(dtype):
            if not _ps_state["wide"]:
                return mm_ps.tile([128, 512], dtype, tag="mm", name="mm")
            if _ps_state["banked"] and dtype == F32:
                i = _ps_state["i"] = (_ps_state["i"] + 1) % 7
                if i == 0:
                    return mm_ps.tile([128, 512], dtype, tag="mm", name="mm")
                if i in (1, 2, 3, 4):
                    buf, bank = divmod(i - 1, 2)
                    if bank == 0:
                        _ps_state["st"][buf] = st_ps.tile(
                            [128, 2, 512], dtype, tag="st", name="stx")
                    return _ps_state["st"][buf][:, bank, :]
                return av_ps.tile([128, 512], dtype, tag="av", name="avx")
            i = _ps_state["i"] = (_ps_state["i"] + 1) % 5
            if i == 0:
                return mm_ps.tile([128, 512], dtype, tag="mm", name="mm")
            if i in (1, 2):
                return st_ps.tile([128, 512], dtype, tag="st", name="stx")
            return av_ps.tile([128, 512], dtype, tag="av", name="avx")

        def transpose_units(g):
            """x arrives pre-transposed and fp8-hi/lo-split from the host:
            just DMA the group's xT columns. Group 0 (prologue, latency-
            critical) goes on the sync HWDGE queue split in half-chunks;
            later groups ride the then-idle SWDGE ring as one transfer per
            hi/lo half."""
            xt = (xt_pool.tile([128, CH, TG], FP8, tag="xth", name=f"xth{g}"),
                  xt_pool.tile([128, CH, TG], FP8, tag="xtl", name=f"xtl{g}"))
            gsl = slice(g * TG, (g + 1) * TG)

            def u():
                if g == 0:
                    # lo directly after hi per chunk-half: every chain's
                    # third (lo*hi) term needs x-lo early
                    for c0 in (0, 4):
                        for t, src in ((xt[0], xh), (xt[1], xl)):
                            nc.sync.dma_start(
                                out=t[:, c0:c0 + 4, :],
                                in_=src[128 * c0:128 * (c0 + 4), gsl].rearrange(
                                    "(c p) n -> p c n", p=128))
                else:
                    # wait-ts biases the tile scheduler so these preps don't
                    # jump the serial DMA lane ahead of the prologue-critical
                    # weight transfers
                    with tc.tile_wait_until(0.010 + 0.038 * (g - 1)):
                        for t, src in ((xt[0], xh), (xt[1], xl)):
                            nc.gpsimd.dma_start(
                                out=t[:, :, :],
                                in_=src[:, gsl].rearrange(
                                    "(c p) n -> p c n", p=128))
            return xt, [u]

        def qkv_units(g, xt):
            """12 units: 4 qt chains, 4 kT chains, 4 V chains."""
            qts = qts_of.setdefault(g, [])
            units = []

            half = {}

            xth, xtl = xt

            def qk_chain(p, qk, h):
                # split into two half-chains (finer interleave granularity).
                # Each half contracts 4 D-chunks = 2 fp8 DoubleRow chunk-pairs
                # x 3 hi/lo cross terms (lo*lo dropped). Terms are ordered
                # hi*hi first so the prologue chains can start as soon as the
                # hi transfers land (lo halves trickle in later).
                if h == 0:
                    half[(p, qk)] = scratch_ps(F32)
                ps = half[(p, qk)]
                col = qk * 512 + p * 128
                first = True
                for wt, xt_ in ((wh_sb, xth), (wl_sb, xth), (wh_sb, xtl)):
                    for c0 in range(4 * h, 4 * h + 4, 2):
                        cs = slice(c0, c0 + 2)
                        nc.tensor.matmul(
                            ps[:, :TG], wt[:, cs, col:col + 128],
                            xt_[:, cs, :],
                            start=(h == 0 and first),
                            stop=(h == 1 and c0 == CH - 2 and xt_ is xtl
                                  and not with_bias),
                            perf_mode=DR)
                        first = False
                if h == 0:
                    return
                if with_bias:
                    nc.tensor.matmul(
                        ps[:, :TG], b_sb[0:1, col:col + 128],
                        ones_row[0:1, :], start=False, stop=True)
                if qk == 0:
                    qt = qt_pool.tile([128, TG], BF16, tag="qt")
                    nc.vector.tensor_copy(out=qt[:], in_=ps[:, :TG])
                    qts.append(qt)
                else:
                    nc.vector.tensor_copy(
                        out=kts[p][:, g * TG:(g + 1) * TG], in_=ps[:, :TG])

            def v_chain(t4):
                tt = g * (TG // 128) + t4
                ps = scratch_ps(F32)
                first = True
                tsl = slice(128 * t4, 128 * (t4 + 1))
                for xt_, wt in ((xth, wh_sb), (xth, wl_sb), (xtl, wh_sb)):
                    for c0 in range(0, CH, 2):
                        cs = slice(c0, c0 + 2)
                        nc.tensor.matmul(
                            ps[:, :512], xt_[:, cs, tsl],
                            wt[:, cs, 1024:1536],
                            start=first,
                            stop=(c0 == CH - 2 and xt_ is xtl
                                  and not with_bias),
                            perf_mode=DR)
                        first = False
                if with_bias:
                    nc.tensor.matmul(
                        ps[:, :512], ones_row[0:1, 0:128],
                        b_sb[0:1, 1024:1536], start=False, stop=True)
                nc.vector.tensor_copy(
                    out=v1[:, :, tt, :],
                    in_=ps[:, :512].rearrange("p (h d) -> p h d", h=HPC))

            k_units = []
            v_units = []
            # in the prologue the qk chains are gated on their W chunks
            # landing (~1us apart): run all h=0 half-chains (chunks 0-3)
            # before any h=1 (chunks 4-7). Requires 4 concurrent psum
            # accumulators - only legal in the wide-scratch prologue.
            hmajor = _ps_state["wide"]
            for lst, qk in ((units, 0), (k_units, 1)):
                if hmajor:
                    for h in range(2):
                        for p in range(PAIRS):
                            lst.append(lambda p=p, h=h, qk=qk: qk_chain(p, qk, h))
                else:
                    for p in range(PAIRS):
                        lst.append(lambda p=p, qk=qk: qk_chain(p, qk, 0))
                        lst.append(lambda p=p, qk=qk: qk_chain(p, qk, 1))
            for t4 in range(TG // 128):
                v_units.append(lambda t4=t4: v_chain(t4))
            return units, k_units, v_units

        def attention_units(g):
            """Per pair: one unit per k-tile (sT+exp+mask, AV carried by one),
            then a normalization unit; finally the out-projection units."""
            units = []
            pending_finish = []
            an_tiles = [an_pool.tile([128, 512], BF16, tag="an",
                                     name=f"an_g{g}_s{s}")
                        for s in range(TG // 128)]
            # zero this group's sums columns once (all 4 pairs' 8-col slices)
            nc.vector.memset(sums[:, (g % 2) * 32:(g % 2) * 32 + 32], 0.0)
            qts = qts_of[g]
            for p in range(PAIRS):
                nkt = 4 * (g + 1)
                soff = (g % 2) * 32 + p * 8
                state = {}

                def start_pair(p=p, state=state, soff=soff):
                    state["av"] = av_ps.tile([128, 512], F32, tag="av",
                                             name=f"av_g{g}_p{p}")
                    state["carry"] = []
                    state["first"] = True

                def kt_unit(kt, pos, p=p, state=state, nkt=nkt,
                            sp=start_pair):
                    if pos == 0:
                        sp()
                    # pop the carried AV FIRST: it is ready now, and the
                    # scores below may head-block on a busy st slot
                    if len(state["carry"]) >= 9:
                        state["emit_av"](*state["carry"].pop(0))
                    qt = qts[p]
                    rdiag = kt - 4 * g
                    col0 = 128 * rdiag if rdiag > 0 else 0
                    ksl = slice(128 * kt, 128 * (kt + 1))
                    st = st_ps.tile([128, 2, 512], F32, tag="st")
                    nc.tensor.matmul(
                        st[:, 0, col0:], kts[p][0:64, ksl], qt[0:64, col0:])
                    nc.tensor.matmul(
                        st[:, 1, col0:], kts[p][64:128, ksl], qt[64:128, col0:])
                    pt = pt_pool.tile([128, 2, 512], BF16, tag="pt")
                    nc.scalar.activation(
                        pt[:, :, col0:], st[:, :, col0:], AF.Exp,
                        scale=SCALE / (WSCALE * WSCALE))
                    if rdiag >= 0:
                        # keep P^T[k, q] only where q >= k (within-block);
                        # one select covers both heads via a stride-0 dim
                        nc.gpsimd.affine_select(
                            out=pt[:, :, col0:col0 + 128],
                            in_=pt[:, :, col0:col0 + 128],
                            compare_op=mybir.AluOpType.is_ge,
                            fill=0.0, base=0, pattern=[[0, 2], [1, 128]],
                            channel_multiplier=-1)
                    state["carry"].append((kt, pos, rdiag, pt))

                def emit_av(kt, pos, rdiag, pt, p=p, state=state, nkt=nkt,
                            soff=soff):
                    av = state["av"]
                    s0 = max(rdiag, 0)
                    last = pos == nkt - 1
                    for s in range(s0, TG // 128):
                        for h in (0, 1):
                            nc.tensor.matmul(
                                av[:, 128 * s + 64 * h:128 * s + 64 * h + 64],
                                pt[:, h, 128 * s:128 * (s + 1)],
                                v1[:, 2 * p + h, kt, :],
                                start=state["first"],
                                stop=(last and s == 3 and h == 1))
                            state["first"] = False
                            nc.tensor.matmul(
                                sums[:, soff + 4 * h + s:soff + 4 * h + s + 1],
                                pt[:, h, 128 * s:128 * (s + 1)],
                                ones_bf[:],
                                start=False, stop=False,
                                skip_group_check=True)

                state["emit_av"] = emit_av

                def emit_block(mm, state, p, soff, stop_last):
                    av = state["av"]
                    for i, (isd, kt, pt, s, h) in enumerate(mm):
                        nc.tensor.matmul(
                            av[:, 128 * s + 64 * h:128 * s + 64 * h + 64],
                            pt[:, h, 128 * s:128 * (s + 1)],
                            v1[:, 2 * p + h, kt, :],
                            start=state["first"],
                            stop=(stop_last and i == len(mm) - 1))
                        state["first"] = False
                        nc.tensor.matmul(
                            sums[:, soff + 4 * h + s:soff + 4 * h + s + 1],
                            pt[:, h, 128 * s:128 * (s + 1)],
                            ones_bf[:],
                            start=False, stop=False,
                            skip_group_check=True)

                def flush_unit(p=p, state=state, soff=soff):
                    # flush the carried non-diagonal AVs at pair end; the
                    # affine-masked diagonal blocks wait for Pool latency, so
                    # they are deferred into finish_unit (emitted two k-tile
                    # units into the NEXT pair) to avoid head-blocking the
                    # PE's 4-deep dependency wait queue
                    mm = []
                    for kt, pos, rdiag, pt in state["carry"]:
                        for s in range(max(rdiag, 0), TG // 128):
                            for h in (0, 1):
                                mm.append((s == rdiag, kt, pt, s, h))
                    state["carry"] = []
                    mm.sort(key=lambda t: t[0])
                    ndiag = sum(1 for t in mm if t[0])
                    split = len(mm) - ndiag
                    emit_block(mm[:split], state, p, soff, stop_last=False)
                    state["diag"] = mm[split:]

                def norm_unit(p=p, state=state, soff=soff):
                    emit_block(state.pop("diag"), state, p, soff,
                               stop_last=True)
                    av = state["av"]
                    rc = rc_pool.tile([128, 2, 4], F32, tag="rc")
                    nc.vector.reciprocal(
                        rc[:], sums[:, soff:soff + 8].rearrange(
                            "p (h s) -> p h s", h=2))
                    for s in range(TG // 128):
                        nc.vector.tensor_mul(
                            an_tiles[s][:, 128 * p:128 * (p + 1)].rearrange(
                                "p (h d) -> p h d", h=2),
                            av[:, 128 * s:128 * (s + 1)].rearrange(
                                "p (h d) -> p h d", h=2),
                            rc[:, :, s:s + 1].broadcast_to([128, 2, 64]))

                pair_units = []
                kt_order = list(range(nkt))
                if g == n_tg - 1 and p == PAIRS - 1:
                    # last pair of the last group: diagonal k-tiles first so
                    # the wind-down tail has no Pool(affine) round-trip on
                    # its critical path
                    kt_order = kt_order[4 * g:] + kt_order[:4 * g]
                for pos, kt in enumerate(kt_order):
                    pair_units.append(
                        lambda kt=kt, pos=pos, f=kt_unit: f(kt, pos))
                if pending_finish:
                    pair_units.insert(min(6, max(2, len(pair_units) - 2)),
                                      pending_finish.pop())
                units += pair_units
                units.append(flush_unit)
                pending_finish.append(norm_unit)

            if pending_finish:
                units.append(pending_finish.pop())

            ats = {}

            def trans_unit(s):
                ps = scratch_ps(BF16)
                for c in range(PAIRS):
                    nc.tensor.transpose(
                        ps[:, 128 * c:128 * (c + 1)],
                        an_tiles[s][:, 128 * c:128 * (c + 1)],
                        ident_bf[:])
                # fp8 hi/lo split of attnT for the DoubleRow out-projection.
                # The hi copy runs on the (mostly idle) Pool engine so the
                # DVE isn't the serial bottleneck of the projection drain.
                # stage the (already-bf16) psum to SBUF in ONE read so the
                # scratch bank's WAR window is a single copy, then quantize
                # hi/lo from SBUF off the psum critical path
                atb = at_pool.tile([128, 512], BF16, tag="atb")
                ath = at_pool.tile([128, 512], FP8, tag="ath")
                atl = at_pool.tile([128, 512], FP8, tag="atl")
                nc.vector.tensor_copy(out=atb[:], in_=ps[:])
                if _ps_state["wide"]:
                    # final drain: ACT is idle and DVE is the serial
                    # bottleneck, so split the quant pair across both
                    nc.scalar.copy(ath[:], atb[:])
                else:
                    nc.vector.tensor_copy(out=ath[:], in_=atb[:])
                nc.vector.tensor_sub(atl[:], atb[:], ath[:])
                ats[s] = (ath, atl)

            def oproj_unit(s, nh):
                row0 = g * TG + 128 * s
                ath, atl = ats[s]
                ps = scratch_ps(F32)
                nsl = slice(512 * nh, 512 * (nh + 1))
                first = True
                for at_, wo in ((ath, wouth_sb), (ath, woutl_sb),
                                (atl, wouth_sb)):
                    for cp in range(2):
                        nc.tensor.matmul(
                            ps[:, :512],
                            at_[:, 256 * cp:256 * (cp + 1)].rearrange(
                                "p (c m) -> p c m", c=2),
                            wo[cp][:, :, nsl],
                            start=first,
                            stop=(at_ is atl and cp == 1),
                            perf_mode=DR)
                        first = False
                ob = ob_pool.tile([128, 512], BF16, tag="ob")
                if _ps_state["wide"] and (s + nh) % 2 == 0:
                    nc.scalar.copy(ob[:], ps[:, :512])
                else:
                    nc.vector.tensor_copy(out=ob[:], in_=ps[:, :512])
                # in the final drain the sync HWDGE queue serializes the last
                # 8 output stores (~625ns descriptor-gen each); spread them
                # over the idle queues so the tail isn't gated on it
                eng = (nc.sync if not _ps_state["wide"] else
                       (nc.sync, nc.scalar, nc.gpsimd)[(3 - s + nh) % 3])
                eng.dma_start(
                    out=out[row0:row0 + 128, 512 * nh:512 * (nh + 1)],
                    in_=ob[:])

            ounits = []
            for s in range(TG // 128):
                ounits.append(lambda s=s: trans_unit(s))
            for s in range(TG // 128):
                for nh in range(2):
                    ounits.append(lambda s=s, nh=nh: oproj_unit(s, nh))
            return units, ounits

        def interleave(a_units, b_units, pre_b=0):
            # pre_b: emit that many b-units before any a-unit (phase 0 needs
            # the group-0 V chains emitted before the first AV flush so the
            # tile framework sees the writes first)
            for u in b_units[:pre_b]:
                u()
            b_units = b_units[pre_b:]
            na, nb = len(a_units), len(b_units)
            ia = ib = 0
            while ia < na or ib < nb:
                fa = (na - ia) / na if na else 0.0
                fb = (nb - ib) / nb if nb else 0.0
                if ia < na and (fa > fb or ib >= nb):
                    a_units[ia]()
                    ia += 1
                else:
                    b_units[ib]()
                    ib += 1

        # prologue: group 0 projection (weight DMAs after group 0's x loads).
        # The scores/AV banks are idle here, so scratch rotates through them.
        set_wide_scratch(True, banked=True)
        # fill the initial x/w DMA latency with dummy PE work (also completes
        # the tensor engine's p-state ramp before real work lands); plain
        # matmuls on a memset tile need no identity, so they start ~1us in
        dummy = const.tile([128, 256], BF16, tag="dummy")
        nc.vector.memset(dummy[:], 0.0)
        xt0, tunits0 = transpose_units(0)
        for u in tunits0:
            u()
        load_weights()
        make_consts()
        # SWDGE gate: the gpsimd descriptor ring would otherwise eagerly
        # prepare the later-group x / wout transfers at t~1us and their
        # transfers would jump the (serial) DMA lane ahead of the
        # prologue-critical weight transfers. A tiny Pool-engine copy that
        # waits on the first x tile holds the ring back until the prologue
        # transfers are in flight.
        gate = const.tile([1, 1], FP8, tag="gate")
        nc.gpsimd.tensor_copy(out=gate[:], in_=xt0[1][0:1, CH - 1, 0:1])
        for _ in range(NDUMMY):
            wps = scratch_ps(F32)
            nc.tensor.matmul(wps[:, :256], dummy[:, 0:128], dummy[:],
                             start=True, stop=True)
        # prologue runs only the q and k chains (their weights land first);
        # group 0's V chains are deferred into the phase-0 fill, where they
        # interleave with the (V-independent) score units while the V-column
        # weights land. Unit order follows the DMA landing order: all h=0
        # halves (chunks 0-3) before h=1 (chunks 4-7). This holds up to 7
        # open psum accumulations (4 q + 3 k), which the banked scratch
        # rotation provides; k p3 waits until the q psums close.
        _ps_state["i"] = 0
        q0, k0, v0 = qkv_units(0, xt0)
        for u in [q0[0], q0[1], q0[2], q0[3],      # q h0 p0-3
                  k0[0], k0[1], k0[2],             # k h0 p0-2
                  q0[4], q0[5], q0[6], q0[7],      # q h1 p0-3 (close q psums)
                  k0[3],                           # k h0 p3
                  k0[4], k0[5], k0[6], k0[7]]:     # k h1 p0-3
            u()
        set_wide_scratch(False)
        # steady state: attention(g) interleaved with transposes(g+1) +
        # projection(g+1); out-projections are deferred up to two groups so
        # the last (largest) attention group still has dense PE fill
        pending_oproj = []  # deferred out-projection unit lists, oldest first
        fill_carry = v0     # group g's V chains lead the phase-g fill: their
        # only consumer is the AV flush (safely late in emission), and the
        # ACT-bound later phases need the q/k fill to shrink, not grow
        for g in range(n_tg):
            attn, ounits = attention_units(g)
            fill = fill_carry
            fill_carry = []
            if g + 1 < n_tg:
                xt1, tunits = transpose_units(g + 1)
                qu, ku, vu = qkv_units(g + 1, xt1)
                fill += tunits + ([load_wout] if g == 0 else []) + qu + ku
                fill_carry = vu
            if False and pending_oproj:
                # phase 2 is ACT(exp)-bound with PE slack: give it the
                # oldest deferred out-projection group
                fill += pending_oproj.pop(0)
            if g + 1 == n_tg:
                if g == 0:
                    fill.append(load_wout)
                # last group is exp(ACT)-bound and has no next-group
                # projection: feed it the remaining deferred out-projections,
                # round-robin across groups so each projection trails its
                # transpose's fp8-quantize chain by 3x more slack
                lists = [list(o) for o in pending_oproj]
                pending_oproj.clear()
                while any(lists):
                    for o in lists:
                        if o:
                            fill.append(o.pop(0))
            interleave(attn, fill, pre_b=len(v0) if g == 0 else 0)
            pending_oproj.append(ounits)
        # final drain: attention is done, scores/AV banks are idle again
        set_wide_scratch(True)
        for ou in pending_oproj:
            for u in ou:
                u()


def build_program(with_bias, seq_len=T):
    nc = bacc.Bacc("TRN2", target_bir_lowering=False, debug=False,
                   enable_asserts=False, num_devices=8)
    xh = nc.dram_tensor("xh", [D, seq_len], FP8, kind="ExternalInput").ap()
    xl = nc.dram_tensor("xl", [D, seq_len], FP8, kind="ExternalInput").ap()
    wqh = nc.dram_tensor("wqh", [D, 3 * HPC * HD], FP8,
                         kind="ExternalInput").ap()
    wql = nc.dram_tensor("wql", [D, 3 * HPC * HD], FP8,
                         kind="ExternalInput").ap()
    wouth = nc.dram_tensor("wouth", [HPC * HD, D], FP8,
                           kind="ExternalInput").ap()
    woutl = nc.dram_tensor("woutl", [HPC * HD, D], FP8,
                           kind="ExternalInput").ap()
    out = nc.dram_tensor("out", [seq_len, D], BF16,
                         kind="ExternalOutput").ap()
    bqkv = None
    if with_bias:
        bqkv = nc.dram_tensor("bqkv", [1, 3 * HPC * HD], F32,
                              kind="ExternalInput").ap()
    with tile.TileContext(nc) as tc:
        build_tile_program(tc, xh, xl, wqh, wql, wouth, woutl, out, bqkv,
                           seq_len=seq_len)
    nc.compile()
    return nc


_PROGRAM_CACHE = {}


def _get_program(with_bias):
    if with_bias not in _PROGRAM_CACHE:
        _PROGRAM_CACHE[with_bias] = build_program(with_bias)
    return _PROGRAM_CACHE[with_bias]


def _split8(a):
    """fp8(e4m3) hi/lo split: a ~= hi + lo to ~7 mantissa bits."""
    import ml_dtypes
    hi = a.astype(ml_dtypes.float8_e4m3)
    lo = (a - hi.astype(np.float32)).astype(ml_dtypes.float8_e4m3)
    return hi, lo


def make_core_inputs(x_core, wqkv_core, wout_core, b_core=None):
    """Quantized inputs for ONE core: x_core [T, D], wqkv_core [D, 1536]
    (q|k|v grouped), wout_core [512, D]."""
    xh, xl = _split8(np.ascontiguousarray(x_core.T))
    wqh, wql = _split8(np.ascontiguousarray(wqkv_core) * WSCALE)
    wouth, woutl = _split8(np.ascontiguousarray(wout_core) * WSCALE)
    m = {
        "xh": xh, "xl": xl, "wqh": wqh, "wql": wql,
        "wouth": wouth, "woutl": woutl,
    }
    if b_core is not None:
        m["bqkv"] = np.ascontiguousarray(b_core * WSCALE).reshape(
            1, -1).astype(np.float32)
    return m


def make_in_maps(x, w_qkv, b_qkv, w_out, with_bias):
    """Per-core input dicts: core c -> batch c//2, head group c%2."""
    in_maps = []
    for core in range(8):
        b, gr = divmod(core, 2)
        qc = slice(512 * gr, 512 * (gr + 1))
        kc = slice(D + 512 * gr, D + 512 * (gr + 1))
        vc = slice(2 * D + 512 * gr, 2 * D + 512 * (gr + 1))
        wq = np.concatenate([w_qkv[:, qc], w_qkv[:, kc], w_qkv[:, vc]], axis=1)
        bq = (np.concatenate([b_qkv[qc], b_qkv[kc], b_qkv[vc]])
              if with_bias else None)
        in_maps.append(make_core_inputs(
            x[b], wq, w_out[512 * gr:512 * (gr + 1), :], bq))
    return in_maps


def kernel(x, w_qkv, b_qkv, w_out, b_out):
    x = np.asarray(x, dtype=np.float32)
    w_qkv = np.asarray(w_qkv, dtype=np.float32)
    b_qkv = np.asarray(b_qkv, dtype=np.float32)
    w_out = np.asarray(w_out, dtype=np.float32)
    b_out = np.asarray(b_out, dtype=np.float32)

    with_bias = bool(np.any(b_qkv))
    nc = _get_program(with_bias)
    in_maps = make_in_maps(x, w_qkv, b_qkv, w_out, with_bias)
    res = run_bass_kernel_spmd(nc, in_maps, core_ids=list(range(8))).results

    out = np.empty((B, T, D), dtype=np.float32)
    for b in range(B):
        out[b] = ((res[2 * b]["out"].astype(np.float32)
                   + res[2 * b + 1]["out"].astype(np.float32)) * OUT_SCALE
                  + b_out[None, :])
    return out

